# revision 6
# baseline (speedup 1.0000x reference)
"""DiffAttention Trainium2 kernel, 8-core SPMD (head-parallel), v2.

Problem (hardcoded): B=2, S=2048, D=128, H=8.
  q = (x@Wq.T+bq).reshape(B,H,S,2D)   # raw reshape: head h <-> rows [256h,256h+256) of proj
  s1 = q1@k1.T; s2 = q2@k2.T; attn = softmax(s1) - lam*softmax(s2)
  out = attn@v -> transpose/reshape -> GroupNorm(H groups) -> *(1-lam) -> concat heads -> @Wo.T+bo

Sharding: core c owns head h=c for both batches (2 units/core). GroupNorm groups
mix all heads -> tiny (32-float) AllGather of partial stats.

Index algebra per unit (b,h), block = proj rows [256h, 256h+256):
  sigma (attn row) = 8r+j, r in [0,256), j in [0,8). We use tau-order sigma' = 256j+r.
  q1T[d, sigma'=256j+r] = qpT_block[f=256j+d, r]   (even 128-col chunks of qp block)
  q2T: odd chunks.  v'[sigma'=256j+r, d] = vp_block[r, 128j+d].
  GroupNorm group g = {sigma': (sigma' mod 256)//32 == g} (32-wide strips).
  Final rows: out[b, 8*rho+h, 128h3+d] = GN(O)[b,h][sigma'=256(rho%8)+32h3+rho//8, d]

v2 changes vs v1:
  - softmax denominators via [128q,1]-output dot matmuls (nearly free on PE)
    + PE transposes into a [1,1024] psum row + DVE reciprocal + gpsimd
    partition_broadcast, replacing full-width ones-matmul accumulations.
  - exp on [128,2048] tiles (half the ACT instruction overhead).
  - bf16 V / E / fT / Wo (output matmuls 4x cheaper); q/k stay f32r.
  - output-stage partials read PSUM directly; collectives scheduled so the
    Pool-queue block lands where PE has a queued qb of slack.
"""

import sys

sys.path.insert(0, "/opt/trn_rl_repo")

import numpy as np

import concourse.bass as bass
import concourse.bacc as bacc
import concourse.mybir as mybir
import concourse.tile as tile

F32 = mybir.dt.float32
F32R = mybir.dt.float32r
BF16 = mybir.dt.bfloat16
AF = mybir.ActivationFunctionType
ALU = mybir.AluOpType

B, S, D, H = 2, 2048, 128, 8
N_CORES = 8
EPS = 1e-5
GROUP_N = float(256 * H * D)  # elements per GroupNorm group

_CACHED = None


def build_nc():
    nc = bacc.Bacc("TRN2", target_bir_lowering=False, debug=False, num_devices=N_CORES)

    # ---- per-core external I/O ----
    qT = nc.dram_tensor("qT", [B, 128, 256], F32, kind="ExternalInput")  # query block.T per batch
    wqT = nc.dram_tensor("wqT", [128, 2048], F32, kind="ExternalInput")
    wkT = nc.dram_tensor("wkT", [128, 2048], F32, kind="ExternalInput")
    wvT = nc.dram_tensor("wvT", [128, 1024], F32, kind="ExternalInput")
    woT = nc.dram_tensor("woT", [1024, 128], F32, kind="ExternalInput")
    bqT = nc.dram_tensor("bqT", [128, 16], F32, kind="ExternalInput")
    bkT = nc.dram_tensor("bkT", [128, 16], F32, kind="ExternalInput")
    bv = nc.dram_tensor("bv", [1, 1024], F32, kind="ExternalInput")
    bo = nc.dram_tensor("bo", [1, 128], F32, kind="ExternalInput")
    gnw2 = nc.dram_tensor("gnw2", [1, 16], F32, kind="ExternalInput")  # tiled x2 (b,g)
    gnb2 = nc.dram_tensor("gnb2", [1, 16], F32, kind="ExternalInput")
    lam = nc.dram_tensor("lam", [1, 1], F32, kind="ExternalInput")
    eye = nc.dram_tensor("eye", [128, 128], F32, kind="ExternalInput")
    outp = nc.dram_tensor("outp", [B, 256, 128], F32, kind="ExternalOutput")

    with tile.TileContext(nc) as tc:
        with (
            tc.tile_pool(name="const", bufs=1) as cpool,
            tc.tile_pool(name="proj", bufs=2) as projpool,
            tc.tile_pool(name="epool", bufs=4) as epool,
            tc.tile_pool(name="otpool", bufs=4) as otpool,
            tc.tile_pool(name="tmp", bufs=2) as tmppool,
            tc.tile_pool(name="dram", bufs=1, space="DRAM") as dram,
        ):
            # ---- load constants / weights (qT first: projections need it) ----
            qt_sb = []
            for u in range(B):
                q = cpool.tile([128, 256], F32, name=f"qt_sb{u}")
                nc.sync.dma_start(q[:], qT[u])
                qt_sb.append(q)

            # small constants go on the gpsimd DMA queue so they don't delay
            # the big weight DMAs on the sync queue
            bq_sb = cpool.tile([128, 16], F32)
            bk_sb = cpool.tile([128, 16], F32)
            nc.gpsimd.dma_start(bq_sb[:], bqT[:])
            nc.gpsimd.dma_start(bk_sb[:], bkT[:])
            bv_sb = cpool.tile([1, 1024], F32)
            nc.gpsimd.dma_start(bv_sb[:], bv[:])
            bo_sb = cpool.tile([1, 128], F32)
            nc.gpsimd.dma_start(bo_sb[:], bo[:])
            gnw_sb = cpool.tile([1, 16], F32)
            gnb_sb = cpool.tile([1, 16], F32)
            nc.gpsimd.dma_start(gnw_sb[:], gnw2[:])
            nc.gpsimd.dma_start(gnb_sb[:], gnb2[:])
            lam_sb = cpool.tile([1, 1], F32)
            nc.gpsimd.dma_start(lam_sb[:], lam[:])
            eye_sb = cpool.tile([128, 128], F32)
            nc.gpsimd.dma_start(eye_sb[:], eye[:])

            # weights loaded and f32r-rounded in 1024-col pieces so projections
            # can start before all input DMA completes.
            wq_rh, wk_rh = [], []
            wv_r = cpool.tile([128, 1024], F32R)
            wpieces = (
                [("wq", wqT, wq_rh, 0), ("wk", wkT, wk_rh, 0),
                 ("wk", wkT, wk_rh, 1), ("wv", wvT, None, 0),
                 ("wq", wqT, wq_rh, 1)]
            )
            for (wnm, dram_w, lst, half) in wpieces:
                wsc = projpool.tile([128, 1024], F32, tag="wsc", name=f"wsc_{wnm}{half}")
                nc.sync.dma_start(wsc[:], dram_w[:, 1024 * half: 1024 * (half + 1)])
                if lst is None:
                    nc.vector.tensor_copy(wv_r[:], wsc[:])
                else:
                    wr = cpool.tile([128, 1024], F32R, name=f"{wnm}_r{half}")
                    nc.vector.tensor_copy(wr[:], wsc[:])
                    lst.append(wr)
            qt_r = []
            for u in range(B):
                qr = cpool.tile([128, 256], F32R, name=f"qt_r{u}")
                nc.vector.tensor_copy(qr[:], qt_sb[u][:])
                qt_r.append(qr)
            lam_rep = cpool.tile([128, 1], F32)
            nc.gpsimd.partition_broadcast(lam_rep[:], lam_sb[:])
            oml = cpool.tile([1, 1], F32)
            nc.vector.tensor_scalar(oml[:], lam_sb[:], -1.0, 1.0, ALU.mult, ALU.add)
            bv_rep = cpool.tile([128, 1024], F32)
            nc.gpsimd.partition_broadcast(bv_rep[:], bv_sb[:])

            ones_f32 = cpool.tile([128, 1], F32)
            nc.vector.memset(ones_f32[:], 1.0)
            ones2_f32 = cpool.tile([128, 2], F32)
            nc.vector.memset(ones2_f32[:], 1.0)
            ones_col = cpool.tile([128, 2], F32R)
            nc.vector.tensor_copy(ones_col[:], ones2_f32[:])

            # Wo chunks in bf16 (moving operand of the output matmuls);
            # DMAs queued after the projection weights
            wo_bf = []
            for h3 in range(8):
                wsc = projpool.tile([128, 128], F32, tag="wosc", name=f"wosc{h3}")
                nc.sync.dma_start(wsc[:], woT[128 * h3: 128 * (h3 + 1), :])
                w = cpool.tile([128, 128], F32, name=f"wo_bf{h3}")
                nc.vector.tensor_copy(w[:], wsc[:])
                wo_bf.append(w)

            p2_tiles = {0: [], 1: []}
            fT_sb = []
            ot_refs = {0: [], 1: []}  # otq tiles per unit (for deferred re-layout)

            cc_in = [dram.tile([1, 16], F32, name=f"cc_in{u}") for u in range(B)]
            cc_out = [dram.tile([8, 16], F32, addr_space="Shared", name=f"cc_out{u}")
                      for u in range(B)]
            gath = [tmppool.tile([1, 128], F32, tag="gath", name=f"gath_{u}", bufs=2)
                    for u in range(B)]
            scal = [{}, {}]

            def ptree(dst, src_tile, parts, width, nm, eng=None):
                eng = eng or nc.vector
                # partition-axis sum: DVE shift-copy + add down to 32 partitions
                # (TT needs equal base partitions; slices are 32-aligned),
                # then one gpsimd C-axis reduce for the final 32 -> 1.
                cur = src_tile
                while parts > 32:
                    parts //= 2
                    sh = tmppool.tile([parts, width], F32, tag=f"ps{parts}",
                                      name=f"ps_{nm}_{parts}", bufs=4)
                    eng.tensor_copy(sh[:], cur[parts: 2 * parts, :])
                    t = tmppool.tile([parts, width], F32, tag=f"pt{parts}",
                                     name=f"pt_{nm}_{parts}", bufs=4)
                    eng.tensor_tensor(t[:], cur[0:parts, :], sh[:], ALU.add)
                    cur = t
                nc.gpsimd.tensor_reduce(dst, cur[:], mybir.AxisListType.C, ALU.add)

            def emit_stats_export(u):
                stats_u = tmppool.tile([1, 16], F32, tag="stats", name=f"stats_{u}", bufs=2)
                for si, p2 in enumerate(p2_tiles[u]):
                    ptree(stats_u[:, 8 * si: 8 * si + 8], p2, 128, 8, f"st{u}{si}")
                nc.sync.dma_start(cc_in[u][:], stats_u[:])
                nc.gpsimd.collective_compute(
                    "AllGather", ALU.bypass,
                    replica_groups=[list(range(N_CORES))],
                    ins=[cc_in[u][:]], outs=[cc_out[u][:]],
                )
                nc.gpsimd.dma_start(gath[u][:],
                                     cc_out[u][:].rearrange("a b -> (a b)").unsqueeze(0))

            def emit_scalars(u, cb_mm=None):
                # global stats for batch u -> A_rep[128,8], cb_rep[128,128]
                t = lambda nm: tmppool.tile([1, 8], F32, tag=nm, name=f"{nm}_{u}", bufs=2)
                glob = tmppool.tile([1, 16], F32, tag="globsb", name=f"glob_{u}", bufs=2)
                nc.vector.tensor_reduce(
                    glob[:], gath[u].rearrange("p (a b) -> p b a", a=8, b=16),
                    mybir.AxisListType.X, ALU.add,
                )
                moments = tmppool.tile([1, 16], F32, tag="mom", name=f"mom_{u}", bufs=2)
                nc.vector.tensor_scalar_mul(moments[:], glob[:], 1.0 / GROUP_N)
                mean, ex2 = moments[:, 0:8], moments[:, 8:16]
                var, veps = t("var"), t("veps")
                nc.vector.tensor_tensor(var[:], mean, mean, ALU.mult)
                nc.vector.tensor_tensor(var[:], ex2, var[:], ALU.subtract)
                nc.vector.tensor_scalar_add(veps[:], var[:], EPS)
                # rsqrt fully on DVE (ACT Sqrt would thrash the exp table set):
                # quake seed + 2 Newton steps
                I32 = mybir.dt.int32
                ti = tmppool.tile([1, 8], I32, tag="rsqi", name=f"rsqi_{u}", bufs=2)
                nc.vector.tensor_scalar(
                    ti[:], veps[:].bitcast(I32), 1, None, ALU.arith_shift_right
                )
                nc.vector.tensor_scalar(ti[:], ti[:], -1, 0x5F3759DF, ALU.mult, ALU.add)
                rstd, hf, nt = t("rstd"), t("hf"), t("nt")
                nc.vector.tensor_copy(rstd[:], ti[:].bitcast(F32))
                nc.vector.tensor_scalar_mul(hf[:], veps[:], 0.5)
                for _ in range(2):
                    nc.vector.tensor_tensor(nt[:], rstd[:], rstd[:], ALU.mult)
                    nc.vector.tensor_tensor(nt[:], nt[:], hf[:], ALU.mult)
                    nc.vector.tensor_scalar(nt[:], nt[:], -1.0, 1.5, ALU.mult, ALU.add)
                    nc.vector.tensor_tensor(rstd[:], rstd[:], nt[:], ALU.mult)
                A, Bc = t("A"), t("Bc")
                nc.vector.tensor_tensor(A[:], rstd[:], gnw_sb[:, 0:8], ALU.mult)
                nc.vector.tensor_tensor(Bc[:], mean, A[:], ALU.mult)
                nc.vector.tensor_tensor(Bc[:], gnb_sb[:, 0:8], Bc[:], ALU.subtract)
                nc.vector.tensor_scalar_mul(A[:], A[:], oml[:, 0:1])
                nc.vector.tensor_scalar_mul(Bc[:], Bc[:], oml[:, 0:1])
                A_rep = tmppool.tile([128, 8], F32, tag="A_rep", name=f"A_rep{u}", bufs=2)
                nc.gpsimd.partition_broadcast(A_rep[:], A[:])
                cb = tmppool.tile([1, 128], F32, tag="cb", name=f"cb_{u}", bufs=2)
                if cb_mm is None:
                    # serial stt chain (fine off the critical path)
                    nc.vector.tensor_scalar_mul(cb[:], wsum_sb[:, 0:128], Bc[:, 0:1])
                    for h3 in range(1, 8):
                        nc.vector.scalar_tensor_tensor(
                            cb[:], wsum_sb[:, 128 * h3: 128 * (h3 + 1)],
                            Bc[:, h3: h3 + 1], cb[:], ALU.mult, ALU.add,
                        )
                    nc.vector.tensor_tensor(cb[:], cb[:], bo_sb[:], ALU.add)
                else:
                    # critical path: cb = Bc(1x8) @ wsum_p8(8x128) via PE
                    # (transpose Bc to a column first), then + bo
                    ps_pool = cb_mm
                    btp = ps_pool.tile([8, 8], F32, tag="btp", name=f"btp_{u}")
                    nc.tensor.matmul(btp[:, 0:1], Bc[:], ones_f32[0:1, 0:1],
                                     is_transpose=True, start=True, stop=True)
                    bcol = tmppool.tile([8, 1], F32, tag="bcol", name=f"bcol_{u}", bufs=2)
                    nc.vector.tensor_copy(bcol[:], btp[:, 0:1])
                    cbp = ps_pool.tile([1, 128], F32, tag="cbp", name=f"cbp_{u}")
                    nc.tensor.matmul(cbp[:], bcol[:], wsum_p8[:], start=True, stop=True)
                    nc.vector.tensor_tensor(cb[:], cbp[:], bo_sb[:], ALU.add)
                cb_rep = tmppool.tile([128, 128], F32, tag="cb_rep", name=f"cbr_{u}", bufs=2)
                nc.gpsimd.partition_broadcast(cb_rep[:], cb[:])
                scal[u] = {"A_rep": A_rep, "cb_rep": cb_rep}

            qk = {}
            vts = []
            wsum_sb = cpool.tile([1, 1024], F32)
            wsum_f8 = cpool.tile([8, 128], F32)
            wsum_p8 = cpool.tile([8, 128], F32)

            # ================= attention-phase PSUM pools =================
            with (
                tc.tile_pool(name="ps_sgrp", bufs=2, space="PSUM") as ps_sgrp,
                tc.tile_pool(name="ps_u", bufs=1, space="PSUM") as ps_u,
                tc.tile_pool(name="ps_rdot", bufs=1, space="PSUM") as ps_rdot,
                tc.tile_pool(name="ps_rrow", bufs=1, space="PSUM") as ps_rrow,
            ):
                def emit_wsum():
                    # Wo column sums (for the GN-beta term), on the sgrp psum ring
                    wps = ps_sgrp.tile([128, 1024], F32, tag="sgrp", name="wps")
                    for h3 in range(8):
                        nc.tensor.matmul(wps[0:1, 128 * h3: 128 * (h3 + 1)],
                                         ones_f32[:, 0:1], wo_bf[h3][:],
                                         start=True, stop=True)
                    nc.vector.tensor_copy(wsum_sb[:, 0:1024], wps[0:1, 0:1024])
                    # row-per-h3 bf16 copy (for the PE-side cb matmul at the tail)
                    nc.sync.dma_start(
                        wsum_f8[:],
                        wsum_sb[:].rearrange("p (a b) -> (p a) b", a=8, b=128))
                    nc.vector.tensor_copy(wsum_p8[:], wsum_f8[:])

                def alloc_qk(u):
                    for nm in ("q1", "q2"):
                        qk[(u, nm)] = [
                            projpool.tile([128, 512], F32R, tag=f"{nm}t",
                                          name=f"{nm}t_{u}_{qb}", bufs=8)
                            for qb in range(4)
                        ]
                    for nm in ("k1", "k2"):
                        qk[(u, nm)] = [
                            projpool.tile([128, 1024], F32R, tag=f"{nm}t",
                                          name=f"{nm}t_{u}_{hh}", bufs=4)
                            for hh in range(2)
                        ]
                    vts.append(projpool.tile([128, 2048], F32R, tag="vp",
                                             name=f"vp_{u}", bufs=2))

                def proj_blocks(u, use_act=False):
                    # generator of closures: 10 psum-ring blocks per unit
                    # (4x q, 4x k with four 256-col chunks each; 2x v halves)
                    def qkblk(blk):
                        def emit():
                            ps = ps_sgrp.tile([128, 1024], F32, tag="sgrp",
                                              name=f"pp_{u}_{blk}")
                            wrh = wq_rh if blk < 4 else wk_rh
                            for c in range(4):
                                j = 4 * (blk % 4) + c
                                nc.tensor.matmul(
                                    ps[:, 256 * c: 256 * (c + 1)],
                                    wrh[j // 8][:, 128 * (j % 8): 128 * (j % 8 + 1)],
                                    qt_r[u][:], start=True, stop=True,
                                )
                            for c in range(4):
                                j = 4 * (blk % 4) + c
                                if blk < 4:
                                    dst = qk[(u, "q1" if j % 2 == 0 else "q2")][j // 4]
                                    col = 256 * ((j // 2) % 2)
                                    bias = bq_sb[:, j: j + 1]
                                else:
                                    dst = qk[(u, "k1" if j % 2 == 0 else "k2")][j // 8]
                                    col = 256 * ((j // 2) % 4)
                                    bias = bk_sb[:, j: j + 1]
                                if use_act and c % 2 == 0:
                                    # head phase: ACT is idle; Identity+bias is
                                    # in every table set (no exp-table thrash)
                                    nc.scalar.activation(
                                        dst[:, col: col + 256],
                                        ps[:, 256 * c: 256 * (c + 1)],
                                        AF.Identity, bias=bias,
                                    )
                                else:
                                    nc.vector.tensor_scalar_add(
                                        dst[:, col: col + 256],
                                        ps[:, 256 * c: 256 * (c + 1)], bias
                                    )
                        return emit

                    def vblk(rc):
                        def emit():
                            vt = vts[u]
                            ps = ps_sgrp.tile([128, 1024], F32, tag="sgrp",
                                              name=f"ppv_{u}_{rc}")
                            for fh in range(2):
                                nc.tensor.matmul(
                                    ps[:, 512 * fh: 512 * (fh + 1)],
                                    qt_r[u][:, 128 * rc: 128 * (rc + 1)],
                                    wv_r[:, 512 * fh: 512 * (fh + 1)],
                                    start=True, stop=True,
                                )
                            for fh in range(2):
                                nc.vector.tensor_tensor(
                                    vt[:, 1024 * rc + 512 * fh:
                                       1024 * rc + 512 * fh + 512],
                                    ps[:, 512 * fh: 512 * (fh + 1)],
                                    bv_rep[:, 512 * fh: 512 * (fh + 1)], ALU.add,
                                )
                        return emit

                    # pre: minimum to start attention qb0 (q j0-3, k j0-7, v);
                    # drip: the rest, fed into attention slots as DMA lands
                    pre = [qkblk(0), qkblk(4), qkblk(5), vblk(0), vblk(1)]
                    drip = [qkblk(6), qkblk(7), qkblk(1), qkblk(2), qkblk(3)]
                    return pre, drip

                def attention_unit(u, boundary_cb, slot_cb):
                    q1l, q2l = qk[(u, "q1")], qk[(u, "q2")]
                    kls = (qk[(u, "k1")], qk[(u, "k2")])
                    vt = vts[u]

                    def vchunk(kc):
                        return vt[:, 1024 * (kc % 2) + 128 * (kc // 2):
                                  1024 * (kc % 2) + 128 * (kc // 2) + 128]

                    fT = tmppool.tile([128, 2048], F32, tag="fT", name=f"fT_{u}")
                    fT_sb.append(fT)
                    p1a = tmppool.tile([128, 16], F32, tag="p1a", name=f"p1a_{u}")
                    p1b = tmppool.tile([128, 16], F32, tag="p1b", name=f"p1b_{u}")

                    LAG = 3  # consume items this many exp-slots behind issue
                    for qb in range(4):
                        u1 = ps_u.tile([128, 512], F32, tag="u1", name=f"u1_{u}_{qb}")
                        u2 = ps_u.tile([128, 512], F32, tag="u2", name=f"u2_{u}_{qb}")
                        rdot = ps_rdot.tile([128, 16], F32, tag="rd", name=f"rd_{u}_{qb}")
                        nc.vector.memset(rdot[:], 0.0)

                        rrep = [None, None]

                        def emit_r_chain(m):
                            # per-branch: [128,8] dots -> f32 transpose into a
                            # [1,512] psum row -> reciprocal -> partition bcast
                            r_sb = tmppool.tile([128, 8], F32, tag=f"rsb{m}",
                                                name=f"rsb_{u}_{qb}_{m}", bufs=2)
                            nc.vector.tensor_copy(r_sb[:], rdot[:, 8 * m: 8 * m + 8])
                            rrow = ps_rrow.tile([1, 512], F32, tag="rr",
                                                name=f"rr_{u}_{qb}_{m}")
                            for s4 in range(4):
                                nc.tensor.matmul(
                                    rrow[0:1, 128 * s4: 128 * (s4 + 1)],
                                    r_sb[:, 2 * s4: 2 * s4 + 1], eye_sb[:],
                                    is_transpose=True, start=True, stop=True,
                                )
                            r_inv = tmppool.tile([1, 512], F32, tag=f"rinv{m}",
                                                 name=f"rinv_{u}_{qb}_{m}", bufs=2)
                            nc.vector.reciprocal(r_inv[:], rrow[:])
                            rr = tmppool.tile([128, 512], F32, tag=f"r{m}rep",
                                              name=f"r{m}rep_{u}_{qb}", bufs=2)
                            nc.gpsimd.partition_broadcast(rr[:], r_inv[:])
                            rrep[m] = rr

                        def consume(item):
                            m, g, eg, uacc = item
                            for c in range(2):
                                kc = 2 * g + c
                                nc.tensor.matmul(
                                    uacc[:], vchunk(kc), eg[:, 512 * c: 512 * (c + 1)],
                                    start=(g == 0 and c == 0),
                                    stop=(g == 7 and c == 1),
                                )
                                for sl4 in range(4):
                                    col = 2 * (4 * m + sl4)
                                    nc.tensor.matmul(
                                        rdot[:, col: col + 2],
                                        eg[:, 512 * c + 128 * sl4: 512 * c + 128 * sl4 + 128],
                                        ones_col[:],
                                        start=False, stop=False, skip_group_check=True,
                                    )
                            if g == 7:
                                emit_r_chain(m)
                                if m == 0:
                                    # t1 = U1/R1 early, while branch 2 streams
                                    t1 = tmppool.tile([128, 512], F32, tag="t1",
                                                      name=f"t1_{u}_{qb}")
                                    nc.vector.tensor_tensor(t1[:], u1[:], rrep[0][:],
                                                            ALU.mult)
                                    tref[0] = t1

                        tref = [None]
                        pending = []
                        slot = 0
                        for m in range(2):
                            ql = (q1l, q2l)[m][qb]
                            kl = kls[m]
                            uacc = (u1, u2)[m]
                            for g in range(8):
                                sg = ps_sgrp.tile([128, 1024], F32, tag="sgrp",
                                                  name=f"sg_{u}_{qb}_{m}_{g}")
                                for c in range(2):
                                    kc = 2 * g + c
                                    nc.tensor.matmul(
                                        sg[:, 512 * c: 512 * (c + 1)],
                                        kl[kc // 8][:, 128 * (kc % 8): 128 * (kc % 8 + 1)],
                                        ql[:], start=True, stop=True,
                                    )
                                eg = epool.tile([128, 1024], F32R, tag="e",
                                                name=f"e_{u}_{qb}_{m}_{g}")
                                nc.scalar.activation(eg[:], sg[:], AF.Exp)
                                pending.append((m, g, eg, uacc))
                                if len(pending) > LAG:
                                    consume(pending.pop(0))
                                slot_cb(qb, slot)
                                slot += 1
                        for item in pending:
                            consume(item)

                        # O = U1/R1 - lam*U2/R2  (t1 emitted early, in-branch)
                        t2 = tmppool.tile([128, 512], F32, tag="t2", name=f"t2_{u}_{qb}")
                        nc.vector.scalar_tensor_tensor(
                            t2[:], u2[:], lam_rep[:, 0:1], rrep[1][:], ALU.mult, ALU.mult
                        )
                        otq = otpool.tile([128, 512], F32, tag="ot", name=f"ot_{u}_{qb}")
                        nc.vector.tensor_tensor(otq[:], tref[0][:], t2[:], ALU.subtract)
                        ot_refs[u].append(otq)

                        # incremental GroupNorm partial stats for this q-block
                        # (free-dim layout within the block: (j2, g8, r32))
                        osl = otq.rearrange("p (j g r) -> p j g r", j=2, g=8, r=32)
                        red = tmppool.tile([128, 16], F32, tag="red", name=f"red_{u}_{qb}")
                        nc.vector.tensor_reduce(red[:], osl, mybir.AxisListType.X, ALU.add)
                        if qb == 0:
                            nc.vector.tensor_copy(p1a[:], red[:])
                        else:
                            nc.vector.tensor_tensor(p1a[:], p1a[:], red[:], ALU.add)
                        sq5 = tmppool.tile([128, 512], F32, tag="t1", name=f"sq5_{u}_{qb}")
                        nc.vector.tensor_tensor(sq5[:], otq[:], otq[:], ALU.mult)
                        redb = tmppool.tile([128, 16], F32, tag="redb", name=f"redb_{u}_{qb}")
                        nc.vector.tensor_reduce(
                            redb[:], sq5.rearrange("p (j g r) -> p j g r", j=2, g=8, r=32),
                            mybir.AxisListType.X, ALU.add,
                        )
                        if qb == 0:
                            nc.vector.tensor_copy(p1b[:], redb[:])
                        else:
                            nc.vector.tensor_tensor(p1b[:], p1b[:], redb[:], ALU.add)
                        if u == 0 or qb < 2:
                            emit_relayout(u, qb)
                        boundary_cb(qb)

                    # fold (j mod 2) pairs -> per-group partials
                    for si, p1x in enumerate((p1a, p1b)):
                        p2 = tmppool.tile([128, 8], F32, tag="p2", name=f"p2_{u}_{si}")
                        nc.vector.tensor_reduce(
                            p2[:], p1x.rearrange("p (j g) -> p g j", j=2, g=8),
                            mybir.AxisListType.X, ALU.add,
                        )
                        p2_tiles[u].append(p2)

                def emit_relayout(u, qb):
                    # re-layout into fT (j-pair slab for this qb);
                    # src re-viewed g-outer to match the dst iteration order
                    fT = fT_sb[u]
                    fv4 = fT.rearrange("p (g j r) -> p g j r", g=8, j=8, r=32)
                    otq = ot_refs[u][qb]
                    nc.vector.tensor_copy(
                        fv4[:, :, 2 * qb: 2 * qb + 2, :],
                        otq.rearrange("p (j g r) -> p g j r", j=2, g=8, r=32),
                    )

                # ================= main schedule =================
                alloc_qk(0)
                alloc_qk(1)
                pre0, _ = proj_blocks(0)
                for blk in pre0:
                    blk()
                _, drip0 = proj_blocks(0)
                pre1, drip1 = proj_blocks(1)
                drip = drip0 + pre1 + drip1  # u0 stragglers first, then all of u1
                # slots at which to emit one proj block into u0's attention:
                # k j8-15 blocks early (needed by qb0 slot 4/6), then every 3rd
                drip_slots = [0, 2] + list(range(5, 64, 3))

                def u0_slot(qb, slot):
                    g = 16 * qb + slot
                    if drip and drip_slots and g >= drip_slots[0]:
                        drip_slots.pop(0)
                        drip.pop(0)()

                attention_unit(0, lambda qb: None, u0_slot)
                while drip:
                    drip.pop(0)()

                def u1_boundary(qb):
                    if qb == 0:
                        # unit-0 stats export + collective #1: lands on the Pool
                        # queue right after qb0's broadcasts; PE has a full qb of
                        # queued work to ride out the 15us Pool block.
                        emit_stats_export(0)
                        emit_wsum()
                    if qb == 2:
                        # unit-0 GN scalars (collective #1 landed long ago)
                        emit_scalars(0)

                attention_unit(1, u1_boundary, lambda qb, slot: None)
                # stats export for unit 1: as early as possible -> tail collective
                emit_stats_export(1)

            # ============== output stage (fills collective #2 window) ==============
            with tc.tile_pool(name="ps_out", bufs=1, space="PSUM") as ps_out:
                for qb in (2, 3):
                    emit_relayout(1, qb)

                P_sb = {}

                def emit_partials(u):
                    # 4 P outputs packed per [128,512] psum bank tile, then
                    # drained to SBUF (cheap to re-read; fills collective #2)
                    fT = fT_sb[u]
                    for rh in range(2):
                        for h4 in range(2):
                            pps = ps_out.tile([128, 512], F32, tag="P", bufs=4,
                                              name=f"pps_{u}_{rh}_{h4}")
                            for hq in range(4):
                                h3 = 4 * h4 + hq
                                lhsT = fT[:, 256 * h3 + 128 * rh: 256 * h3 + 128 * rh + 128]
                                nc.tensor.matmul(pps[:, 128 * hq: 128 * (hq + 1)],
                                                 lhsT, wo_bf[h3][:], start=True, stop=True)
                            P_sb[(u, rh, h4)] = pps

                def emit_combine(u, use_pool):
                    # result = sum_h3 A[u,h3]*P[u][rh][h3] + cb_rep[u]
                    # split per rh into a DVE half-chain (h3 0-3) and a Pool
                    # half-chain (h3 4-7), joined by one add
                    A_rep = scal[u]["A_rep"]
                    cb_rep = scal[u]["cb_rep"]
                    accs = {}
                    for rh in range(2):
                        for h4 in range(2):
                            psb = P_sb[(u, rh, h4)]
                            acc = tmppool.tile([128, 128], F32, tag=f"acc{h4}",
                                               name=f"acc_{u}_{rh}_{h4}")
                            if False:
                                # Pool path: tsm x4 + tt tree (no stt on Pool)
                                sc = tmppool.tile([128, 512], F32, tag="pscl",
                                                  name=f"pscl_{u}_{rh}", bufs=2)
                                for hq in range(4):
                                    nc.gpsimd.tensor_scalar_mul(
                                        sc[:, 128 * hq: 128 * (hq + 1)],
                                        psb[:, 128 * hq: 128 * (hq + 1)],
                                        A_rep[:, 4 * h4 + hq: 4 * h4 + hq + 1])
                                nc.gpsimd.tensor_tensor(
                                    sc[:, 0:128], sc[:, 0:128], sc[:, 128:256], ALU.add)
                                nc.gpsimd.tensor_tensor(
                                    sc[:, 256:384], sc[:, 256:384], sc[:, 384:512], ALU.add)
                                nc.gpsimd.tensor_tensor(
                                    acc[:], sc[:, 0:128], sc[:, 256:384], ALU.add)
                            else:
                                nc.vector.tensor_scalar_mul(
                                    acc[:], psb[:, 0:128], A_rep[:, 4 * h4: 4 * h4 + 1]
                                )
                                for hq in range(1, 4):
                                    h3 = 4 * h4 + hq
                                    nc.vector.scalar_tensor_tensor(
                                        acc[:], psb[:, 128 * hq: 128 * (hq + 1)],
                                        A_rep[:, h3: h3 + 1], acc[:],
                                        ALU.mult, ALU.add,
                                    )
                            accs[(rh, h4)] = acc
                    for rh in range(2):
                        rsb = tmppool.tile([128, 128], F32, tag="rsb2",
                                           name=f"rsb_{u}_{rh}")
                        nc.vector.tensor_tensor(rsb[:], accs[(rh, 0)][:],
                                                accs[(rh, 1)][:], ALU.add)
                        nc.vector.tensor_tensor(rsb[:], rsb[:], cb_rep[:], ALU.add)
                        # contiguous block write; host undoes the row permutation
                        # (device row 128*rh+m'' holds rho = 8*(m''%32)+4*rh+m''//32)
                        nc.sync.dma_start(outp[u][128 * rh: 128 * (rh + 1), :], rsb[:])

                emit_partials(0)
                emit_combine(0, use_pool=False)  # fill work; Pool is blocked by collective #2
                emit_partials(1)
                emit_scalars(1)       # waits on collective #2 (hidden behind fill work)
                emit_combine(1, use_pool=True)

    nc.compile()
    return nc


def _prep_inputs(inputs):
    """Host-side: slice/transpose full inputs into per-core in_maps."""
    query = np.asarray(inputs["query"], np.float32)
    Wq = np.asarray(inputs["Wq"], np.float32)
    Wk = np.asarray(inputs["Wk"], np.float32)
    Wv = np.asarray(inputs["Wv"], np.float32)
    Wo = np.asarray(inputs["Wo"], np.float32)
    bq = np.asarray(inputs["bq"], np.float32)
    bk = np.asarray(inputs["bk"], np.float32)
    bv = np.asarray(inputs["bv"], np.float32)
    bo = np.asarray(inputs["bo"], np.float32)
    gn_w = np.asarray(inputs["gn_w"], np.float32)
    gn_b = np.asarray(inputs["gn_b"], np.float32)
    lam = np.asarray(inputs["lam"], np.float32).reshape(1, 1)

    shared = {
        "wqT": np.ascontiguousarray(Wq.T),
        "wkT": np.ascontiguousarray(Wk.T),
        "wvT": np.ascontiguousarray(Wv.T),
        "woT": np.ascontiguousarray(Wo.T),
        "bqT": np.ascontiguousarray(bq.reshape(16, 128).T),
        "bkT": np.ascontiguousarray(bk.reshape(16, 128).T),
        "bv": bv.reshape(1, 1024),
        "bo": bo.reshape(1, 128),
        "gnw2": np.tile(gn_w, 2).reshape(1, 16),
        "gnb2": np.tile(gn_b, 2).reshape(1, 16),
        "lam": lam,
        "eye": np.eye(128, dtype=np.float32),
    }
    in_maps = []
    for c in range(N_CORES):
        blk = query[:, 256 * c: 256 * (c + 1), :]  # [B, 256, 128]
        qT = np.ascontiguousarray(blk.transpose(0, 2, 1))  # [B, 128, 256]
        in_maps.append({"qT": qT, **shared})
    return in_maps


class _Runner:
    """Cached-jit SPMD executor (one trace/compile; cheap repeated calls)."""

    def __init__(self, nc):
        import jax
        from jax.sharding import Mesh, PartitionSpec
        from jax.experimental.shard_map import shard_map
        from concourse.bass2jax import (
            install_neuronx_cc_hook, _bass_exec_p, partition_id_tensor,
        )

        install_neuronx_cc_hook()
        self.jax = jax
        pname = nc.partition_id_tensor.name if nc.partition_id_tensor else None
        in_names, out_names, out_avals, zero_outs = [], [], [], []
        for alloc in nc.m.functions[0].allocations:
            if not isinstance(alloc, mybir.MemoryLocationSet):
                continue
            name = alloc.memorylocations[0].name
            if alloc.kind == "ExternalInput":
                if name != pname:
                    in_names.append(name)
            elif alloc.kind == "ExternalOutput":
                out_names.append(name)
                shape = tuple(alloc.tensor_shape)
                dtype = mybir.dt.np(alloc.dtype)
                out_avals.append(jax.core.ShapedArray(shape, dtype))
                zero_outs.append(np.zeros(shape, dtype))
        self.in_names, self.out_names = in_names, out_names
        n_params = len(in_names)
        all_names = list(in_names) + out_names
        if pname is not None:
            all_names.append(pname)

        def _body(*args):
            operands = list(args)
            if pname is not None:
                operands.append(partition_id_tensor())
            return tuple(_bass_exec_p.bind(
                *operands, out_avals=tuple(out_avals), in_names=tuple(all_names),
                out_names=tuple(out_names), lowering_input_output_aliases=(),
                sim_require_finite=True, sim_require_nnan=True, nc=nc))

        devices = jax.devices()[:N_CORES]
        mesh = Mesh(np.asarray(devices), ("core",))
        nio = n_params + len(out_names)
        self.fn = jax.jit(
            shard_map(_body, mesh=mesh, in_specs=(PartitionSpec("core"),) * nio,
                      out_specs=(PartitionSpec("core"),) * len(out_names),
                      check_rep=False),
            keep_unused=True,
        )
        self.zeros = [
            jax.device_put(np.zeros((N_CORES * z.shape[0], *z.shape[1:]), z.dtype))
            for z in zero_outs
        ]
        self.out_shapes = [tuple(a.shape) for a in out_avals]

    def run(self, in_maps):
        cat = [
            np.concatenate([np.asarray(m[n]) for m in in_maps], axis=0)
            for n in self.in_names
        ]
        # the accelerator intermittently throws a transient
        # NRT_EXEC_UNIT_UNRECOVERABLE (status 101); retry once
        for attempt in range(3):
            try:
                outs = self.fn(*cat, *self.zeros)
                self.jax.block_until_ready(outs)
                outs = [np.asarray(o) for o in outs]
                break
            except Exception:
                if attempt == 2:
                    raise
                import time as _t
                _t.sleep(5.0)
        return [
            {n: outs[i].reshape(N_CORES, *self.out_shapes[i])[c]
             for i, n in enumerate(self.out_names)}
            for c in range(N_CORES)
        ]


_CACHED_NC = None


def kernel(**inputs) -> np.ndarray:
    global _CACHED, _CACHED_NC
    if _CACHED is None:
        _CACHED_NC = build_nc()
        _CACHED = _Runner(_CACHED_NC)
    in_maps = _prep_inputs(inputs)
    results = _CACHED.run(in_maps)
    # device row (rh, m'') holds output row rho = 8*(m'' % 32) + 4*rh + m''//32
    mpp = np.arange(128)
    rho = np.concatenate([8 * (mpp % 32) + 4 * rh + mpp // 32 for rh in (0, 1)])
    inv = np.argsort(rho)
    out = np.empty((B, S, H * D // 8), np.float32)  # (2, 2048, 128)
    for c in range(N_CORES):
        o = results[c]["outp"]  # [B, 256, 128] in device (rh, m'') row order
        for b in range(B):
            out[b, c::8, :] = o[b][inv]  # rows s3 = 8*rho + c
    return out


# revision 7
# speedup vs baseline: 1.0043x; 1.0043x over previous
"""DiffAttention Trainium2 kernel, 8-core SPMD (head-parallel), v2.

Problem (hardcoded): B=2, S=2048, D=128, H=8.
  q = (x@Wq.T+bq).reshape(B,H,S,2D)   # raw reshape: head h <-> rows [256h,256h+256) of proj
  s1 = q1@k1.T; s2 = q2@k2.T; attn = softmax(s1) - lam*softmax(s2)
  out = attn@v -> transpose/reshape -> GroupNorm(H groups) -> *(1-lam) -> concat heads -> @Wo.T+bo

Sharding: core c owns head h=c for both batches (2 units/core). GroupNorm groups
mix all heads -> tiny (32-float) AllGather of partial stats.

Index algebra per unit (b,h), block = proj rows [256h, 256h+256):
  sigma (attn row) = 8r+j, r in [0,256), j in [0,8). We use tau-order sigma' = 256j+r.
  q1T[d, sigma'=256j+r] = qpT_block[f=256j+d, r]   (even 128-col chunks of qp block)
  q2T: odd chunks.  v'[sigma'=256j+r, d] = vp_block[r, 128j+d].
  GroupNorm group g = {sigma': (sigma' mod 256)//32 == g} (32-wide strips).
  Final rows: out[b, 8*rho+h, 128h3+d] = GN(O)[b,h][sigma'=256(rho%8)+32h3+rho//8, d]

v2 changes vs v1:
  - softmax denominators via [128q,1]-output dot matmuls (nearly free on PE)
    + PE transposes into a [1,1024] psum row + DVE reciprocal + gpsimd
    partition_broadcast, replacing full-width ones-matmul accumulations.
  - exp on [128,2048] tiles (half the ACT instruction overhead).
  - bf16 V / E / fT / Wo (output matmuls 4x cheaper); q/k stay f32r.
  - output-stage partials read PSUM directly; collectives scheduled so the
    Pool-queue block lands where PE has a queued qb of slack.
"""

import sys

sys.path.insert(0, "/opt/trn_rl_repo")

import numpy as np

import concourse.bass as bass
import concourse.bacc as bacc
import concourse.mybir as mybir
import concourse.tile as tile

F32 = mybir.dt.float32
F32R = mybir.dt.float32r
BF16 = mybir.dt.bfloat16
AF = mybir.ActivationFunctionType
ALU = mybir.AluOpType

B, S, D, H = 2, 2048, 128, 8
N_CORES = 8
EPS = 1e-5
GROUP_N = float(256 * H * D)  # elements per GroupNorm group

_CACHED = None


def build_nc():
    nc = bacc.Bacc("TRN2", target_bir_lowering=False, debug=False, num_devices=N_CORES)

    # ---- per-core external I/O ----
    qT = nc.dram_tensor("qT", [B, 128, 256], F32, kind="ExternalInput")  # query block.T per batch
    wqT = nc.dram_tensor("wqT", [128, 2048], F32, kind="ExternalInput")
    wkT = nc.dram_tensor("wkT", [128, 2048], F32, kind="ExternalInput")
    wvT = nc.dram_tensor("wvT", [128, 1024], F32, kind="ExternalInput")
    woT = nc.dram_tensor("woT", [1024, 128], F32, kind="ExternalInput")
    bqT = nc.dram_tensor("bqT", [128, 16], F32, kind="ExternalInput")
    bkT = nc.dram_tensor("bkT", [128, 16], F32, kind="ExternalInput")
    bv = nc.dram_tensor("bv", [1, 1024], F32, kind="ExternalInput")
    bo = nc.dram_tensor("bo", [1, 128], F32, kind="ExternalInput")
    gnw2 = nc.dram_tensor("gnw2", [1, 16], F32, kind="ExternalInput")  # tiled x2 (b,g)
    gnb2 = nc.dram_tensor("gnb2", [1, 16], F32, kind="ExternalInput")
    lam = nc.dram_tensor("lam", [1, 1], F32, kind="ExternalInput")
    eye = nc.dram_tensor("eye", [128, 128], F32, kind="ExternalInput")
    outp = nc.dram_tensor("outp", [B, 256, 128], F32, kind="ExternalOutput")

    with tile.TileContext(nc) as tc:
        with (
            tc.tile_pool(name="const", bufs=1) as cpool,
            tc.tile_pool(name="proj", bufs=2) as projpool,
            tc.tile_pool(name="epool", bufs=4) as epool,
            tc.tile_pool(name="otpool", bufs=4) as otpool,
            tc.tile_pool(name="tmp", bufs=2) as tmppool,
            tc.tile_pool(name="dram", bufs=1, space="DRAM") as dram,
        ):
            # ---- load constants / weights (qT first: projections need it) ----
            qt_sb = []
            for u in range(B):
                q = cpool.tile([128, 256], F32, name=f"qt_sb{u}")
                nc.sync.dma_start(q[:], qT[u])
                qt_sb.append(q)

            # small constants go on the gpsimd DMA queue so they don't delay
            # the big weight DMAs on the sync queue
            bq_sb = cpool.tile([128, 16], F32)
            bk_sb = cpool.tile([128, 16], F32)
            nc.gpsimd.dma_start(bq_sb[:], bqT[:])
            nc.gpsimd.dma_start(bk_sb[:], bkT[:])
            bv_sb = cpool.tile([1, 1024], F32)
            nc.gpsimd.dma_start(bv_sb[:], bv[:])
            bo_sb = cpool.tile([1, 128], F32)
            nc.gpsimd.dma_start(bo_sb[:], bo[:])
            gnw_sb = cpool.tile([1, 16], F32)
            gnb_sb = cpool.tile([1, 16], F32)
            nc.gpsimd.dma_start(gnw_sb[:], gnw2[:])
            nc.gpsimd.dma_start(gnb_sb[:], gnb2[:])
            lam_sb = cpool.tile([1, 1], F32)
            nc.gpsimd.dma_start(lam_sb[:], lam[:])
            eye_sb = cpool.tile([128, 128], F32)
            nc.gpsimd.dma_start(eye_sb[:], eye[:])

            # weights loaded and f32r-rounded in 1024-col pieces so projections
            # can start before all input DMA completes.
            wq_rh, wk_rh = [], []
            wv_r = cpool.tile([128, 1024], F32R)
            wpieces = (
                [("wq", wqT, wq_rh, 0), ("wk", wkT, wk_rh, 0),
                 ("wk", wkT, wk_rh, 1), ("wv", wvT, None, 0),
                 ("wq", wqT, wq_rh, 1)]
            )
            for (wnm, dram_w, lst, half) in wpieces:
                wsc = projpool.tile([128, 1024], F32, tag="wsc", name=f"wsc_{wnm}{half}")
                nc.sync.dma_start(wsc[:], dram_w[:, 1024 * half: 1024 * (half + 1)])
                if lst is None:
                    nc.vector.tensor_copy(wv_r[:], wsc[:])
                else:
                    wr = cpool.tile([128, 1024], F32R, name=f"{wnm}_r{half}")
                    nc.vector.tensor_copy(wr[:], wsc[:])
                    lst.append(wr)
            qt_r = []
            for u in range(B):
                qr = cpool.tile([128, 256], F32R, name=f"qt_r{u}")
                nc.vector.tensor_copy(qr[:], qt_sb[u][:])
                qt_r.append(qr)
            lam_rep = cpool.tile([128, 1], F32)
            nc.gpsimd.partition_broadcast(lam_rep[:], lam_sb[:])
            oml = cpool.tile([1, 1], F32)
            nc.vector.tensor_scalar(oml[:], lam_sb[:], -1.0, 1.0, ALU.mult, ALU.add)
            bv_rep = cpool.tile([128, 1024], F32)
            nc.gpsimd.partition_broadcast(bv_rep[:], bv_sb[:])

            ones_f32 = cpool.tile([128, 1], F32)
            nc.vector.memset(ones_f32[:], 1.0)
            ones2_f32 = cpool.tile([128, 2], F32)
            nc.vector.memset(ones2_f32[:], 1.0)
            ones_col = cpool.tile([128, 2], F32R)
            nc.vector.tensor_copy(ones_col[:], ones2_f32[:])

            # Wo chunks in bf16 (moving operand of the output matmuls);
            # DMAs queued after the projection weights
            wo_bf = []
            for h3 in range(8):
                wsc = projpool.tile([128, 128], F32, tag="wosc", name=f"wosc{h3}")
                nc.sync.dma_start(wsc[:], woT[128 * h3: 128 * (h3 + 1), :])
                w = cpool.tile([128, 128], F32, name=f"wo_bf{h3}")
                nc.vector.tensor_copy(w[:], wsc[:])
                wo_bf.append(w)

            p2_tiles = {0: [], 1: []}
            fT_sb = []
            ot_refs = {0: [], 1: []}  # otq tiles per unit (for deferred re-layout)

            cc_in = [dram.tile([1, 16], F32, name=f"cc_in{u}") for u in range(B)]
            cc_out = [dram.tile([8, 16], F32, addr_space="Shared", name=f"cc_out{u}")
                      for u in range(B)]
            gath = [tmppool.tile([1, 128], F32, tag="gath", name=f"gath_{u}", bufs=2)
                    for u in range(B)]
            scal = [{}, {}]

            def ptree(dst, src_tile, parts, width, nm, eng=None):
                eng = eng or nc.vector
                # partition-axis sum: DVE shift-copy + add down to 32 partitions
                # (TT needs equal base partitions; slices are 32-aligned),
                # then one gpsimd C-axis reduce for the final 32 -> 1.
                cur = src_tile
                while parts > 32:
                    parts //= 2
                    sh = tmppool.tile([parts, width], F32, tag=f"ps{parts}",
                                      name=f"ps_{nm}_{parts}", bufs=4)
                    eng.tensor_copy(sh[:], cur[parts: 2 * parts, :])
                    t = tmppool.tile([parts, width], F32, tag=f"pt{parts}",
                                     name=f"pt_{nm}_{parts}", bufs=4)
                    eng.tensor_tensor(t[:], cur[0:parts, :], sh[:], ALU.add)
                    cur = t
                nc.gpsimd.tensor_reduce(dst, cur[:], mybir.AxisListType.C, ALU.add)

            def emit_stats_export(u):
                stats_u = tmppool.tile([1, 16], F32, tag="stats", name=f"stats_{u}", bufs=2)
                for si, p2 in enumerate(p2_tiles[u]):
                    ptree(stats_u[:, 8 * si: 8 * si + 8], p2, 128, 8, f"st{u}{si}",
                          eng=(nc.vector, nc.gpsimd)[si])
                nc.sync.dma_start(cc_in[u][:], stats_u[:])
                nc.gpsimd.collective_compute(
                    "AllGather", ALU.bypass,
                    replica_groups=[list(range(N_CORES))],
                    ins=[cc_in[u][:]], outs=[cc_out[u][:]],
                )
                nc.gpsimd.dma_start(gath[u][:],
                                     cc_out[u][:].rearrange("a b -> (a b)").unsqueeze(0))

            def emit_scalars(u, cb_mm=None):
                # global stats for batch u -> A_rep[128,8], cb_rep[128,128]
                t = lambda nm: tmppool.tile([1, 8], F32, tag=nm, name=f"{nm}_{u}", bufs=2)
                glob = tmppool.tile([1, 16], F32, tag="globsb", name=f"glob_{u}", bufs=2)
                nc.vector.tensor_reduce(
                    glob[:], gath[u].rearrange("p (a b) -> p b a", a=8, b=16),
                    mybir.AxisListType.X, ALU.add,
                )
                moments = tmppool.tile([1, 16], F32, tag="mom", name=f"mom_{u}", bufs=2)
                nc.vector.tensor_scalar_mul(moments[:], glob[:], 1.0 / GROUP_N)
                mean, ex2 = moments[:, 0:8], moments[:, 8:16]
                var, veps = t("var"), t("veps")
                nc.vector.tensor_tensor(var[:], mean, mean, ALU.mult)
                nc.vector.tensor_tensor(var[:], ex2, var[:], ALU.subtract)
                nc.vector.tensor_scalar_add(veps[:], var[:], EPS)
                # rsqrt fully on DVE (ACT Sqrt would thrash the exp table set):
                # quake seed + 2 Newton steps
                I32 = mybir.dt.int32
                ti = tmppool.tile([1, 8], I32, tag="rsqi", name=f"rsqi_{u}", bufs=2)
                nc.vector.tensor_scalar(
                    ti[:], veps[:].bitcast(I32), 1, None, ALU.arith_shift_right
                )
                nc.vector.tensor_scalar(ti[:], ti[:], -1, 0x5F3759DF, ALU.mult, ALU.add)
                rstd, hf, nt = t("rstd"), t("hf"), t("nt")
                nc.vector.tensor_copy(rstd[:], ti[:].bitcast(F32))
                nc.vector.tensor_scalar_mul(hf[:], veps[:], 0.5)
                for _ in range(2):
                    nc.vector.tensor_tensor(nt[:], rstd[:], rstd[:], ALU.mult)
                    nc.vector.tensor_tensor(nt[:], nt[:], hf[:], ALU.mult)
                    nc.vector.tensor_scalar(nt[:], nt[:], -1.0, 1.5, ALU.mult, ALU.add)
                    nc.vector.tensor_tensor(rstd[:], rstd[:], nt[:], ALU.mult)
                A, Bc = t("A"), t("Bc")
                nc.vector.tensor_tensor(A[:], rstd[:], gnw_sb[:, 0:8], ALU.mult)
                nc.vector.tensor_tensor(Bc[:], mean, A[:], ALU.mult)
                nc.vector.tensor_tensor(Bc[:], gnb_sb[:, 0:8], Bc[:], ALU.subtract)
                nc.vector.tensor_scalar_mul(A[:], A[:], oml[:, 0:1])
                nc.vector.tensor_scalar_mul(Bc[:], Bc[:], oml[:, 0:1])
                A_rep = tmppool.tile([128, 8], F32, tag="A_rep", name=f"A_rep{u}", bufs=2)
                nc.gpsimd.partition_broadcast(A_rep[:], A[:])
                cb = tmppool.tile([1, 128], F32, tag="cb", name=f"cb_{u}", bufs=2)
                if cb_mm is None:
                    # serial stt chain (fine off the critical path)
                    nc.vector.tensor_scalar_mul(cb[:], wsum_sb[:, 0:128], Bc[:, 0:1])
                    for h3 in range(1, 8):
                        nc.vector.scalar_tensor_tensor(
                            cb[:], wsum_sb[:, 128 * h3: 128 * (h3 + 1)],
                            Bc[:, h3: h3 + 1], cb[:], ALU.mult, ALU.add,
                        )
                    nc.vector.tensor_tensor(cb[:], cb[:], bo_sb[:], ALU.add)
                else:
                    # critical path: cb = Bc(1x8) @ wsum_p8(8x128) via PE
                    # (transpose Bc to a column first), then + bo
                    ps_pool = cb_mm
                    btp = ps_pool.tile([8, 8], F32, tag="btp", name=f"btp_{u}")
                    nc.tensor.matmul(btp[:, 0:1], Bc[:], ones_f32[0:1, 0:1],
                                     is_transpose=True, start=True, stop=True)
                    bcol = tmppool.tile([8, 1], F32, tag="bcol", name=f"bcol_{u}", bufs=2)
                    nc.vector.tensor_copy(bcol[:], btp[:, 0:1])
                    cbp = ps_pool.tile([1, 128], F32, tag="cbp", name=f"cbp_{u}")
                    nc.tensor.matmul(cbp[:], bcol[:], wsum_p8[:], start=True, stop=True)
                    nc.vector.tensor_tensor(cb[:], cbp[:], bo_sb[:], ALU.add)
                cb_rep = tmppool.tile([128, 128], F32, tag="cb_rep", name=f"cbr_{u}", bufs=2)
                nc.gpsimd.partition_broadcast(cb_rep[:], cb[:])
                scal[u] = {"A_rep": A_rep, "cb_rep": cb_rep}

            qk = {}
            vts = []
            wsum_sb = cpool.tile([1, 1024], F32)
            wsum_f8 = cpool.tile([8, 128], F32)
            wsum_p8 = cpool.tile([8, 128], F32)

            # ================= attention-phase PSUM pools =================
            with (
                tc.tile_pool(name="ps_sgrp", bufs=2, space="PSUM") as ps_sgrp,
                tc.tile_pool(name="ps_u", bufs=1, space="PSUM") as ps_u,
                tc.tile_pool(name="ps_rdot", bufs=1, space="PSUM") as ps_rdot,
                tc.tile_pool(name="ps_rrow", bufs=1, space="PSUM") as ps_rrow,
            ):
                def emit_wsum():
                    # Wo column sums (for the GN-beta term), on the sgrp psum ring
                    wps = ps_sgrp.tile([128, 1024], F32, tag="sgrp", name="wps")
                    for h3 in range(8):
                        nc.tensor.matmul(wps[0:1, 128 * h3: 128 * (h3 + 1)],
                                         ones_f32[:, 0:1], wo_bf[h3][:],
                                         start=True, stop=True)
                    nc.vector.tensor_copy(wsum_sb[:, 0:1024], wps[0:1, 0:1024])
                    # row-per-h3 bf16 copy (for the PE-side cb matmul at the tail)
                    nc.sync.dma_start(
                        wsum_f8[:],
                        wsum_sb[:].rearrange("p (a b) -> (p a) b", a=8, b=128))
                    nc.vector.tensor_copy(wsum_p8[:], wsum_f8[:])

                def alloc_qk(u):
                    for nm in ("q1", "q2"):
                        qk[(u, nm)] = [
                            projpool.tile([128, 512], F32R, tag=f"{nm}t",
                                          name=f"{nm}t_{u}_{qb}", bufs=8)
                            for qb in range(4)
                        ]
                    for nm in ("k1", "k2"):
                        qk[(u, nm)] = [
                            projpool.tile([128, 1024], F32R, tag=f"{nm}t",
                                          name=f"{nm}t_{u}_{hh}", bufs=4)
                            for hh in range(2)
                        ]
                    vts.append(projpool.tile([128, 2048], F32R, tag="vp",
                                             name=f"vp_{u}", bufs=2))

                def proj_blocks(u, use_act=False):
                    # generator of closures: 10 psum-ring blocks per unit
                    # (4x q, 4x k with four 256-col chunks each; 2x v halves)
                    def qkblk(blk):
                        def emit():
                            ps = ps_sgrp.tile([128, 1024], F32, tag="sgrp",
                                              name=f"pp_{u}_{blk}")
                            wrh = wq_rh if blk < 4 else wk_rh
                            for c in range(4):
                                j = 4 * (blk % 4) + c
                                nc.tensor.matmul(
                                    ps[:, 256 * c: 256 * (c + 1)],
                                    wrh[j // 8][:, 128 * (j % 8): 128 * (j % 8 + 1)],
                                    qt_r[u][:], start=True, stop=True,
                                )
                            for c in range(4):
                                j = 4 * (blk % 4) + c
                                if blk < 4:
                                    dst = qk[(u, "q1" if j % 2 == 0 else "q2")][j // 4]
                                    col = 256 * ((j // 2) % 2)
                                    bias = bq_sb[:, j: j + 1]
                                else:
                                    dst = qk[(u, "k1" if j % 2 == 0 else "k2")][j // 8]
                                    col = 256 * ((j // 2) % 4)
                                    bias = bk_sb[:, j: j + 1]
                                if use_act and c % 2 == 0:
                                    # head phase: ACT is idle; Identity+bias is
                                    # in every table set (no exp-table thrash)
                                    nc.scalar.activation(
                                        dst[:, col: col + 256],
                                        ps[:, 256 * c: 256 * (c + 1)],
                                        AF.Identity, bias=bias,
                                    )
                                else:
                                    nc.vector.tensor_scalar_add(
                                        dst[:, col: col + 256],
                                        ps[:, 256 * c: 256 * (c + 1)], bias
                                    )
                        return emit

                    def vblk(rc):
                        def emit():
                            vt = vts[u]
                            ps = ps_sgrp.tile([128, 1024], F32, tag="sgrp",
                                              name=f"ppv_{u}_{rc}")
                            for fh in range(2):
                                nc.tensor.matmul(
                                    ps[:, 512 * fh: 512 * (fh + 1)],
                                    qt_r[u][:, 128 * rc: 128 * (rc + 1)],
                                    wv_r[:, 512 * fh: 512 * (fh + 1)],
                                    start=True, stop=True,
                                )
                            for fh in range(2):
                                nc.vector.tensor_tensor(
                                    vt[:, 1024 * rc + 512 * fh:
                                       1024 * rc + 512 * fh + 512],
                                    ps[:, 512 * fh: 512 * (fh + 1)],
                                    bv_rep[:, 512 * fh: 512 * (fh + 1)], ALU.add,
                                )
                        return emit

                    # pre: minimum to start attention qb0 (q j0-3, k j0-7, v);
                    # drip: the rest, fed into attention slots as DMA lands
                    pre = [qkblk(0), qkblk(4), qkblk(5), vblk(0), vblk(1)]
                    drip = [qkblk(6), qkblk(7), qkblk(1), qkblk(2), qkblk(3)]
                    return pre, drip

                def attention_unit(u, boundary_cb, slot_cb):
                    q1l, q2l = qk[(u, "q1")], qk[(u, "q2")]
                    kls = (qk[(u, "k1")], qk[(u, "k2")])
                    vt = vts[u]

                    def vchunk(kc):
                        return vt[:, 1024 * (kc % 2) + 128 * (kc // 2):
                                  1024 * (kc % 2) + 128 * (kc // 2) + 128]

                    fT = tmppool.tile([128, 2048], F32, tag="fT", name=f"fT_{u}")
                    fT_sb.append(fT)
                    p1a = tmppool.tile([128, 16], F32, tag="p1a", name=f"p1a_{u}")
                    p1b = tmppool.tile([128, 16], F32, tag="p1b", name=f"p1b_{u}")

                    LAG = 3  # consume items this many exp-slots behind issue
                    for qb in range(4):
                        u1 = ps_u.tile([128, 512], F32, tag="u1", name=f"u1_{u}_{qb}")
                        u2 = ps_u.tile([128, 512], F32, tag="u2", name=f"u2_{u}_{qb}")
                        rdot = ps_rdot.tile([128, 16], F32, tag="rd", name=f"rd_{u}_{qb}")
                        nc.vector.memset(rdot[:], 0.0)

                        rrep = [None, None]

                        def emit_r_chain(m):
                            # per-branch: [128,8] dots -> f32 transpose into a
                            # [1,512] psum row -> reciprocal -> partition bcast
                            r_sb = tmppool.tile([128, 8], F32, tag=f"rsb{m}",
                                                name=f"rsb_{u}_{qb}_{m}", bufs=2)
                            nc.vector.tensor_copy(r_sb[:], rdot[:, 8 * m: 8 * m + 8])
                            rrow = ps_rrow.tile([1, 512], F32, tag="rr",
                                                name=f"rr_{u}_{qb}_{m}")
                            for s4 in range(4):
                                nc.tensor.matmul(
                                    rrow[0:1, 128 * s4: 128 * (s4 + 1)],
                                    r_sb[:, 2 * s4: 2 * s4 + 1], eye_sb[:],
                                    is_transpose=True, start=True, stop=True,
                                )
                            r_inv = tmppool.tile([1, 512], F32, tag=f"rinv{m}",
                                                 name=f"rinv_{u}_{qb}_{m}", bufs=2)
                            nc.vector.reciprocal(r_inv[:], rrow[:])
                            rr = tmppool.tile([128, 512], F32, tag=f"r{m}rep",
                                              name=f"r{m}rep_{u}_{qb}", bufs=2)
                            nc.gpsimd.partition_broadcast(rr[:], r_inv[:])
                            rrep[m] = rr

                        def consume(item):
                            m, g, eg, uacc = item
                            for c in range(2):
                                kc = 2 * g + c
                                nc.tensor.matmul(
                                    uacc[:], vchunk(kc), eg[:, 512 * c: 512 * (c + 1)],
                                    start=(g == 0 and c == 0),
                                    stop=(g == 7 and c == 1),
                                )
                                for sl4 in range(4):
                                    col = 2 * (4 * m + sl4)
                                    nc.tensor.matmul(
                                        rdot[:, col: col + 2],
                                        eg[:, 512 * c + 128 * sl4: 512 * c + 128 * sl4 + 128],
                                        ones_col[:],
                                        start=False, stop=False, skip_group_check=True,
                                    )
                            if g == 7:
                                emit_r_chain(m)
                                if m == 0:
                                    # t1 = U1/R1 early, while branch 2 streams
                                    t1 = tmppool.tile([128, 512], F32, tag="t1",
                                                      name=f"t1_{u}_{qb}")
                                    nc.vector.tensor_tensor(t1[:], u1[:], rrep[0][:],
                                                            ALU.mult)
                                    tref[0] = t1

                        tref = [None]
                        pending = []
                        slot = 0
                        for m in range(2):
                            ql = (q1l, q2l)[m][qb]
                            kl = kls[m]
                            uacc = (u1, u2)[m]
                            for g in range(8):
                                sg = ps_sgrp.tile([128, 1024], F32, tag="sgrp",
                                                  name=f"sg_{u}_{qb}_{m}_{g}")
                                for c in range(2):
                                    kc = 2 * g + c
                                    nc.tensor.matmul(
                                        sg[:, 512 * c: 512 * (c + 1)],
                                        kl[kc // 8][:, 128 * (kc % 8): 128 * (kc % 8 + 1)],
                                        ql[:], start=True, stop=True,
                                    )
                                eg = epool.tile([128, 1024], F32R, tag="e",
                                                name=f"e_{u}_{qb}_{m}_{g}")
                                nc.scalar.activation(eg[:], sg[:], AF.Exp)
                                pending.append((m, g, eg, uacc))
                                if len(pending) > LAG:
                                    consume(pending.pop(0))
                                slot_cb(qb, slot)
                                slot += 1
                        for item in pending:
                            consume(item)

                        # O = U1/R1 - lam*U2/R2  (t1 emitted early, in-branch)
                        t2 = tmppool.tile([128, 512], F32, tag="t2", name=f"t2_{u}_{qb}")
                        nc.vector.scalar_tensor_tensor(
                            t2[:], u2[:], lam_rep[:, 0:1], rrep[1][:], ALU.mult, ALU.mult
                        )
                        otq = otpool.tile([128, 512], F32, tag="ot", name=f"ot_{u}_{qb}")
                        nc.vector.tensor_tensor(otq[:], tref[0][:], t2[:], ALU.subtract)
                        ot_refs[u].append(otq)

                        # incremental GroupNorm partial stats for this q-block
                        # (free-dim layout within the block: (j2, g8, r32))
                        osl = otq.rearrange("p (j g r) -> p j g r", j=2, g=8, r=32)
                        red = tmppool.tile([128, 16], F32, tag="red", name=f"red_{u}_{qb}")
                        nc.vector.tensor_reduce(red[:], osl, mybir.AxisListType.X, ALU.add)
                        if qb == 0:
                            nc.vector.tensor_copy(p1a[:], red[:])
                        else:
                            nc.vector.tensor_tensor(p1a[:], p1a[:], red[:], ALU.add)
                        sq5 = tmppool.tile([128, 512], F32, tag="t1", name=f"sq5_{u}_{qb}")
                        sq_eng = nc.gpsimd if (u == 1 and qb == 3) else nc.vector
                        sq_eng.tensor_tensor(sq5[:], otq[:], otq[:], ALU.mult)
                        redb = tmppool.tile([128, 16], F32, tag="redb", name=f"redb_{u}_{qb}")
                        nc.vector.tensor_reduce(
                            redb[:], sq5.rearrange("p (j g r) -> p j g r", j=2, g=8, r=32),
                            mybir.AxisListType.X, ALU.add,
                        )
                        if qb == 0:
                            nc.vector.tensor_copy(p1b[:], redb[:])
                        else:
                            nc.vector.tensor_tensor(p1b[:], p1b[:], redb[:], ALU.add)
                        if u == 0 or qb < 2:
                            emit_relayout(u, qb)
                        boundary_cb(qb)

                    # fold (j mod 2) pairs -> per-group partials
                    for si, p1x in enumerate((p1a, p1b)):
                        p2 = tmppool.tile([128, 8], F32, tag="p2", name=f"p2_{u}_{si}")
                        nc.vector.tensor_reduce(
                            p2[:], p1x.rearrange("p (j g) -> p g j", j=2, g=8),
                            mybir.AxisListType.X, ALU.add,
                        )
                        p2_tiles[u].append(p2)

                def emit_relayout(u, qb):
                    # re-layout into fT (j-pair slab for this qb);
                    # src re-viewed g-outer to match the dst iteration order
                    fT = fT_sb[u]
                    fv4 = fT.rearrange("p (g j r) -> p g j r", g=8, j=8, r=32)
                    otq = ot_refs[u][qb]
                    nc.vector.tensor_copy(
                        fv4[:, :, 2 * qb: 2 * qb + 2, :],
                        otq.rearrange("p (j g r) -> p g j r", j=2, g=8, r=32),
                    )

                # ================= main schedule =================
                alloc_qk(0)
                alloc_qk(1)
                pre0, _ = proj_blocks(0)
                for blk in pre0:
                    blk()
                _, drip0 = proj_blocks(0)
                pre1, drip1 = proj_blocks(1)
                drip = drip0 + pre1 + drip1  # u0 stragglers first, then all of u1
                # slots at which to emit one proj block into u0's attention:
                # k j8-15 blocks early (needed by qb0 slot 4/6), then every 3rd
                drip_slots = [0, 2] + list(range(5, 64, 3))

                def u0_slot(qb, slot):
                    g = 16 * qb + slot
                    if drip and drip_slots and g >= drip_slots[0]:
                        drip_slots.pop(0)
                        drip.pop(0)()

                attention_unit(0, lambda qb: None, u0_slot)
                while drip:
                    drip.pop(0)()

                def u1_boundary(qb):
                    if qb == 0:
                        # unit-0 stats export + collective #1: lands on the Pool
                        # queue right after qb0's broadcasts; PE has a full qb of
                        # queued work to ride out the 15us Pool block.
                        emit_stats_export(0)
                        emit_wsum()
                    if qb == 2:
                        # unit-0 GN scalars (collective #1 landed long ago)
                        emit_scalars(0)

                attention_unit(1, u1_boundary, lambda qb, slot: None)
                # stats export for unit 1: as early as possible -> tail collective
                emit_stats_export(1)

            # ============== output stage (fills collective #2 window) ==============
            with tc.tile_pool(name="ps_out", bufs=1, space="PSUM") as ps_out:
                for qb in (2, 3):
                    emit_relayout(1, qb)

                P_sb = {}

                def emit_partials(u):
                    # 4 P outputs packed per [128,512] psum bank tile, then
                    # drained to SBUF (cheap to re-read; fills collective #2)
                    fT = fT_sb[u]
                    for rh in range(2):
                        for h4 in range(2):
                            pps = ps_out.tile([128, 512], F32, tag="P", bufs=4,
                                              name=f"pps_{u}_{rh}_{h4}")
                            for hq in range(4):
                                h3 = 4 * h4 + hq
                                lhsT = fT[:, 256 * h3 + 128 * rh: 256 * h3 + 128 * rh + 128]
                                nc.tensor.matmul(pps[:, 128 * hq: 128 * (hq + 1)],
                                                 lhsT, wo_bf[h3][:], start=True, stop=True)
                            P_sb[(u, rh, h4)] = pps

                def emit_combine(u, use_pool):
                    # result = sum_h3 A[u,h3]*P[u][rh][h3] + cb_rep[u]
                    # split per rh into a DVE half-chain (h3 0-3) and a Pool
                    # half-chain (h3 4-7), joined by one add
                    A_rep = scal[u]["A_rep"]
                    cb_rep = scal[u]["cb_rep"]
                    accs = {}
                    for rh in range(2):
                        for h4 in range(2):
                            psb = P_sb[(u, rh, h4)]
                            acc = tmppool.tile([128, 128], F32, tag=f"acc{h4}",
                                               name=f"acc_{u}_{rh}_{h4}")
                            if False:
                                # Pool path: tsm x4 + tt tree (no stt on Pool)
                                sc = tmppool.tile([128, 512], F32, tag="pscl",
                                                  name=f"pscl_{u}_{rh}", bufs=2)
                                for hq in range(4):
                                    nc.gpsimd.tensor_scalar_mul(
                                        sc[:, 128 * hq: 128 * (hq + 1)],
                                        psb[:, 128 * hq: 128 * (hq + 1)],
                                        A_rep[:, 4 * h4 + hq: 4 * h4 + hq + 1])
                                nc.gpsimd.tensor_tensor(
                                    sc[:, 0:128], sc[:, 0:128], sc[:, 128:256], ALU.add)
                                nc.gpsimd.tensor_tensor(
                                    sc[:, 256:384], sc[:, 256:384], sc[:, 384:512], ALU.add)
                                nc.gpsimd.tensor_tensor(
                                    acc[:], sc[:, 0:128], sc[:, 256:384], ALU.add)
                            else:
                                nc.vector.tensor_scalar_mul(
                                    acc[:], psb[:, 0:128], A_rep[:, 4 * h4: 4 * h4 + 1]
                                )
                                for hq in range(1, 4):
                                    h3 = 4 * h4 + hq
                                    nc.vector.scalar_tensor_tensor(
                                        acc[:], psb[:, 128 * hq: 128 * (hq + 1)],
                                        A_rep[:, h3: h3 + 1], acc[:],
                                        ALU.mult, ALU.add,
                                    )
                            accs[(rh, h4)] = acc
                    for rh in range(2):
                        rsb = tmppool.tile([128, 128], F32, tag="rsb2",
                                           name=f"rsb_{u}_{rh}")
                        nc.vector.tensor_tensor(rsb[:], accs[(rh, 0)][:],
                                                accs[(rh, 1)][:], ALU.add)
                        nc.vector.tensor_tensor(rsb[:], rsb[:], cb_rep[:], ALU.add)
                        # contiguous block write; host undoes the row permutation
                        # (device row 128*rh+m'' holds rho = 8*(m''%32)+4*rh+m''//32)
                        nc.sync.dma_start(outp[u][128 * rh: 128 * (rh + 1), :], rsb[:])

                emit_partials(0)
                emit_combine(0, use_pool=False)  # fill work; Pool is blocked by collective #2
                emit_partials(1)
                emit_scalars(1)       # waits on collective #2 (hidden behind fill work)
                emit_combine(1, use_pool=True)

    nc.compile()
    return nc


def _prep_inputs(inputs):
    """Host-side: slice/transpose full inputs into per-core in_maps."""
    query = np.asarray(inputs["query"], np.float32)
    Wq = np.asarray(inputs["Wq"], np.float32)
    Wk = np.asarray(inputs["Wk"], np.float32)
    Wv = np.asarray(inputs["Wv"], np.float32)
    Wo = np.asarray(inputs["Wo"], np.float32)
    bq = np.asarray(inputs["bq"], np.float32)
    bk = np.asarray(inputs["bk"], np.float32)
    bv = np.asarray(inputs["bv"], np.float32)
    bo = np.asarray(inputs["bo"], np.float32)
    gn_w = np.asarray(inputs["gn_w"], np.float32)
    gn_b = np.asarray(inputs["gn_b"], np.float32)
    lam = np.asarray(inputs["lam"], np.float32).reshape(1, 1)

    shared = {
        "wqT": np.ascontiguousarray(Wq.T),
        "wkT": np.ascontiguousarray(Wk.T),
        "wvT": np.ascontiguousarray(Wv.T),
        "woT": np.ascontiguousarray(Wo.T),
        "bqT": np.ascontiguousarray(bq.reshape(16, 128).T),
        "bkT": np.ascontiguousarray(bk.reshape(16, 128).T),
        "bv": bv.reshape(1, 1024),
        "bo": bo.reshape(1, 128),
        "gnw2": np.tile(gn_w, 2).reshape(1, 16),
        "gnb2": np.tile(gn_b, 2).reshape(1, 16),
        "lam": lam,
        "eye": np.eye(128, dtype=np.float32),
    }
    in_maps = []
    for c in range(N_CORES):
        blk = query[:, 256 * c: 256 * (c + 1), :]  # [B, 256, 128]
        qT = np.ascontiguousarray(blk.transpose(0, 2, 1))  # [B, 128, 256]
        in_maps.append({"qT": qT, **shared})
    return in_maps


class _Runner:
    """Cached-jit SPMD executor (one trace/compile; cheap repeated calls)."""

    def __init__(self, nc):
        import jax
        from jax.sharding import Mesh, PartitionSpec
        from jax.experimental.shard_map import shard_map
        from concourse.bass2jax import (
            install_neuronx_cc_hook, _bass_exec_p, partition_id_tensor,
        )

        install_neuronx_cc_hook()
        self.jax = jax
        pname = nc.partition_id_tensor.name if nc.partition_id_tensor else None
        in_names, out_names, out_avals, zero_outs = [], [], [], []
        for alloc in nc.m.functions[0].allocations:
            if not isinstance(alloc, mybir.MemoryLocationSet):
                continue
            name = alloc.memorylocations[0].name
            if alloc.kind == "ExternalInput":
                if name != pname:
                    in_names.append(name)
            elif alloc.kind == "ExternalOutput":
                out_names.append(name)
                shape = tuple(alloc.tensor_shape)
                dtype = mybir.dt.np(alloc.dtype)
                out_avals.append(jax.core.ShapedArray(shape, dtype))
                zero_outs.append(np.zeros(shape, dtype))
        self.in_names, self.out_names = in_names, out_names
        n_params = len(in_names)
        all_names = list(in_names) + out_names
        if pname is not None:
            all_names.append(pname)

        def _body(*args):
            operands = list(args)
            if pname is not None:
                operands.append(partition_id_tensor())
            return tuple(_bass_exec_p.bind(
                *operands, out_avals=tuple(out_avals), in_names=tuple(all_names),
                out_names=tuple(out_names), lowering_input_output_aliases=(),
                sim_require_finite=True, sim_require_nnan=True, nc=nc))

        devices = jax.devices()[:N_CORES]
        mesh = Mesh(np.asarray(devices), ("core",))
        nio = n_params + len(out_names)
        self.fn = jax.jit(
            shard_map(_body, mesh=mesh, in_specs=(PartitionSpec("core"),) * nio,
                      out_specs=(PartitionSpec("core"),) * len(out_names),
                      check_rep=False),
            keep_unused=True,
        )
        self.zeros = [
            jax.device_put(np.zeros((N_CORES * z.shape[0], *z.shape[1:]), z.dtype))
            for z in zero_outs
        ]
        self.out_shapes = [tuple(a.shape) for a in out_avals]

    def run(self, in_maps):
        cat = [
            np.concatenate([np.asarray(m[n]) for m in in_maps], axis=0)
            for n in self.in_names
        ]
        # the accelerator intermittently throws a transient
        # NRT_EXEC_UNIT_UNRECOVERABLE (status 101); retry once
        for attempt in range(3):
            try:
                outs = self.fn(*cat, *self.zeros)
                self.jax.block_until_ready(outs)
                outs = [np.asarray(o) for o in outs]
                break
            except Exception:
                if attempt == 2:
                    raise
                import time as _t
                _t.sleep(5.0)
        return [
            {n: outs[i].reshape(N_CORES, *self.out_shapes[i])[c]
             for i, n in enumerate(self.out_names)}
            for c in range(N_CORES)
        ]


_CACHED_NC = None


def kernel(**inputs) -> np.ndarray:
    global _CACHED, _CACHED_NC
    if _CACHED is None:
        _CACHED_NC = build_nc()
        _CACHED = _Runner(_CACHED_NC)
    in_maps = _prep_inputs(inputs)
    results = _CACHED.run(in_maps)
    # device row (rh, m'') holds output row rho = 8*(m'' % 32) + 4*rh + m''//32
    mpp = np.arange(128)
    rho = np.concatenate([8 * (mpp % 32) + 4 * rh + mpp // 32 for rh in (0, 1)])
    inv = np.argsort(rho)
    out = np.empty((B, S, H * D // 8), np.float32)  # (2, 2048, 128)
    for c in range(N_CORES):
        o = results[c]["outp"]  # [B, 256, 128] in device (rh, m'') row order
        for b in range(B):
            out[b, c::8, :] = o[b][inv]  # rows s3 = 8*rho + c
    return out


# revision 8
# speedup vs baseline: 1.0199x; 1.0155x over previous
"""DiffAttention Trainium2 kernel, 8-core SPMD (head-parallel), v2.

Problem (hardcoded): B=2, S=2048, D=128, H=8.
  q = (x@Wq.T+bq).reshape(B,H,S,2D)   # raw reshape: head h <-> rows [256h,256h+256) of proj
  s1 = q1@k1.T; s2 = q2@k2.T; attn = softmax(s1) - lam*softmax(s2)
  out = attn@v -> transpose/reshape -> GroupNorm(H groups) -> *(1-lam) -> concat heads -> @Wo.T+bo

Sharding: core c owns head h=c for both batches (2 units/core). GroupNorm groups
mix all heads -> tiny (32-float) AllGather of partial stats.

Index algebra per unit (b,h), block = proj rows [256h, 256h+256):
  sigma (attn row) = 8r+j, r in [0,256), j in [0,8). We use tau-order sigma' = 256j+r.
  q1T[d, sigma'=256j+r] = qpT_block[f=256j+d, r]   (even 128-col chunks of qp block)
  q2T: odd chunks.  v'[sigma'=256j+r, d] = vp_block[r, 128j+d].
  GroupNorm group g = {sigma': (sigma' mod 256)//32 == g} (32-wide strips).
  Final rows: out[b, 8*rho+h, 128h3+d] = GN(O)[b,h][sigma'=256(rho%8)+32h3+rho//8, d]

v2 changes vs v1:
  - softmax denominators via [128q,1]-output dot matmuls (nearly free on PE)
    + PE transposes into a [1,1024] psum row + DVE reciprocal + gpsimd
    partition_broadcast, replacing full-width ones-matmul accumulations.
  - exp on [128,2048] tiles (half the ACT instruction overhead).
  - bf16 V / E / fT / Wo (output matmuls 4x cheaper); q/k stay f32r.
  - output-stage partials read PSUM directly; collectives scheduled so the
    Pool-queue block lands where PE has a queued qb of slack.
"""

import sys

sys.path.insert(0, "/opt/trn_rl_repo")

import numpy as np

import concourse.bass as bass
import concourse.bacc as bacc
import concourse.mybir as mybir
import concourse.tile as tile

F32 = mybir.dt.float32
F32R = mybir.dt.float32r
BF16 = mybir.dt.bfloat16
AF = mybir.ActivationFunctionType
ALU = mybir.AluOpType

B, S, D, H = 2, 2048, 128, 8
N_CORES = 8
EPS = 1e-5
GROUP_N = float(256 * H * D)  # elements per GroupNorm group

_CACHED = None


def build_nc():
    nc = bacc.Bacc("TRN2", target_bir_lowering=False, debug=False, num_devices=N_CORES)

    # ---- per-core external I/O ----
    qT = nc.dram_tensor("qT", [B, 128, 256], F32, kind="ExternalInput")  # query block.T per batch
    wqT = nc.dram_tensor("wqT", [128, 2048], F32, kind="ExternalInput")
    wkT = nc.dram_tensor("wkT", [128, 2048], F32, kind="ExternalInput")
    wvT = nc.dram_tensor("wvT", [128, 1024], F32, kind="ExternalInput")
    woT = nc.dram_tensor("woT", [1024, 128], F32, kind="ExternalInput")
    bqT = nc.dram_tensor("bqT", [128, 16], F32, kind="ExternalInput")
    bkT = nc.dram_tensor("bkT", [128, 16], F32, kind="ExternalInput")
    bv = nc.dram_tensor("bv", [1, 1024], F32, kind="ExternalInput")
    bo = nc.dram_tensor("bo", [1, 128], F32, kind="ExternalInput")
    gnw2 = nc.dram_tensor("gnw2", [1, 16], F32, kind="ExternalInput")  # tiled x2 (b,g)
    gnb2 = nc.dram_tensor("gnb2", [1, 16], F32, kind="ExternalInput")
    lam = nc.dram_tensor("lam", [1, 1], F32, kind="ExternalInput")
    eye = nc.dram_tensor("eye", [128, 128], F32, kind="ExternalInput")
    outp = nc.dram_tensor("outp", [B, 256, 128], F32, kind="ExternalOutput")

    with tile.TileContext(nc) as tc:
        with (
            tc.tile_pool(name="const", bufs=1) as cpool,
            tc.tile_pool(name="proj", bufs=2) as projpool,
            tc.tile_pool(name="epool", bufs=4) as epool,
            tc.tile_pool(name="otpool", bufs=4) as otpool,
            tc.tile_pool(name="tmp", bufs=2) as tmppool,
            tc.tile_pool(name="dram", bufs=1, space="DRAM") as dram,
        ):
            # ---- load constants / weights (qT first: projections need it) ----
            qt_sb = []
            for u in range(B):
                q = cpool.tile([128, 256], F32, name=f"qt_sb{u}")
                nc.sync.dma_start(q[:], qT[u])
                qt_sb.append(q)

            # small constants go on the gpsimd DMA queue so they don't delay
            # the big weight DMAs on the sync queue
            bq_sb = cpool.tile([128, 16], F32)
            bk_sb = cpool.tile([128, 16], F32)
            nc.gpsimd.dma_start(bq_sb[:], bqT[:])
            nc.gpsimd.dma_start(bk_sb[:], bkT[:])
            bv_sb = cpool.tile([1, 1024], F32)
            nc.gpsimd.dma_start(bv_sb[:], bv[:])
            bo_sb = cpool.tile([1, 128], F32)
            nc.gpsimd.dma_start(bo_sb[:], bo[:])
            gnw_sb = cpool.tile([1, 16], F32)
            gnb_sb = cpool.tile([1, 16], F32)
            nc.gpsimd.dma_start(gnw_sb[:], gnw2[:])
            nc.gpsimd.dma_start(gnb_sb[:], gnb2[:])
            lam_sb = cpool.tile([1, 1], F32)
            nc.gpsimd.dma_start(lam_sb[:], lam[:])
            eye_sb = cpool.tile([128, 128], F32)
            nc.gpsimd.dma_start(eye_sb[:], eye[:])

            # weights loaded and f32r-rounded in 1024-col pieces so projections
            # can start before all input DMA completes.
            wq_rh, wk_rh = [], []
            wv_r = cpool.tile([128, 1024], F32R)
            wpieces = (
                [("wq", wqT, wq_rh, 0), ("wk", wkT, wk_rh, 0),
                 ("wk", wkT, wk_rh, 1), ("wv", wvT, None, 0),
                 ("wq", wqT, wq_rh, 1)]
            )
            for (wnm, dram_w, lst, half) in wpieces:
                wsc = projpool.tile([128, 1024], F32, tag="wsc", name=f"wsc_{wnm}{half}")
                split = (wnm == "wk" and half == 1)
                if split:
                    # 512-col pieces: the dripped j8-11 k-projection can start
                    # ~0.8us earlier
                    nc.sync.dma_start(wsc[:, 0:512], dram_w[:, 1024:1536])
                else:
                    nc.sync.dma_start(wsc[:], dram_w[:, 1024 * half: 1024 * (half + 1)])
                if lst is None:
                    nc.vector.tensor_copy(wv_r[:], wsc[:])
                else:
                    wr = cpool.tile([128, 1024], F32R, name=f"{wnm}_r{half}")
                    if split:
                        nc.vector.tensor_copy(wr[:, 0:512], wsc[:, 0:512])
                        nc.sync.dma_start(wsc[:, 512:1024], dram_w[:, 1536:2048])
                        nc.vector.tensor_copy(wr[:, 512:1024], wsc[:, 512:1024])
                    else:
                        nc.vector.tensor_copy(wr[:], wsc[:])
                    lst.append(wr)
            qt_r = []
            for u in range(B):
                qr = cpool.tile([128, 256], F32R, name=f"qt_r{u}")
                nc.vector.tensor_copy(qr[:], qt_sb[u][:])
                qt_r.append(qr)
            lam_rep = cpool.tile([128, 1], F32)
            nc.gpsimd.partition_broadcast(lam_rep[:], lam_sb[:])
            oml = cpool.tile([1, 1], F32)
            nc.vector.tensor_scalar(oml[:], lam_sb[:], -1.0, 1.0, ALU.mult, ALU.add)
            bv_rep = cpool.tile([128, 1024], F32)
            nc.gpsimd.partition_broadcast(bv_rep[:], bv_sb[:])

            ones_f32 = cpool.tile([128, 1], F32)
            nc.vector.memset(ones_f32[:], 1.0)
            ones2_f32 = cpool.tile([128, 2], F32)
            nc.vector.memset(ones2_f32[:], 1.0)
            ones_col = cpool.tile([128, 2], F32R)
            nc.vector.tensor_copy(ones_col[:], ones2_f32[:])

            # Wo chunks in bf16 (moving operand of the output matmuls);
            # DMAs queued after the projection weights
            wo_bf = []
            for h3 in range(8):
                wsc = projpool.tile([128, 128], F32, tag="wosc", name=f"wosc{h3}")
                nc.sync.dma_start(wsc[:], woT[128 * h3: 128 * (h3 + 1), :])
                w = cpool.tile([128, 128], F32, name=f"wo_bf{h3}")
                nc.vector.tensor_copy(w[:], wsc[:])
                wo_bf.append(w)

            p2_tiles = {0: [], 1: []}
            fT_sb = []
            ot_refs = {0: [], 1: []}  # otq tiles per unit (for deferred re-layout)

            cc_in = [dram.tile([1, 16], F32, name=f"cc_in{u}") for u in range(B)]
            cc_out = [dram.tile([8, 16], F32, addr_space="Shared", name=f"cc_out{u}")
                      for u in range(B)]
            gath = [tmppool.tile([1, 128], F32, tag="gath", name=f"gath_{u}", bufs=2)
                    for u in range(B)]
            scal = [{}, {}]

            def ptree(dst, src_tile, parts, width, nm, eng=None):
                eng = eng or nc.vector
                # partition-axis sum: DVE shift-copy + add down to 32 partitions
                # (TT needs equal base partitions; slices are 32-aligned),
                # then one gpsimd C-axis reduce for the final 32 -> 1.
                cur = src_tile
                while parts > 32:
                    parts //= 2
                    sh = tmppool.tile([parts, width], F32, tag=f"ps{parts}",
                                      name=f"ps_{nm}_{parts}", bufs=4)
                    eng.tensor_copy(sh[:], cur[parts: 2 * parts, :])
                    t = tmppool.tile([parts, width], F32, tag=f"pt{parts}",
                                     name=f"pt_{nm}_{parts}", bufs=4)
                    eng.tensor_tensor(t[:], cur[0:parts, :], sh[:], ALU.add)
                    cur = t
                nc.gpsimd.tensor_reduce(dst, cur[:], mybir.AxisListType.C, ALU.add)

            def emit_stats_export(u):
                stats_u = tmppool.tile([1, 16], F32, tag="stats", name=f"stats_{u}", bufs=2)
                for si, p2 in enumerate(p2_tiles[u]):
                    ptree(stats_u[:, 8 * si: 8 * si + 8], p2, 128, 8, f"st{u}{si}",
                          eng=(nc.vector, nc.gpsimd)[si])
                nc.sync.dma_start(cc_in[u][:], stats_u[:])
                nc.gpsimd.collective_compute(
                    "AllGather", ALU.bypass,
                    replica_groups=[list(range(N_CORES))],
                    ins=[cc_in[u][:]], outs=[cc_out[u][:]],
                )
                nc.gpsimd.dma_start(gath[u][:],
                                     cc_out[u][:].rearrange("a b -> (a b)").unsqueeze(0))

            def emit_scalars(u, cb_mm=None):
                # global stats for batch u -> A_rep[128,8], cb_rep[128,128]
                t = lambda nm: tmppool.tile([1, 8], F32, tag=nm, name=f"{nm}_{u}", bufs=2)
                glob = tmppool.tile([1, 16], F32, tag="globsb", name=f"glob_{u}", bufs=2)
                nc.vector.tensor_reduce(
                    glob[:], gath[u].rearrange("p (a b) -> p b a", a=8, b=16),
                    mybir.AxisListType.X, ALU.add,
                )
                moments = tmppool.tile([1, 16], F32, tag="mom", name=f"mom_{u}", bufs=2)
                nc.vector.tensor_scalar_mul(moments[:], glob[:], 1.0 / GROUP_N)
                mean, ex2 = moments[:, 0:8], moments[:, 8:16]
                var, veps = t("var"), t("veps")
                nc.vector.tensor_tensor(var[:], mean, mean, ALU.mult)
                nc.vector.tensor_tensor(var[:], ex2, var[:], ALU.subtract)
                nc.vector.tensor_scalar_add(veps[:], var[:], EPS)
                # rsqrt fully on DVE (ACT Sqrt would thrash the exp table set):
                # quake seed + 2 Newton steps
                I32 = mybir.dt.int32
                ti = tmppool.tile([1, 8], I32, tag="rsqi", name=f"rsqi_{u}", bufs=2)
                nc.vector.tensor_scalar(
                    ti[:], veps[:].bitcast(I32), 1, None, ALU.arith_shift_right
                )
                nc.vector.tensor_scalar(ti[:], ti[:], -1, 0x5F3759DF, ALU.mult, ALU.add)
                rstd, hf, nt = t("rstd"), t("hf"), t("nt")
                nc.vector.tensor_copy(rstd[:], ti[:].bitcast(F32))
                nc.vector.tensor_scalar_mul(hf[:], veps[:], 0.5)
                for _ in range(2):
                    nc.vector.tensor_tensor(nt[:], rstd[:], rstd[:], ALU.mult)
                    nc.vector.tensor_tensor(nt[:], nt[:], hf[:], ALU.mult)
                    nc.vector.tensor_scalar(nt[:], nt[:], -1.0, 1.5, ALU.mult, ALU.add)
                    nc.vector.tensor_tensor(rstd[:], rstd[:], nt[:], ALU.mult)
                A, Bc = t("A"), t("Bc")
                nc.vector.tensor_tensor(A[:], rstd[:], gnw_sb[:, 0:8], ALU.mult)
                nc.vector.tensor_tensor(Bc[:], mean, A[:], ALU.mult)
                nc.vector.tensor_tensor(Bc[:], gnb_sb[:, 0:8], Bc[:], ALU.subtract)
                nc.vector.tensor_scalar_mul(A[:], A[:], oml[:, 0:1])
                nc.vector.tensor_scalar_mul(Bc[:], Bc[:], oml[:, 0:1])
                A_rep = tmppool.tile([128, 8], F32, tag="A_rep", name=f"A_rep{u}", bufs=2)
                nc.gpsimd.partition_broadcast(A_rep[:], A[:])
                cb = tmppool.tile([1, 128], F32, tag="cb", name=f"cb_{u}", bufs=2)
                if cb_mm is None:
                    # serial stt chain (fine off the critical path)
                    nc.vector.tensor_scalar_mul(cb[:], wsum_sb[:, 0:128], Bc[:, 0:1])
                    for h3 in range(1, 8):
                        nc.vector.scalar_tensor_tensor(
                            cb[:], wsum_sb[:, 128 * h3: 128 * (h3 + 1)],
                            Bc[:, h3: h3 + 1], cb[:], ALU.mult, ALU.add,
                        )
                    nc.vector.tensor_tensor(cb[:], cb[:], bo_sb[:], ALU.add)
                else:
                    # critical path: cb = Bc(1x8) @ wsum_p8(8x128) via PE
                    # (transpose Bc to a column first), then + bo
                    ps_pool = cb_mm
                    btp = ps_pool.tile([8, 8], F32, tag="btp", name=f"btp_{u}")
                    nc.tensor.matmul(btp[:, 0:1], Bc[:], ones_f32[0:1, 0:1],
                                     is_transpose=True, start=True, stop=True)
                    bcol = tmppool.tile([8, 1], F32, tag="bcol", name=f"bcol_{u}", bufs=2)
                    nc.vector.tensor_copy(bcol[:], btp[:, 0:1])
                    cbp = ps_pool.tile([1, 128], F32, tag="cbp", name=f"cbp_{u}")
                    nc.tensor.matmul(cbp[:], bcol[:], wsum_p8[:], start=True, stop=True)
                    nc.vector.tensor_tensor(cb[:], cbp[:], bo_sb[:], ALU.add)
                cb_rep = tmppool.tile([128, 128], F32, tag="cb_rep", name=f"cbr_{u}", bufs=2)
                nc.gpsimd.partition_broadcast(cb_rep[:], cb[:])
                scal[u] = {"A_rep": A_rep, "cb_rep": cb_rep}

            qk = {}
            vts = []
            wsum_sb = cpool.tile([1, 1024], F32)
            wsum_f8 = cpool.tile([8, 128], F32)
            wsum_p8 = cpool.tile([8, 128], F32)

            # ================= attention-phase PSUM pools =================
            with (
                tc.tile_pool(name="ps_sgrp", bufs=2, space="PSUM") as ps_sgrp,
                tc.tile_pool(name="ps_u", bufs=1, space="PSUM") as ps_u,
                tc.tile_pool(name="ps_rdot", bufs=1, space="PSUM") as ps_rdot,
                tc.tile_pool(name="ps_rrow", bufs=1, space="PSUM") as ps_rrow,
            ):
                def emit_wsum():
                    # Wo column sums (for the GN-beta term), on the sgrp psum ring
                    wps = ps_sgrp.tile([128, 1024], F32, tag="sgrp", name="wps")
                    for h3 in range(8):
                        nc.tensor.matmul(wps[0:1, 128 * h3: 128 * (h3 + 1)],
                                         ones_f32[:, 0:1], wo_bf[h3][:],
                                         start=True, stop=True)
                    nc.vector.tensor_copy(wsum_sb[:, 0:1024], wps[0:1, 0:1024])
                    # row-per-h3 bf16 copy (for the PE-side cb matmul at the tail)
                    nc.sync.dma_start(
                        wsum_f8[:],
                        wsum_sb[:].rearrange("p (a b) -> (p a) b", a=8, b=128))
                    nc.vector.tensor_copy(wsum_p8[:], wsum_f8[:])

                def alloc_qk(u):
                    for nm in ("q1", "q2"):
                        qk[(u, nm)] = [
                            projpool.tile([128, 512], F32R, tag=f"{nm}t",
                                          name=f"{nm}t_{u}_{qb}", bufs=8)
                            for qb in range(4)
                        ]
                    for nm in ("k1", "k2"):
                        qk[(u, nm)] = [
                            projpool.tile([128, 1024], F32R, tag=f"{nm}t",
                                          name=f"{nm}t_{u}_{hh}", bufs=4)
                            for hh in range(2)
                        ]
                    vts.append(projpool.tile([128, 2048], F32R, tag="vp",
                                             name=f"vp_{u}", bufs=2))

                def proj_blocks(u, use_act=False):
                    # generator of closures: 10 psum-ring blocks per unit
                    # (4x q, 4x k with four 256-col chunks each; 2x v halves)
                    def qkblk(blk):
                        def emit():
                            ps = ps_sgrp.tile([128, 1024], F32, tag="sgrp",
                                              name=f"pp_{u}_{blk}")
                            wrh = wq_rh if blk < 4 else wk_rh
                            for c in range(4):
                                j = 4 * (blk % 4) + c
                                nc.tensor.matmul(
                                    ps[:, 256 * c: 256 * (c + 1)],
                                    wrh[j // 8][:, 128 * (j % 8): 128 * (j % 8 + 1)],
                                    qt_r[u][:], start=True, stop=True,
                                )
                            for c in range(4):
                                j = 4 * (blk % 4) + c
                                if blk < 4:
                                    dst = qk[(u, "q1" if j % 2 == 0 else "q2")][j // 4]
                                    col = 256 * ((j // 2) % 2)
                                    bias = bq_sb[:, j: j + 1]
                                else:
                                    dst = qk[(u, "k1" if j % 2 == 0 else "k2")][j // 8]
                                    col = 256 * ((j // 2) % 4)
                                    bias = bk_sb[:, j: j + 1]
                                if use_act and c % 2 == 0:
                                    # head phase: ACT is idle; Identity+bias is
                                    # in every table set (no exp-table thrash)
                                    nc.scalar.activation(
                                        dst[:, col: col + 256],
                                        ps[:, 256 * c: 256 * (c + 1)],
                                        AF.Identity, bias=bias,
                                    )
                                else:
                                    nc.vector.tensor_scalar_add(
                                        dst[:, col: col + 256],
                                        ps[:, 256 * c: 256 * (c + 1)], bias
                                    )
                        return emit

                    def vblk(rc):
                        def emit():
                            vt = vts[u]
                            ps = ps_sgrp.tile([128, 1024], F32, tag="sgrp",
                                              name=f"ppv_{u}_{rc}")
                            for fh in range(2):
                                nc.tensor.matmul(
                                    ps[:, 512 * fh: 512 * (fh + 1)],
                                    qt_r[u][:, 128 * rc: 128 * (rc + 1)],
                                    wv_r[:, 512 * fh: 512 * (fh + 1)],
                                    start=True, stop=True,
                                )
                            for fh in range(2):
                                nc.vector.tensor_tensor(
                                    vt[:, 1024 * rc + 512 * fh:
                                       1024 * rc + 512 * fh + 512],
                                    ps[:, 512 * fh: 512 * (fh + 1)],
                                    bv_rep[:, 512 * fh: 512 * (fh + 1)], ALU.add,
                                )
                        return emit

                    # pre: minimum to start attention qb0 (q j0-3, k j0-7, v);
                    # drip: the rest, fed into attention slots as DMA lands
                    pre = [qkblk(0), qkblk(4), qkblk(5), vblk(0), vblk(1)]
                    drip = [qkblk(6), qkblk(7), qkblk(1), qkblk(2), qkblk(3)]
                    return pre, drip

                def attention_unit(u, boundary_cb, slot_cb):
                    q1l, q2l = qk[(u, "q1")], qk[(u, "q2")]
                    kls = (qk[(u, "k1")], qk[(u, "k2")])
                    vt = vts[u]

                    def vchunk(kc):
                        return vt[:, 1024 * (kc % 2) + 128 * (kc // 2):
                                  1024 * (kc % 2) + 128 * (kc // 2) + 128]

                    fT = tmppool.tile([128, 2048], F32, tag="fT", name=f"fT_{u}")
                    fT_sb.append(fT)
                    p1a = tmppool.tile([128, 16], F32, tag="p1a", name=f"p1a_{u}")
                    p1b = tmppool.tile([128, 16], F32, tag="p1b", name=f"p1b_{u}")

                    LAG = 3  # consume items this many exp-slots behind issue
                    state = {}

                    def emit_r_chain(qb, m):
                        # per-branch: [128,8] dots -> f32 transpose into a
                        # [1,512] psum row -> reciprocal -> partition bcast
                        st = state[qb]
                        r_sb = tmppool.tile([128, 8], F32, tag=f"rsb{m}",
                                            name=f"rsb_{u}_{qb}_{m}", bufs=2)
                        nc.vector.tensor_copy(r_sb[:], st["rdot"][:, 8 * m: 8 * m + 8])
                        rrow = ps_rrow.tile([1, 512], F32, tag="rr",
                                            name=f"rr_{u}_{qb}_{m}")
                        for s4 in range(4):
                            nc.tensor.matmul(
                                rrow[0:1, 128 * s4: 128 * (s4 + 1)],
                                r_sb[:, 2 * s4: 2 * s4 + 1], eye_sb[:],
                                is_transpose=True, start=True, stop=True,
                            )
                        r_inv = tmppool.tile([1, 512], F32, tag=f"rinv{m}",
                                             name=f"rinv_{u}_{qb}_{m}", bufs=2)
                        nc.vector.reciprocal(r_inv[:], rrow[:])
                        rr = tmppool.tile([128, 512], F32, tag=f"r{m}rep",
                                          name=f"r{m}rep_{u}_{qb}", bufs=2)
                        nc.gpsimd.partition_broadcast(rr[:], r_inv[:])
                        st["rrep"][m] = rr

                    def finish_qb(qb):
                        # O = U1/R1 - lam*U2/R2  (t1 emitted early, in-branch)
                        st = state[qb]
                        t2 = tmppool.tile([128, 512], F32, tag="t2", name=f"t2_{u}_{qb}")
                        nc.vector.scalar_tensor_tensor(
                            t2[:], st["u2"][:], lam_rep[:, 0:1], st["rrep"][1][:],
                            ALU.mult, ALU.mult
                        )
                        otq = otpool.tile([128, 512], F32, tag="ot", name=f"ot_{u}_{qb}")
                        nc.vector.tensor_tensor(otq[:], st["t1"][:], t2[:], ALU.subtract)
                        ot_refs[u].append(otq)

                        # incremental GroupNorm partial stats for this q-block
                        # (free-dim layout within the block: (j2, g8, r32))
                        osl = otq.rearrange("p (j g r) -> p j g r", j=2, g=8, r=32)
                        red = tmppool.tile([128, 16], F32, tag="red", name=f"red_{u}_{qb}")
                        nc.vector.tensor_reduce(red[:], osl, mybir.AxisListType.X, ALU.add)
                        if qb == 0:
                            nc.vector.tensor_copy(p1a[:], red[:])
                        else:
                            nc.vector.tensor_tensor(p1a[:], p1a[:], red[:], ALU.add)
                        sq5 = tmppool.tile([128, 512], F32, tag="t1", name=f"sq5_{u}_{qb}")
                        sq_eng = nc.gpsimd if (u == 1 and qb == 3) else nc.vector
                        sq_eng.tensor_tensor(sq5[:], otq[:], otq[:], ALU.mult)
                        redb = tmppool.tile([128, 16], F32, tag="redb", name=f"redb_{u}_{qb}")
                        nc.vector.tensor_reduce(
                            redb[:], sq5.rearrange("p (j g r) -> p j g r", j=2, g=8, r=32),
                            mybir.AxisListType.X, ALU.add,
                        )
                        if qb == 0:
                            nc.vector.tensor_copy(p1b[:], redb[:])
                        else:
                            nc.vector.tensor_tensor(p1b[:], p1b[:], redb[:], ALU.add)
                        if u == 0 or qb < 2:
                            emit_relayout(u, qb)
                        boundary_cb(qb)

                    def consume(item):
                        qb, m, g, eg = item
                        if m == 0 and g == 0:
                            # lazy per-qb psum state: allocated only once the
                            # previous qb's readers are already emitted (FIFO)
                            rdot = ps_rdot.tile([128, 16], F32, tag="rd",
                                                name=f"rd_{u}_{qb}")
                            nc.vector.memset(rdot[:], 0.0)
                            state[qb] = {
                                "u1": ps_u.tile([128, 512], F32, tag="u1",
                                                name=f"u1_{u}_{qb}"),
                                "u2": ps_u.tile([128, 512], F32, tag="u2",
                                                name=f"u2_{u}_{qb}"),
                                "rdot": rdot, "rrep": [None, None], "t1": None,
                            }
                        st = state[qb]
                        uacc = (st["u1"], st["u2"])[m]
                        for c in range(2):
                            kc = 2 * g + c
                            nc.tensor.matmul(
                                uacc[:], vchunk(kc), eg[:, 512 * c: 512 * (c + 1)],
                                start=(g == 0 and c == 0),
                                stop=(g == 7 and c == 1),
                            )
                            for sl4 in range(4):
                                col = 2 * (4 * m + sl4)
                                nc.tensor.matmul(
                                    st["rdot"][:, col: col + 2],
                                    eg[:, 512 * c + 128 * sl4: 512 * c + 128 * sl4 + 128],
                                    ones_col[:],
                                    start=False, stop=False, skip_group_check=True,
                                )
                        if g == 7:
                            emit_r_chain(qb, m)
                            if m == 0:
                                # t1 = U1/R1 early, while branch 2 streams
                                t1 = tmppool.tile([128, 512], F32, tag="t1",
                                                  name=f"t1_{u}_{qb}")
                                nc.vector.tensor_tensor(t1[:], st["u1"][:],
                                                        st["rrep"][0][:], ALU.mult)
                                st["t1"] = t1
                            else:
                                finish_qb(qb)

                    # flat 64-slot pipeline: the sgrp/exp stream never pauses
                    # at qb boundaries; qb bookkeeping rides inside the lagged
                    # consume stream
                    pending = []
                    for slot in range(64):
                        qb, rem = divmod(slot, 16)
                        m, g = divmod(rem, 8)
                        ql = (q1l, q2l)[m][qb]
                        kl = kls[m]
                        sg = ps_sgrp.tile([128, 1024], F32, tag="sgrp",
                                          name=f"sg_{u}_{qb}_{m}_{g}")
                        for c in range(2):
                            kc = 2 * g + c
                            nc.tensor.matmul(
                                sg[:, 512 * c: 512 * (c + 1)],
                                kl[kc // 8][:, 128 * (kc % 8): 128 * (kc % 8 + 1)],
                                ql[:], start=True, stop=True,
                            )
                        eg = epool.tile([128, 1024], F32R, tag="e",
                                        name=f"e_{u}_{qb}_{m}_{g}")
                        nc.scalar.activation(eg[:], sg[:], AF.Exp)
                        pending.append((qb, m, g, eg))
                        if len(pending) > LAG:
                            consume(pending.pop(0))
                        slot_cb(qb, rem)
                    for item in pending:
                        consume(item)

                    # fold (j mod 2) pairs -> per-group partials
                    for si, p1x in enumerate((p1a, p1b)):
                        p2 = tmppool.tile([128, 8], F32, tag="p2", name=f"p2_{u}_{si}")
                        nc.vector.tensor_reduce(
                            p2[:], p1x.rearrange("p (j g) -> p g j", j=2, g=8),
                            mybir.AxisListType.X, ALU.add,
                        )
                        p2_tiles[u].append(p2)

                def emit_relayout(u, qb):
                    # re-layout into fT (j-pair slab for this qb);
                    # src re-viewed g-outer to match the dst iteration order
                    fT = fT_sb[u]
                    fv4 = fT.rearrange("p (g j r) -> p g j r", g=8, j=8, r=32)
                    otq = ot_refs[u][qb]
                    nc.vector.tensor_copy(
                        fv4[:, :, 2 * qb: 2 * qb + 2, :],
                        otq.rearrange("p (j g r) -> p g j r", j=2, g=8, r=32),
                    )

                # ================= main schedule =================
                alloc_qk(0)
                alloc_qk(1)
                pre0, _ = proj_blocks(0)
                for blk in pre0:
                    blk()
                _, drip0 = proj_blocks(0)
                pre1, drip1 = proj_blocks(1)
                drip = drip0 + pre1 + drip1  # u0 stragglers first, then all of u1
                # slots at which to emit one proj block into u0's attention:
                # k j8-15 blocks early (needed by qb0 slot 4/6), then every 3rd
                drip_slots = [0, 2] + list(range(5, 64, 3))

                def u0_slot(qb, slot):
                    g = 16 * qb + slot
                    if drip and drip_slots and g >= drip_slots[0]:
                        drip_slots.pop(0)
                        drip.pop(0)()

                attention_unit(0, lambda qb: None, u0_slot)
                while drip:
                    drip.pop(0)()

                def u1_boundary(qb):
                    if qb == 0:
                        # unit-0 stats export + collective #1: lands on the Pool
                        # queue right after qb0's broadcasts; PE has a full qb of
                        # queued work to ride out the 15us Pool block.
                        emit_stats_export(0)
                        emit_wsum()
                    if qb == 2:
                        # unit-0 GN scalars (collective #1 landed long ago)
                        emit_scalars(0)

                attention_unit(1, u1_boundary, lambda qb, slot: None)
                # stats export for unit 1: as early as possible -> tail collective
                emit_stats_export(1)

            # ============== output stage (fills collective #2 window) ==============
            with tc.tile_pool(name="ps_out", bufs=1, space="PSUM") as ps_out:
                for qb in (2, 3):
                    emit_relayout(1, qb)

                P_sb = {}

                def emit_partials(u):
                    # 4 P outputs packed per [128,512] psum bank tile, then
                    # drained to SBUF (cheap to re-read; fills collective #2)
                    fT = fT_sb[u]
                    for rh in range(2):
                        for h4 in range(2):
                            pps = ps_out.tile([128, 512], F32, tag="P", bufs=4,
                                              name=f"pps_{u}_{rh}_{h4}")
                            for hq in range(4):
                                h3 = 4 * h4 + hq
                                lhsT = fT[:, 256 * h3 + 128 * rh: 256 * h3 + 128 * rh + 128]
                                nc.tensor.matmul(pps[:, 128 * hq: 128 * (hq + 1)],
                                                 lhsT, wo_bf[h3][:], start=True, stop=True)
                            P_sb[(u, rh, h4)] = pps

                def emit_combine(u, use_pool):
                    # result = sum_h3 A[u,h3]*P[u][rh][h3] + cb_rep[u]
                    # split per rh into a DVE half-chain (h3 0-3) and a Pool
                    # half-chain (h3 4-7), joined by one add
                    A_rep = scal[u]["A_rep"]
                    cb_rep = scal[u]["cb_rep"]
                    accs = {}
                    for rh in range(2):
                        for h4 in range(2):
                            psb = P_sb[(u, rh, h4)]
                            acc = tmppool.tile([128, 128], F32, tag=f"acc{h4}",
                                               name=f"acc_{u}_{rh}_{h4}")
                            if False:
                                # Pool path: tsm x4 + tt tree (no stt on Pool)
                                sc = tmppool.tile([128, 512], F32, tag="pscl",
                                                  name=f"pscl_{u}_{rh}", bufs=2)
                                for hq in range(4):
                                    nc.gpsimd.tensor_scalar_mul(
                                        sc[:, 128 * hq: 128 * (hq + 1)],
                                        psb[:, 128 * hq: 128 * (hq + 1)],
                                        A_rep[:, 4 * h4 + hq: 4 * h4 + hq + 1])
                                nc.gpsimd.tensor_tensor(
                                    sc[:, 0:128], sc[:, 0:128], sc[:, 128:256], ALU.add)
                                nc.gpsimd.tensor_tensor(
                                    sc[:, 256:384], sc[:, 256:384], sc[:, 384:512], ALU.add)
                                nc.gpsimd.tensor_tensor(
                                    acc[:], sc[:, 0:128], sc[:, 256:384], ALU.add)
                            else:
                                nc.vector.tensor_scalar_mul(
                                    acc[:], psb[:, 0:128], A_rep[:, 4 * h4: 4 * h4 + 1]
                                )
                                for hq in range(1, 4):
                                    h3 = 4 * h4 + hq
                                    nc.vector.scalar_tensor_tensor(
                                        acc[:], psb[:, 128 * hq: 128 * (hq + 1)],
                                        A_rep[:, h3: h3 + 1], acc[:],
                                        ALU.mult, ALU.add,
                                    )
                            accs[(rh, h4)] = acc
                    for rh in range(2):
                        rsb = tmppool.tile([128, 128], F32, tag="rsb2",
                                           name=f"rsb_{u}_{rh}")
                        nc.vector.tensor_tensor(rsb[:], accs[(rh, 0)][:],
                                                accs[(rh, 1)][:], ALU.add)
                        nc.vector.tensor_tensor(rsb[:], rsb[:], cb_rep[:], ALU.add)
                        # contiguous block write; host undoes the row permutation
                        # (device row 128*rh+m'' holds rho = 8*(m''%32)+4*rh+m''//32)
                        nc.sync.dma_start(outp[u][128 * rh: 128 * (rh + 1), :], rsb[:])

                emit_partials(0)
                emit_combine(0, use_pool=False)  # fill work; Pool is blocked by collective #2
                emit_partials(1)
                emit_scalars(1)       # waits on collective #2 (hidden behind fill work)
                emit_combine(1, use_pool=True)

    nc.compile()
    return nc


def _prep_inputs(inputs):
    """Host-side: slice/transpose full inputs into per-core in_maps."""
    query = np.asarray(inputs["query"], np.float32)
    Wq = np.asarray(inputs["Wq"], np.float32)
    Wk = np.asarray(inputs["Wk"], np.float32)
    Wv = np.asarray(inputs["Wv"], np.float32)
    Wo = np.asarray(inputs["Wo"], np.float32)
    bq = np.asarray(inputs["bq"], np.float32)
    bk = np.asarray(inputs["bk"], np.float32)
    bv = np.asarray(inputs["bv"], np.float32)
    bo = np.asarray(inputs["bo"], np.float32)
    gn_w = np.asarray(inputs["gn_w"], np.float32)
    gn_b = np.asarray(inputs["gn_b"], np.float32)
    lam = np.asarray(inputs["lam"], np.float32).reshape(1, 1)

    shared = {
        "wqT": np.ascontiguousarray(Wq.T),
        "wkT": np.ascontiguousarray(Wk.T),
        "wvT": np.ascontiguousarray(Wv.T),
        "woT": np.ascontiguousarray(Wo.T),
        "bqT": np.ascontiguousarray(bq.reshape(16, 128).T),
        "bkT": np.ascontiguousarray(bk.reshape(16, 128).T),
        "bv": bv.reshape(1, 1024),
        "bo": bo.reshape(1, 128),
        "gnw2": np.tile(gn_w, 2).reshape(1, 16),
        "gnb2": np.tile(gn_b, 2).reshape(1, 16),
        "lam": lam,
        "eye": np.eye(128, dtype=np.float32),
    }
    in_maps = []
    for c in range(N_CORES):
        blk = query[:, 256 * c: 256 * (c + 1), :]  # [B, 256, 128]
        qT = np.ascontiguousarray(blk.transpose(0, 2, 1))  # [B, 128, 256]
        in_maps.append({"qT": qT, **shared})
    return in_maps


class _Runner:
    """Cached-jit SPMD executor (one trace/compile; cheap repeated calls)."""

    def __init__(self, nc):
        import jax
        from jax.sharding import Mesh, PartitionSpec
        from jax.experimental.shard_map import shard_map
        from concourse.bass2jax import (
            install_neuronx_cc_hook, _bass_exec_p, partition_id_tensor,
        )

        install_neuronx_cc_hook()
        self.jax = jax
        pname = nc.partition_id_tensor.name if nc.partition_id_tensor else None
        in_names, out_names, out_avals, zero_outs = [], [], [], []
        for alloc in nc.m.functions[0].allocations:
            if not isinstance(alloc, mybir.MemoryLocationSet):
                continue
            name = alloc.memorylocations[0].name
            if alloc.kind == "ExternalInput":
                if name != pname:
                    in_names.append(name)
            elif alloc.kind == "ExternalOutput":
                out_names.append(name)
                shape = tuple(alloc.tensor_shape)
                dtype = mybir.dt.np(alloc.dtype)
                out_avals.append(jax.core.ShapedArray(shape, dtype))
                zero_outs.append(np.zeros(shape, dtype))
        self.in_names, self.out_names = in_names, out_names
        n_params = len(in_names)
        all_names = list(in_names) + out_names
        if pname is not None:
            all_names.append(pname)

        def _body(*args):
            operands = list(args)
            if pname is not None:
                operands.append(partition_id_tensor())
            return tuple(_bass_exec_p.bind(
                *operands, out_avals=tuple(out_avals), in_names=tuple(all_names),
                out_names=tuple(out_names), lowering_input_output_aliases=(),
                sim_require_finite=True, sim_require_nnan=True, nc=nc))

        devices = jax.devices()[:N_CORES]
        mesh = Mesh(np.asarray(devices), ("core",))
        nio = n_params + len(out_names)
        self.fn = jax.jit(
            shard_map(_body, mesh=mesh, in_specs=(PartitionSpec("core"),) * nio,
                      out_specs=(PartitionSpec("core"),) * len(out_names),
                      check_rep=False),
            keep_unused=True,
        )
        self.zeros = [
            jax.device_put(np.zeros((N_CORES * z.shape[0], *z.shape[1:]), z.dtype))
            for z in zero_outs
        ]
        self.out_shapes = [tuple(a.shape) for a in out_avals]

    def run(self, in_maps):
        cat = [
            np.concatenate([np.asarray(m[n]) for m in in_maps], axis=0)
            for n in self.in_names
        ]
        # the accelerator intermittently throws a transient
        # NRT_EXEC_UNIT_UNRECOVERABLE (status 101); retry once
        for attempt in range(3):
            try:
                outs = self.fn(*cat, *self.zeros)
                self.jax.block_until_ready(outs)
                outs = [np.asarray(o) for o in outs]
                break
            except Exception:
                if attempt == 2:
                    raise
                import time as _t
                _t.sleep(5.0)
        return [
            {n: outs[i].reshape(N_CORES, *self.out_shapes[i])[c]
             for i, n in enumerate(self.out_names)}
            for c in range(N_CORES)
        ]


_CACHED_NC = None


def kernel(**inputs) -> np.ndarray:
    global _CACHED, _CACHED_NC
    if _CACHED is None:
        _CACHED_NC = build_nc()
        _CACHED = _Runner(_CACHED_NC)
    in_maps = _prep_inputs(inputs)
    results = _CACHED.run(in_maps)
    # device row (rh, m'') holds output row rho = 8*(m'' % 32) + 4*rh + m''//32
    mpp = np.arange(128)
    rho = np.concatenate([8 * (mpp % 32) + 4 * rh + mpp // 32 for rh in (0, 1)])
    inv = np.argsort(rho)
    out = np.empty((B, S, H * D // 8), np.float32)  # (2, 2048, 128)
    for c in range(N_CORES):
        o = results[c]["outp"]  # [B, 256, 128] in device (rh, m'') row order
        for b in range(B):
            out[b, c::8, :] = o[b][inv]  # rows s3 = 8*rho + c
    return out


# revision 9
# speedup vs baseline: 1.0307x; 1.0106x over previous
"""DiffAttention Trainium2 kernel, 8-core SPMD (head-parallel), v2.

Problem (hardcoded): B=2, S=2048, D=128, H=8.
  q = (x@Wq.T+bq).reshape(B,H,S,2D)   # raw reshape: head h <-> rows [256h,256h+256) of proj
  s1 = q1@k1.T; s2 = q2@k2.T; attn = softmax(s1) - lam*softmax(s2)
  out = attn@v -> transpose/reshape -> GroupNorm(H groups) -> *(1-lam) -> concat heads -> @Wo.T+bo

Sharding: core c owns head h=c for both batches (2 units/core). GroupNorm groups
mix all heads -> tiny (32-float) AllGather of partial stats.

Index algebra per unit (b,h), block = proj rows [256h, 256h+256):
  sigma (attn row) = 8r+j, r in [0,256), j in [0,8). We use tau-order sigma' = 256j+r.
  q1T[d, sigma'=256j+r] = qpT_block[f=256j+d, r]   (even 128-col chunks of qp block)
  q2T: odd chunks.  v'[sigma'=256j+r, d] = vp_block[r, 128j+d].
  GroupNorm group g = {sigma': (sigma' mod 256)//32 == g} (32-wide strips).
  Final rows: out[b, 8*rho+h, 128h3+d] = GN(O)[b,h][sigma'=256(rho%8)+32h3+rho//8, d]

v2 changes vs v1:
  - softmax denominators via [128q,1]-output dot matmuls (nearly free on PE)
    + PE transposes into a [1,1024] psum row + DVE reciprocal + gpsimd
    partition_broadcast, replacing full-width ones-matmul accumulations.
  - exp on [128,2048] tiles (half the ACT instruction overhead).
  - bf16 V / E / fT / Wo (output matmuls 4x cheaper); q/k stay f32r.
  - output-stage partials read PSUM directly; collectives scheduled so the
    Pool-queue block lands where PE has a queued qb of slack.
"""

import sys

sys.path.insert(0, "/opt/trn_rl_repo")

import numpy as np

import concourse.bass as bass
import concourse.bacc as bacc
import concourse.mybir as mybir
import concourse.tile as tile

F32 = mybir.dt.float32
F32R = mybir.dt.float32r
BF16 = mybir.dt.bfloat16
AF = mybir.ActivationFunctionType
ALU = mybir.AluOpType

B, S, D, H = 2, 2048, 128, 8
N_CORES = 8
EPS = 1e-5
GROUP_N = float(256 * H * D)  # elements per GroupNorm group

_CACHED = None


def build_nc():
    nc = bacc.Bacc("TRN2", target_bir_lowering=False, debug=False, num_devices=N_CORES)

    # ---- per-core external I/O ----
    qT = nc.dram_tensor("qT", [B, 128, 256], F32, kind="ExternalInput")  # query block.T per batch
    wqT = nc.dram_tensor("wqT", [128, 2048], F32, kind="ExternalInput")
    wkT = nc.dram_tensor("wkT", [128, 2048], F32, kind="ExternalInput")
    wvT = nc.dram_tensor("wvT", [128, 1024], F32, kind="ExternalInput")
    woT = nc.dram_tensor("woT", [1024, 128], F32, kind="ExternalInput")
    bqT = nc.dram_tensor("bqT", [128, 16], F32, kind="ExternalInput")
    bkT = nc.dram_tensor("bkT", [128, 16], F32, kind="ExternalInput")
    bv = nc.dram_tensor("bv", [1, 1024], F32, kind="ExternalInput")
    bo = nc.dram_tensor("bo", [1, 128], F32, kind="ExternalInput")
    gnw2 = nc.dram_tensor("gnw2", [1, 16], F32, kind="ExternalInput")  # tiled x2 (b,g)
    gnb2 = nc.dram_tensor("gnb2", [1, 16], F32, kind="ExternalInput")
    lam = nc.dram_tensor("lam", [1, 1], F32, kind="ExternalInput")
    eye = nc.dram_tensor("eye", [128, 128], F32, kind="ExternalInput")
    outp = nc.dram_tensor("outp", [B, 256, 128], F32, kind="ExternalOutput")

    with tile.TileContext(nc) as tc:
        with (
            tc.tile_pool(name="const", bufs=1) as cpool,
            tc.tile_pool(name="proj", bufs=2) as projpool,
            tc.tile_pool(name="epool", bufs=4) as epool,
            tc.tile_pool(name="otpool", bufs=4) as otpool,
            tc.tile_pool(name="tmp", bufs=2) as tmppool,
            tc.tile_pool(name="dram", bufs=1, space="DRAM") as dram,
        ):
            # ---- load constants / weights (qT first: projections need it) ----
            qt_sb = []
            for u in range(B):
                q = cpool.tile([128, 256], F32, name=f"qt_sb{u}")
                nc.sync.dma_start(q[:], qT[u])
                qt_sb.append(q)

            # small constants go on the gpsimd DMA queue so they don't delay
            # the big weight DMAs on the sync queue
            bq_sb = cpool.tile([128, 16], F32)
            bk_sb = cpool.tile([128, 16], F32)
            nc.gpsimd.dma_start(bq_sb[:], bqT[:])
            nc.gpsimd.dma_start(bk_sb[:], bkT[:])
            bv_sb = cpool.tile([1, 1024], F32)
            nc.gpsimd.dma_start(bv_sb[:], bv[:])
            bo_sb = cpool.tile([1, 128], F32)
            nc.gpsimd.dma_start(bo_sb[:], bo[:])
            gnw_sb = cpool.tile([1, 16], F32)
            gnb_sb = cpool.tile([1, 16], F32)
            nc.gpsimd.dma_start(gnw_sb[:], gnw2[:])
            nc.gpsimd.dma_start(gnb_sb[:], gnb2[:])
            lam_sb = cpool.tile([1, 1], F32)
            nc.gpsimd.dma_start(lam_sb[:], lam[:])
            eye_sb = cpool.tile([128, 128], F32)
            nc.gpsimd.dma_start(eye_sb[:], eye[:])

            # weights loaded and f32r-rounded in 1024-col pieces so projections
            # can start before all input DMA completes.
            wq_rh, wk_rh = [], []
            wv_r = cpool.tile([128, 1024], F32R)
            wpieces = (
                [("wq", wqT, wq_rh, 0), ("wk", wkT, wk_rh, 0),
                 ("wk", wkT, wk_rh, 1), ("wv", wvT, None, 0),
                 ("wq", wqT, wq_rh, 1)]
            )
            for (wnm, dram_w, lst, half) in wpieces:
                wsc = projpool.tile([128, 1024], F32, tag="wsc", name=f"wsc_{wnm}{half}")
                split = (wnm == "wk" and half == 1)
                if split:
                    # 512-col pieces: the dripped j8-11 k-projection can start
                    # ~0.8us earlier
                    nc.sync.dma_start(wsc[:, 0:512], dram_w[:, 1024:1536])
                else:
                    nc.sync.dma_start(wsc[:], dram_w[:, 1024 * half: 1024 * (half + 1)])
                if lst is None:
                    nc.vector.tensor_copy(wv_r[:], wsc[:])
                else:
                    wr = cpool.tile([128, 1024], F32R, name=f"{wnm}_r{half}")
                    if split:
                        nc.vector.tensor_copy(wr[:, 0:512], wsc[:, 0:512])
                        nc.sync.dma_start(wsc[:, 512:1024], dram_w[:, 1536:2048])
                        nc.vector.tensor_copy(wr[:, 512:1024], wsc[:, 512:1024])
                    else:
                        nc.vector.tensor_copy(wr[:], wsc[:])
                    lst.append(wr)
            qt_r = []
            for u in range(B):
                qr = cpool.tile([128, 256], F32R, name=f"qt_r{u}")
                nc.vector.tensor_copy(qr[:], qt_sb[u][:])
                qt_r.append(qr)
            lam_rep = cpool.tile([128, 1], F32)
            nc.gpsimd.partition_broadcast(lam_rep[:], lam_sb[:])
            oml = cpool.tile([1, 1], F32)
            nc.vector.tensor_scalar(oml[:], lam_sb[:], -1.0, 1.0, ALU.mult, ALU.add)
            bv_rep = cpool.tile([128, 1024], F32)
            nc.gpsimd.partition_broadcast(bv_rep[:], bv_sb[:])

            ones_f32 = cpool.tile([128, 1], F32)
            nc.vector.memset(ones_f32[:], 1.0)
            ones2_f32 = cpool.tile([128, 2], F32)
            nc.vector.memset(ones2_f32[:], 1.0)
            ones_col = cpool.tile([128, 2], F32R)
            nc.vector.tensor_copy(ones_col[:], ones2_f32[:])

            # Wo chunks in bf16 (moving operand of the output matmuls);
            # DMAs queued after the projection weights
            wo_bf = []
            wo_stage = []
            for h3 in range(8):
                wsc = projpool.tile([128, 128], F32, tag="wosc", name=f"wosc{h3}", bufs=8)
                nc.sync.dma_start(wsc[:], woT[128 * h3: 128 * (h3 + 1), :])
                wo_stage.append(wsc)

            def emit_wo_copies():
                for h3 in range(8):
                    w = cpool.tile([128, 128], F32, name=f"wo_bf{h3}")
                    nc.vector.tensor_copy(w[:], wo_stage[h3][:])
                    wo_bf.append(w)

            p2_tiles = {0: [], 1: []}
            fT_sb = []
            ot_refs = {0: [], 1: []}  # otq tiles per unit (for deferred re-layout)

            cc_in = [dram.tile([1, 16], F32, name=f"cc_in{u}") for u in range(B)]
            cc_out = [dram.tile([8, 16], F32, addr_space="Shared", name=f"cc_out{u}")
                      for u in range(B)]
            gath = [tmppool.tile([1, 128], F32, tag="gath", name=f"gath_{u}", bufs=2)
                    for u in range(B)]
            scal = [{}, {}]

            def ptree(dst, src_tile, parts, width, nm, eng=None):
                eng = eng or nc.vector
                # partition-axis sum: DVE shift-copy + add down to 32 partitions
                # (TT needs equal base partitions; slices are 32-aligned),
                # then one gpsimd C-axis reduce for the final 32 -> 1.
                cur = src_tile
                while parts > 32:
                    parts //= 2
                    sh = tmppool.tile([parts, width], F32, tag=f"ps{parts}",
                                      name=f"ps_{nm}_{parts}", bufs=4)
                    eng.tensor_copy(sh[:], cur[parts: 2 * parts, :])
                    t = tmppool.tile([parts, width], F32, tag=f"pt{parts}",
                                     name=f"pt_{nm}_{parts}", bufs=4)
                    eng.tensor_tensor(t[:], cur[0:parts, :], sh[:], ALU.add)
                    cur = t
                nc.gpsimd.tensor_reduce(dst, cur[:], mybir.AxisListType.C, ALU.add)

            def emit_stats_export(u):
                stats_u = tmppool.tile([1, 16], F32, tag="stats", name=f"stats_{u}", bufs=2)
                for si, p2 in enumerate(p2_tiles[u]):
                    ptree(stats_u[:, 8 * si: 8 * si + 8], p2, 128, 8, f"st{u}{si}",
                          eng=(nc.vector, nc.gpsimd)[si])
                nc.sync.dma_start(cc_in[u][:], stats_u[:])
                nc.gpsimd.collective_compute(
                    "AllGather", ALU.bypass,
                    replica_groups=[list(range(N_CORES))],
                    ins=[cc_in[u][:]], outs=[cc_out[u][:]],
                )
                nc.gpsimd.dma_start(gath[u][:],
                                     cc_out[u][:].rearrange("a b -> (a b)").unsqueeze(0))

            def emit_scalars(u, cb_mm=None):
                # global stats for batch u -> A_rep[128,8], cb_rep[128,128]
                t = lambda nm: tmppool.tile([1, 8], F32, tag=nm, name=f"{nm}_{u}", bufs=2)
                glob = tmppool.tile([1, 16], F32, tag="globsb", name=f"glob_{u}", bufs=2)
                nc.vector.tensor_reduce(
                    glob[:], gath[u].rearrange("p (a b) -> p b a", a=8, b=16),
                    mybir.AxisListType.X, ALU.add,
                )
                moments = tmppool.tile([1, 16], F32, tag="mom", name=f"mom_{u}", bufs=2)
                nc.vector.tensor_scalar_mul(moments[:], glob[:], 1.0 / GROUP_N)
                mean, ex2 = moments[:, 0:8], moments[:, 8:16]
                var, veps = t("var"), t("veps")
                nc.vector.tensor_tensor(var[:], mean, mean, ALU.mult)
                nc.vector.tensor_tensor(var[:], ex2, var[:], ALU.subtract)
                nc.vector.tensor_scalar_add(veps[:], var[:], EPS)
                # rsqrt fully on DVE (ACT Sqrt would thrash the exp table set):
                # quake seed + 2 Newton steps
                I32 = mybir.dt.int32
                ti = tmppool.tile([1, 8], I32, tag="rsqi", name=f"rsqi_{u}", bufs=2)
                nc.vector.tensor_scalar(
                    ti[:], veps[:].bitcast(I32), 1, None, ALU.arith_shift_right
                )
                nc.vector.tensor_scalar(ti[:], ti[:], -1, 0x5F3759DF, ALU.mult, ALU.add)
                rstd, hf, nt = t("rstd"), t("hf"), t("nt")
                nc.vector.tensor_copy(rstd[:], ti[:].bitcast(F32))
                nc.vector.tensor_scalar_mul(hf[:], veps[:], 0.5)
                for _ in range(2):
                    nc.vector.tensor_tensor(nt[:], rstd[:], rstd[:], ALU.mult)
                    nc.vector.tensor_tensor(nt[:], nt[:], hf[:], ALU.mult)
                    nc.vector.tensor_scalar(nt[:], nt[:], -1.0, 1.5, ALU.mult, ALU.add)
                    nc.vector.tensor_tensor(rstd[:], rstd[:], nt[:], ALU.mult)
                A, Bc = t("A"), t("Bc")
                nc.vector.tensor_tensor(A[:], rstd[:], gnw_sb[:, 0:8], ALU.mult)
                nc.vector.tensor_tensor(Bc[:], mean, A[:], ALU.mult)
                nc.vector.tensor_tensor(Bc[:], gnb_sb[:, 0:8], Bc[:], ALU.subtract)
                nc.vector.tensor_scalar_mul(A[:], A[:], oml[:, 0:1])
                nc.vector.tensor_scalar_mul(Bc[:], Bc[:], oml[:, 0:1])
                A_rep = tmppool.tile([128, 8], F32, tag="A_rep", name=f"A_rep{u}", bufs=2)
                nc.gpsimd.partition_broadcast(A_rep[:], A[:])
                cb = tmppool.tile([1, 128], F32, tag="cb", name=f"cb_{u}", bufs=2)
                if cb_mm is None:
                    # serial stt chain (fine off the critical path)
                    nc.vector.tensor_scalar_mul(cb[:], wsum_sb[:, 0:128], Bc[:, 0:1])
                    for h3 in range(1, 8):
                        nc.vector.scalar_tensor_tensor(
                            cb[:], wsum_sb[:, 128 * h3: 128 * (h3 + 1)],
                            Bc[:, h3: h3 + 1], cb[:], ALU.mult, ALU.add,
                        )
                    nc.vector.tensor_tensor(cb[:], cb[:], bo_sb[:], ALU.add)
                else:
                    # critical path: cb = Bc(1x8) @ wsum_p8(8x128) via PE
                    # (transpose Bc to a column first), then + bo
                    ps_pool = cb_mm
                    btp = ps_pool.tile([8, 8], F32, tag="btp", name=f"btp_{u}")
                    nc.tensor.matmul(btp[:, 0:1], Bc[:], ones_f32[0:1, 0:1],
                                     is_transpose=True, start=True, stop=True)
                    bcol = tmppool.tile([8, 1], F32, tag="bcol", name=f"bcol_{u}", bufs=2)
                    nc.vector.tensor_copy(bcol[:], btp[:, 0:1])
                    cbp = ps_pool.tile([1, 128], F32, tag="cbp", name=f"cbp_{u}")
                    nc.tensor.matmul(cbp[:], bcol[:], wsum_p8[:], start=True, stop=True)
                    nc.vector.tensor_tensor(cb[:], cbp[:], bo_sb[:], ALU.add)
                cb_rep = tmppool.tile([128, 128], F32, tag="cb_rep", name=f"cbr_{u}", bufs=2)
                nc.gpsimd.partition_broadcast(cb_rep[:], cb[:])
                scal[u] = {"A_rep": A_rep, "cb_rep": cb_rep}

            qk = {}
            vts = []
            wsum_sb = cpool.tile([1, 1024], F32)
            wsum_f8 = cpool.tile([8, 128], F32)
            wsum_p8 = cpool.tile([8, 128], F32)

            # ================= attention-phase PSUM pools =================
            with (
                tc.tile_pool(name="ps_sgrp", bufs=2, space="PSUM") as ps_sgrp,
                tc.tile_pool(name="ps_u", bufs=1, space="PSUM") as ps_u,
                tc.tile_pool(name="ps_rdot", bufs=1, space="PSUM") as ps_rdot,
                tc.tile_pool(name="ps_rrow", bufs=1, space="PSUM") as ps_rrow,
            ):
                def emit_wsum():
                    # Wo column sums (for the GN-beta term), on the sgrp psum ring
                    wps = ps_sgrp.tile([128, 1024], F32, tag="sgrp", name="wps")
                    for h3 in range(8):
                        nc.tensor.matmul(wps[0:1, 128 * h3: 128 * (h3 + 1)],
                                         ones_f32[:, 0:1], wo_bf[h3][:],
                                         start=True, stop=True)
                    nc.vector.tensor_copy(wsum_sb[:, 0:1024], wps[0:1, 0:1024])
                    # row-per-h3 bf16 copy (for the PE-side cb matmul at the tail)
                    nc.sync.dma_start(
                        wsum_f8[:],
                        wsum_sb[:].rearrange("p (a b) -> (p a) b", a=8, b=128))
                    nc.vector.tensor_copy(wsum_p8[:], wsum_f8[:])

                def alloc_qk(u):
                    for nm in ("q1", "q2"):
                        qk[(u, nm)] = [
                            projpool.tile([128, 512], F32R, tag=f"{nm}t",
                                          name=f"{nm}t_{u}_{qb}", bufs=8)
                            for qb in range(4)
                        ]
                    for nm in ("k1", "k2"):
                        qk[(u, nm)] = [
                            projpool.tile([128, 1024], F32R, tag=f"{nm}t",
                                          name=f"{nm}t_{u}_{hh}", bufs=4)
                            for hh in range(2)
                        ]
                    vts.append(projpool.tile([128, 2048], F32R, tag="vp",
                                             name=f"vp_{u}", bufs=2))

                def proj_blocks(u, use_act=False):
                    # generator of closures: 10 psum-ring blocks per unit
                    # (4x q, 4x k with four 256-col chunks each; 2x v halves)
                    def qkblk(blk):
                        def emit():
                            ps = ps_sgrp.tile([128, 1024], F32, tag="sgrp",
                                              name=f"pp_{u}_{blk}")
                            wrh = wq_rh if blk < 4 else wk_rh
                            for c in range(4):
                                j = 4 * (blk % 4) + c
                                nc.tensor.matmul(
                                    ps[:, 256 * c: 256 * (c + 1)],
                                    wrh[j // 8][:, 128 * (j % 8): 128 * (j % 8 + 1)],
                                    qt_r[u][:], start=True, stop=True,
                                )
                            for c in range(4):
                                j = 4 * (blk % 4) + c
                                if blk < 4:
                                    dst = qk[(u, "q1" if j % 2 == 0 else "q2")][j // 4]
                                    col = 256 * ((j // 2) % 2)
                                    bias = bq_sb[:, j: j + 1]
                                else:
                                    dst = qk[(u, "k1" if j % 2 == 0 else "k2")][j // 8]
                                    col = 256 * ((j // 2) % 4)
                                    bias = bk_sb[:, j: j + 1]
                                if use_act and c % 2 == 0:
                                    # head phase: ACT is idle; Identity+bias is
                                    # in every table set (no exp-table thrash)
                                    nc.scalar.activation(
                                        dst[:, col: col + 256],
                                        ps[:, 256 * c: 256 * (c + 1)],
                                        AF.Identity, bias=bias,
                                    )
                                else:
                                    nc.vector.tensor_scalar_add(
                                        dst[:, col: col + 256],
                                        ps[:, 256 * c: 256 * (c + 1)], bias
                                    )
                        return emit

                    def vblk(rc):
                        def emit():
                            vt = vts[u]
                            ps = ps_sgrp.tile([128, 1024], F32, tag="sgrp",
                                              name=f"ppv_{u}_{rc}")
                            for fh in range(2):
                                nc.tensor.matmul(
                                    ps[:, 512 * fh: 512 * (fh + 1)],
                                    qt_r[u][:, 128 * rc: 128 * (rc + 1)],
                                    wv_r[:, 512 * fh: 512 * (fh + 1)],
                                    start=True, stop=True,
                                )
                            for fh in range(2):
                                nc.vector.tensor_tensor(
                                    vt[:, 1024 * rc + 512 * fh:
                                       1024 * rc + 512 * fh + 512],
                                    ps[:, 512 * fh: 512 * (fh + 1)],
                                    bv_rep[:, 512 * fh: 512 * (fh + 1)], ALU.add,
                                )
                        return emit

                    # pre: minimum to start attention qb0 (q j0-3, k j0-7, v);
                    # drip: the rest, fed into attention slots as DMA lands
                    pre = [qkblk(0), qkblk(4), qkblk(5), vblk(0), vblk(1)]
                    drip = [qkblk(6), qkblk(7), qkblk(1), qkblk(2), qkblk(3)]
                    return pre, drip

                def attention_unit(u, boundary_cb, slot_cb):
                    q1l, q2l = qk[(u, "q1")], qk[(u, "q2")]
                    kls = (qk[(u, "k1")], qk[(u, "k2")])
                    vt = vts[u]

                    def vchunk(kc):
                        return vt[:, 1024 * (kc % 2) + 128 * (kc // 2):
                                  1024 * (kc % 2) + 128 * (kc // 2) + 128]

                    fT = tmppool.tile([128, 2048], F32, tag="fT", name=f"fT_{u}")
                    fT_sb.append(fT)
                    p1a = tmppool.tile([128, 16], F32, tag="p1a", name=f"p1a_{u}")
                    p1b = tmppool.tile([128, 16], F32, tag="p1b", name=f"p1b_{u}")

                    LAG = 3  # consume items this many exp-slots behind issue
                    state = {}

                    def emit_r_chain(qb, m):
                        # per-branch: [128,8] dots -> f32 transpose into a
                        # [1,512] psum row -> reciprocal -> partition bcast
                        st = state[qb]
                        r_sb = tmppool.tile([128, 8], F32, tag=f"rsb{m}",
                                            name=f"rsb_{u}_{qb}_{m}", bufs=2)
                        nc.vector.tensor_copy(r_sb[:], st["rdot"][:, 8 * m: 8 * m + 8])
                        rrow = ps_rrow.tile([1, 512], F32, tag="rr",
                                            name=f"rr_{u}_{qb}_{m}")
                        for s4 in range(4):
                            nc.tensor.matmul(
                                rrow[0:1, 128 * s4: 128 * (s4 + 1)],
                                r_sb[:, 2 * s4: 2 * s4 + 1], eye_sb[:],
                                is_transpose=True, start=True, stop=True,
                            )
                        r_inv = tmppool.tile([1, 512], F32, tag=f"rinv{m}",
                                             name=f"rinv_{u}_{qb}_{m}", bufs=2)
                        nc.vector.reciprocal(r_inv[:], rrow[:])
                        rr = tmppool.tile([128, 512], F32, tag=f"r{m}rep",
                                          name=f"r{m}rep_{u}_{qb}", bufs=2)
                        nc.gpsimd.partition_broadcast(rr[:], r_inv[:])
                        st["rrep"][m] = rr

                    def finish_qb(qb):
                        # O = U1/R1 - lam*U2/R2  (t1 emitted early, in-branch)
                        st = state[qb]
                        t2 = tmppool.tile([128, 512], F32, tag="t2", name=f"t2_{u}_{qb}")
                        nc.vector.scalar_tensor_tensor(
                            t2[:], st["u2"][:], lam_rep[:, 0:1], st["rrep"][1][:],
                            ALU.mult, ALU.mult
                        )
                        otq = otpool.tile([128, 512], F32, tag="ot", name=f"ot_{u}_{qb}")
                        nc.vector.tensor_tensor(otq[:], st["t1"][:], t2[:], ALU.subtract)
                        ot_refs[u].append(otq)

                        # incremental GroupNorm partial stats for this q-block
                        # (free-dim layout within the block: (j2, g8, r32))
                        osl = otq.rearrange("p (j g r) -> p j g r", j=2, g=8, r=32)
                        red = tmppool.tile([128, 16], F32, tag="red", name=f"red_{u}_{qb}")
                        nc.vector.tensor_reduce(red[:], osl, mybir.AxisListType.X, ALU.add)
                        if qb == 0:
                            nc.vector.tensor_copy(p1a[:], red[:])
                        else:
                            nc.vector.tensor_tensor(p1a[:], p1a[:], red[:], ALU.add)
                        sq5 = tmppool.tile([128, 512], F32, tag="t1", name=f"sq5_{u}_{qb}")
                        sq_eng = nc.gpsimd if (u == 1 and qb == 3) else nc.vector
                        sq_eng.tensor_tensor(sq5[:], otq[:], otq[:], ALU.mult)
                        redb = tmppool.tile([128, 16], F32, tag="redb", name=f"redb_{u}_{qb}")
                        nc.vector.tensor_reduce(
                            redb[:], sq5.rearrange("p (j g r) -> p j g r", j=2, g=8, r=32),
                            mybir.AxisListType.X, ALU.add,
                        )
                        if qb == 0:
                            nc.vector.tensor_copy(p1b[:], redb[:])
                        else:
                            nc.vector.tensor_tensor(p1b[:], p1b[:], redb[:], ALU.add)
                        if u == 0 or qb < 2:
                            emit_relayout(u, qb)
                        boundary_cb(qb)

                    def consume(item):
                        qb, m, g, eg = item
                        if m == 0 and g == 0:
                            # lazy per-qb psum state: allocated only once the
                            # previous qb's readers are already emitted (FIFO)
                            rdot = ps_rdot.tile([128, 16], F32, tag="rd",
                                                name=f"rd_{u}_{qb}")
                            nc.vector.memset(rdot[:], 0.0)
                            state[qb] = {
                                "u1": ps_u.tile([128, 512], F32, tag="u1",
                                                name=f"u1_{u}_{qb}"),
                                "u2": ps_u.tile([128, 512], F32, tag="u2",
                                                name=f"u2_{u}_{qb}"),
                                "rdot": rdot, "rrep": [None, None], "t1": None,
                            }
                        st = state[qb]
                        uacc = (st["u1"], st["u2"])[m]
                        for c in range(2):
                            kc = 2 * g + c
                            nc.tensor.matmul(
                                uacc[:], vchunk(kc), eg[:, 512 * c: 512 * (c + 1)],
                                start=(g == 0 and c == 0),
                                stop=(g == 7 and c == 1),
                            )
                            for sl4 in range(4):
                                col = 2 * (4 * m + sl4)
                                nc.tensor.matmul(
                                    st["rdot"][:, col: col + 2],
                                    eg[:, 512 * c + 128 * sl4: 512 * c + 128 * sl4 + 128],
                                    ones_col[:],
                                    start=False, stop=False, skip_group_check=True,
                                )
                        if g == 7:
                            emit_r_chain(qb, m)
                            if m == 0:
                                # t1 = U1/R1 early, while branch 2 streams
                                t1 = tmppool.tile([128, 512], F32, tag="t1",
                                                  name=f"t1_{u}_{qb}")
                                nc.vector.tensor_tensor(t1[:], st["u1"][:],
                                                        st["rrep"][0][:], ALU.mult)
                                st["t1"] = t1
                            else:
                                finish_qb(qb)

                    # flat 64-slot pipeline: the sgrp/exp stream never pauses
                    # at qb boundaries; qb bookkeeping rides inside the lagged
                    # consume stream
                    pending = []
                    for slot in range(64):
                        qb, rem = divmod(slot, 16)
                        m, g = divmod(rem, 8)
                        ql = (q1l, q2l)[m][qb]
                        kl = kls[m]
                        sg = ps_sgrp.tile([128, 1024], F32, tag="sgrp",
                                          name=f"sg_{u}_{qb}_{m}_{g}")
                        for c in range(2):
                            kc = 2 * g + c
                            nc.tensor.matmul(
                                sg[:, 512 * c: 512 * (c + 1)],
                                kl[kc // 8][:, 128 * (kc % 8): 128 * (kc % 8 + 1)],
                                ql[:], start=True, stop=True,
                            )
                        eg = epool.tile([128, 1024], F32R, tag="e",
                                        name=f"e_{u}_{qb}_{m}_{g}")
                        nc.scalar.activation(eg[:], sg[:], AF.Exp)
                        pending.append((qb, m, g, eg))
                        if len(pending) > LAG:
                            consume(pending.pop(0))
                        slot_cb(qb, rem)
                    for item in pending:
                        consume(item)

                    # fold (j mod 2) pairs -> per-group partials
                    for si, p1x in enumerate((p1a, p1b)):
                        p2 = tmppool.tile([128, 8], F32, tag="p2", name=f"p2_{u}_{si}")
                        nc.vector.tensor_reduce(
                            p2[:], p1x.rearrange("p (j g) -> p g j", j=2, g=8),
                            mybir.AxisListType.X, ALU.add,
                        )
                        p2_tiles[u].append(p2)

                def emit_relayout(u, qb):
                    # re-layout into fT (j-pair slab for this qb);
                    # src re-viewed g-outer to match the dst iteration order
                    fT = fT_sb[u]
                    fv4 = fT.rearrange("p (g j r) -> p g j r", g=8, j=8, r=32)
                    otq = ot_refs[u][qb]
                    nc.vector.tensor_copy(
                        fv4[:, :, 2 * qb: 2 * qb + 2, :],
                        otq.rearrange("p (j g r) -> p g j r", j=2, g=8, r=32),
                    )

                # ================= main schedule =================
                alloc_qk(0)
                alloc_qk(1)
                pre0, _ = proj_blocks(0, use_act=True)
                for blk in pre0:
                    blk()
                _, drip0 = proj_blocks(0)
                pre1, drip1 = proj_blocks(1)
                drip = drip0 + pre1 + drip1  # u0 stragglers first, then all of u1
                # slots at which to emit one proj block into u0's attention:
                # k j8-15 blocks early (needed by qb0 slot 4/6), then every 3rd
                drip_slots = [3, 5] + list(range(8, 64, 3))

                def u0_slot(qb, slot):
                    g = 16 * qb + slot
                    if drip and drip_slots and g >= drip_slots[0]:
                        drip_slots.pop(0)
                        drip.pop(0)()

                attention_unit(0, lambda qb: None, u0_slot)
                while drip:
                    drip.pop(0)()

                def u1_boundary(qb):
                    if qb == 0:
                        # unit-0 stats export + collective #1: lands on the Pool
                        # queue right after qb0's broadcasts; PE has a full qb of
                        # queued work to ride out the 15us Pool block.
                        emit_stats_export(0)
                        emit_wo_copies()
                        emit_wsum()
                    if qb == 2:
                        # unit-0 GN scalars (collective #1 landed long ago)
                        emit_scalars(0)

                attention_unit(1, u1_boundary, lambda qb, slot: None)
                # stats export for unit 1: as early as possible -> tail collective
                emit_stats_export(1)

            # ============== output stage (fills collective #2 window) ==============
            with tc.tile_pool(name="ps_out", bufs=1, space="PSUM") as ps_out:
                for qb in (2, 3):
                    emit_relayout(1, qb)

                P_sb = {}

                def emit_partials(u):
                    # 4 P outputs packed per [128,512] psum bank tile, then
                    # drained to SBUF (cheap to re-read; fills collective #2)
                    fT = fT_sb[u]
                    for rh in range(2):
                        for h4 in range(2):
                            pps = ps_out.tile([128, 512], F32, tag="P", bufs=4,
                                              name=f"pps_{u}_{rh}_{h4}")
                            for hq in range(4):
                                h3 = 4 * h4 + hq
                                lhsT = fT[:, 256 * h3 + 128 * rh: 256 * h3 + 128 * rh + 128]
                                nc.tensor.matmul(pps[:, 128 * hq: 128 * (hq + 1)],
                                                 lhsT, wo_bf[h3][:], start=True, stop=True)
                            P_sb[(u, rh, h4)] = pps

                def emit_combine(u, use_pool):
                    # result = sum_h3 A[u,h3]*P[u][rh][h3] + cb_rep[u]
                    # split per rh into a DVE half-chain (h3 0-3) and a Pool
                    # half-chain (h3 4-7), joined by one add
                    A_rep = scal[u]["A_rep"]
                    cb_rep = scal[u]["cb_rep"]
                    accs = {}
                    for rh in range(2):
                        for h4 in range(2):
                            psb = P_sb[(u, rh, h4)]
                            acc = tmppool.tile([128, 128], F32, tag=f"acc{h4}",
                                               name=f"acc_{u}_{rh}_{h4}")
                            if False:
                                # Pool path: tsm x4 + tt tree (no stt on Pool)
                                sc = tmppool.tile([128, 512], F32, tag="pscl",
                                                  name=f"pscl_{u}_{rh}", bufs=2)
                                for hq in range(4):
                                    nc.gpsimd.tensor_scalar_mul(
                                        sc[:, 128 * hq: 128 * (hq + 1)],
                                        psb[:, 128 * hq: 128 * (hq + 1)],
                                        A_rep[:, 4 * h4 + hq: 4 * h4 + hq + 1])
                                nc.gpsimd.tensor_tensor(
                                    sc[:, 0:128], sc[:, 0:128], sc[:, 128:256], ALU.add)
                                nc.gpsimd.tensor_tensor(
                                    sc[:, 256:384], sc[:, 256:384], sc[:, 384:512], ALU.add)
                                nc.gpsimd.tensor_tensor(
                                    acc[:], sc[:, 0:128], sc[:, 256:384], ALU.add)
                            else:
                                nc.vector.tensor_scalar_mul(
                                    acc[:], psb[:, 0:128], A_rep[:, 4 * h4: 4 * h4 + 1]
                                )
                                for hq in range(1, 4):
                                    h3 = 4 * h4 + hq
                                    nc.vector.scalar_tensor_tensor(
                                        acc[:], psb[:, 128 * hq: 128 * (hq + 1)],
                                        A_rep[:, h3: h3 + 1], acc[:],
                                        ALU.mult, ALU.add,
                                    )
                            accs[(rh, h4)] = acc
                    for rh in range(2):
                        rsb = tmppool.tile([128, 128], F32, tag="rsb2",
                                           name=f"rsb_{u}_{rh}")
                        nc.vector.tensor_tensor(rsb[:], accs[(rh, 0)][:],
                                                accs[(rh, 1)][:], ALU.add)
                        nc.vector.tensor_tensor(rsb[:], rsb[:], cb_rep[:], ALU.add)
                        # contiguous block write; host undoes the row permutation
                        # (device row 128*rh+m'' holds rho = 8*(m''%32)+4*rh+m''//32)
                        nc.sync.dma_start(outp[u][128 * rh: 128 * (rh + 1), :], rsb[:])

                emit_partials(0)
                emit_combine(0, use_pool=False)  # fill work; Pool is blocked by collective #2
                emit_partials(1)
                emit_scalars(1)       # waits on collective #2 (hidden behind fill work)
                emit_combine(1, use_pool=True)

    nc.compile()
    return nc


def _prep_inputs(inputs):
    """Host-side: slice/transpose full inputs into per-core in_maps."""
    query = np.asarray(inputs["query"], np.float32)
    Wq = np.asarray(inputs["Wq"], np.float32)
    Wk = np.asarray(inputs["Wk"], np.float32)
    Wv = np.asarray(inputs["Wv"], np.float32)
    Wo = np.asarray(inputs["Wo"], np.float32)
    bq = np.asarray(inputs["bq"], np.float32)
    bk = np.asarray(inputs["bk"], np.float32)
    bv = np.asarray(inputs["bv"], np.float32)
    bo = np.asarray(inputs["bo"], np.float32)
    gn_w = np.asarray(inputs["gn_w"], np.float32)
    gn_b = np.asarray(inputs["gn_b"], np.float32)
    lam = np.asarray(inputs["lam"], np.float32).reshape(1, 1)

    shared = {
        "wqT": np.ascontiguousarray(Wq.T),
        "wkT": np.ascontiguousarray(Wk.T),
        "wvT": np.ascontiguousarray(Wv.T),
        "woT": np.ascontiguousarray(Wo.T),
        "bqT": np.ascontiguousarray(bq.reshape(16, 128).T),
        "bkT": np.ascontiguousarray(bk.reshape(16, 128).T),
        "bv": bv.reshape(1, 1024),
        "bo": bo.reshape(1, 128),
        "gnw2": np.tile(gn_w, 2).reshape(1, 16),
        "gnb2": np.tile(gn_b, 2).reshape(1, 16),
        "lam": lam,
        "eye": np.eye(128, dtype=np.float32),
    }
    in_maps = []
    for c in range(N_CORES):
        blk = query[:, 256 * c: 256 * (c + 1), :]  # [B, 256, 128]
        qT = np.ascontiguousarray(blk.transpose(0, 2, 1))  # [B, 128, 256]
        in_maps.append({"qT": qT, **shared})
    return in_maps


class _Runner:
    """Cached-jit SPMD executor (one trace/compile; cheap repeated calls)."""

    def __init__(self, nc):
        import jax
        from jax.sharding import Mesh, PartitionSpec
        from jax.experimental.shard_map import shard_map
        from concourse.bass2jax import (
            install_neuronx_cc_hook, _bass_exec_p, partition_id_tensor,
        )

        install_neuronx_cc_hook()
        self.jax = jax
        pname = nc.partition_id_tensor.name if nc.partition_id_tensor else None
        in_names, out_names, out_avals, zero_outs = [], [], [], []
        for alloc in nc.m.functions[0].allocations:
            if not isinstance(alloc, mybir.MemoryLocationSet):
                continue
            name = alloc.memorylocations[0].name
            if alloc.kind == "ExternalInput":
                if name != pname:
                    in_names.append(name)
            elif alloc.kind == "ExternalOutput":
                out_names.append(name)
                shape = tuple(alloc.tensor_shape)
                dtype = mybir.dt.np(alloc.dtype)
                out_avals.append(jax.core.ShapedArray(shape, dtype))
                zero_outs.append(np.zeros(shape, dtype))
        self.in_names, self.out_names = in_names, out_names
        n_params = len(in_names)
        all_names = list(in_names) + out_names
        if pname is not None:
            all_names.append(pname)

        def _body(*args):
            operands = list(args)
            if pname is not None:
                operands.append(partition_id_tensor())
            return tuple(_bass_exec_p.bind(
                *operands, out_avals=tuple(out_avals), in_names=tuple(all_names),
                out_names=tuple(out_names), lowering_input_output_aliases=(),
                sim_require_finite=True, sim_require_nnan=True, nc=nc))

        devices = jax.devices()[:N_CORES]
        mesh = Mesh(np.asarray(devices), ("core",))
        nio = n_params + len(out_names)
        self.fn = jax.jit(
            shard_map(_body, mesh=mesh, in_specs=(PartitionSpec("core"),) * nio,
                      out_specs=(PartitionSpec("core"),) * len(out_names),
                      check_rep=False),
            keep_unused=True,
        )
        self.zeros = [
            jax.device_put(np.zeros((N_CORES * z.shape[0], *z.shape[1:]), z.dtype))
            for z in zero_outs
        ]
        self.out_shapes = [tuple(a.shape) for a in out_avals]

    def run(self, in_maps):
        cat = [
            np.concatenate([np.asarray(m[n]) for m in in_maps], axis=0)
            for n in self.in_names
        ]
        # the accelerator intermittently throws a transient
        # NRT_EXEC_UNIT_UNRECOVERABLE (status 101); retry once
        for attempt in range(3):
            try:
                outs = self.fn(*cat, *self.zeros)
                self.jax.block_until_ready(outs)
                outs = [np.asarray(o) for o in outs]
                break
            except Exception:
                if attempt == 2:
                    raise
                import time as _t
                _t.sleep(5.0)
        return [
            {n: outs[i].reshape(N_CORES, *self.out_shapes[i])[c]
             for i, n in enumerate(self.out_names)}
            for c in range(N_CORES)
        ]


_CACHED_NC = None


def kernel(**inputs) -> np.ndarray:
    global _CACHED, _CACHED_NC
    if _CACHED is None:
        _CACHED_NC = build_nc()
        _CACHED = _Runner(_CACHED_NC)
    in_maps = _prep_inputs(inputs)
    results = _CACHED.run(in_maps)
    # device row (rh, m'') holds output row rho = 8*(m'' % 32) + 4*rh + m''//32
    mpp = np.arange(128)
    rho = np.concatenate([8 * (mpp % 32) + 4 * rh + mpp // 32 for rh in (0, 1)])
    inv = np.argsort(rho)
    out = np.empty((B, S, H * D // 8), np.float32)  # (2, 2048, 128)
    for c in range(N_CORES):
        o = results[c]["outp"]  # [B, 256, 128] in device (rh, m'') row order
        for b in range(B):
            out[b, c::8, :] = o[b][inv]  # rows s3 = 8*rho + c
    return out


# revision 10
# speedup vs baseline: 1.0332x; 1.0024x over previous
"""DiffAttention Trainium2 kernel, 8-core SPMD (head-parallel), v2.

Problem (hardcoded): B=2, S=2048, D=128, H=8.
  q = (x@Wq.T+bq).reshape(B,H,S,2D)   # raw reshape: head h <-> rows [256h,256h+256) of proj
  s1 = q1@k1.T; s2 = q2@k2.T; attn = softmax(s1) - lam*softmax(s2)
  out = attn@v -> transpose/reshape -> GroupNorm(H groups) -> *(1-lam) -> concat heads -> @Wo.T+bo

Sharding: core c owns head h=c for both batches (2 units/core). GroupNorm groups
mix all heads -> tiny (32-float) AllGather of partial stats.

Index algebra per unit (b,h), block = proj rows [256h, 256h+256):
  sigma (attn row) = 8r+j, r in [0,256), j in [0,8). We use tau-order sigma' = 256j+r.
  q1T[d, sigma'=256j+r] = qpT_block[f=256j+d, r]   (even 128-col chunks of qp block)
  q2T: odd chunks.  v'[sigma'=256j+r, d] = vp_block[r, 128j+d].
  GroupNorm group g = {sigma': (sigma' mod 256)//32 == g} (32-wide strips).
  Final rows: out[b, 8*rho+h, 128h3+d] = GN(O)[b,h][sigma'=256(rho%8)+32h3+rho//8, d]

v2 changes vs v1:
  - softmax denominators via [128q,1]-output dot matmuls (nearly free on PE)
    + PE transposes into a [1,1024] psum row + DVE reciprocal + gpsimd
    partition_broadcast, replacing full-width ones-matmul accumulations.
  - exp on [128,2048] tiles (half the ACT instruction overhead).
  - bf16 V / E / fT / Wo (output matmuls 4x cheaper); q/k stay f32r.
  - output-stage partials read PSUM directly; collectives scheduled so the
    Pool-queue block lands where PE has a queued qb of slack.
"""

import sys

sys.path.insert(0, "/opt/trn_rl_repo")

import numpy as np

import concourse.bass as bass
import concourse.bacc as bacc
import concourse.mybir as mybir
import concourse.tile as tile

F32 = mybir.dt.float32
F32R = mybir.dt.float32r
BF16 = mybir.dt.bfloat16
AF = mybir.ActivationFunctionType
ALU = mybir.AluOpType

B, S, D, H = 2, 2048, 128, 8
N_CORES = 8
EPS = 1e-5
GROUP_N = float(256 * H * D)  # elements per GroupNorm group

_CACHED = None


def build_nc():
    nc = bacc.Bacc("TRN2", target_bir_lowering=False, debug=False, num_devices=N_CORES)

    # ---- per-core external I/O ----
    qT = nc.dram_tensor("qT", [B, 128, 256], F32, kind="ExternalInput")  # query block.T per batch
    wqT = nc.dram_tensor("wqT", [128, 2048], F32, kind="ExternalInput")
    wkT = nc.dram_tensor("wkT", [128, 2048], F32, kind="ExternalInput")
    wvT = nc.dram_tensor("wvT", [128, 1024], F32, kind="ExternalInput")
    woT = nc.dram_tensor("woT", [1024, 128], F32, kind="ExternalInput")
    bqT = nc.dram_tensor("bqT", [128, 16], F32, kind="ExternalInput")
    bkT = nc.dram_tensor("bkT", [128, 16], F32, kind="ExternalInput")
    bv = nc.dram_tensor("bv", [1, 1024], F32, kind="ExternalInput")
    bo = nc.dram_tensor("bo", [1, 128], F32, kind="ExternalInput")
    gnw2 = nc.dram_tensor("gnw2", [1, 16], F32, kind="ExternalInput")  # tiled x2 (b,g)
    gnb2 = nc.dram_tensor("gnb2", [1, 16], F32, kind="ExternalInput")
    lam = nc.dram_tensor("lam", [1, 1], F32, kind="ExternalInput")
    eye = nc.dram_tensor("eye", [128, 128], F32, kind="ExternalInput")
    outp = nc.dram_tensor("outp", [B, 256, 128], F32, kind="ExternalOutput")

    with tile.TileContext(nc) as tc:
        with (
            tc.tile_pool(name="const", bufs=1) as cpool,
            tc.tile_pool(name="proj", bufs=2) as projpool,
            tc.tile_pool(name="epool", bufs=5) as epool,
            tc.tile_pool(name="otpool", bufs=3) as otpool,
            tc.tile_pool(name="tmp", bufs=2) as tmppool,
            tc.tile_pool(name="dram", bufs=1, space="DRAM") as dram,
        ):
            # ---- load constants / weights (qT first: projections need it) ----
            qt_sb = []
            for u in range(B):
                q = cpool.tile([128, 256], F32, name=f"qt_sb{u}")
                nc.sync.dma_start(q[:], qT[u])
                qt_sb.append(q)

            # small constants go on the gpsimd DMA queue so they don't delay
            # the big weight DMAs on the sync queue
            bq_sb = cpool.tile([128, 16], F32)
            bk_sb = cpool.tile([128, 16], F32)
            nc.gpsimd.dma_start(bq_sb[:], bqT[:])
            nc.gpsimd.dma_start(bk_sb[:], bkT[:])
            bv_sb = cpool.tile([1, 1024], F32)
            nc.gpsimd.dma_start(bv_sb[:], bv[:])
            bo_sb = cpool.tile([1, 128], F32)
            nc.gpsimd.dma_start(bo_sb[:], bo[:])
            gnw_sb = cpool.tile([1, 16], F32)
            gnb_sb = cpool.tile([1, 16], F32)
            nc.gpsimd.dma_start(gnw_sb[:], gnw2[:])
            nc.gpsimd.dma_start(gnb_sb[:], gnb2[:])
            lam_sb = cpool.tile([1, 1], F32)
            nc.gpsimd.dma_start(lam_sb[:], lam[:])
            eye_sb = cpool.tile([128, 128], F32)
            nc.gpsimd.dma_start(eye_sb[:], eye[:])

            # weights loaded and f32r-rounded in 1024-col pieces so projections
            # can start before all input DMA completes.
            wq_rh, wk_rh = [], []
            wv_r = cpool.tile([128, 1024], F32R)
            wpieces = (
                [("wq", wqT, wq_rh, 0), ("wk", wkT, wk_rh, 0),
                 ("wk", wkT, wk_rh, 1), ("wv", wvT, None, 0),
                 ("wq", wqT, wq_rh, 1)]
            )
            for (wnm, dram_w, lst, half) in wpieces:
                wsc = projpool.tile([128, 1024], F32, tag="wsc", name=f"wsc_{wnm}{half}")
                split = (wnm == "wk" and half == 1)
                if split:
                    # 512-col pieces: the dripped j8-11 k-projection can start
                    # ~0.8us earlier
                    nc.sync.dma_start(wsc[:, 0:512], dram_w[:, 1024:1536])
                else:
                    nc.sync.dma_start(wsc[:], dram_w[:, 1024 * half: 1024 * (half + 1)])
                if lst is None:
                    nc.vector.tensor_copy(wv_r[:], wsc[:])
                else:
                    wr = cpool.tile([128, 1024], F32R, name=f"{wnm}_r{half}")
                    if split:
                        nc.vector.tensor_copy(wr[:, 0:512], wsc[:, 0:512])
                        nc.sync.dma_start(wsc[:, 512:1024], dram_w[:, 1536:2048])
                        nc.vector.tensor_copy(wr[:, 512:1024], wsc[:, 512:1024])
                    else:
                        nc.vector.tensor_copy(wr[:], wsc[:])
                    lst.append(wr)
            qt_r = []
            for u in range(B):
                qr = cpool.tile([128, 256], F32R, name=f"qt_r{u}")
                nc.vector.tensor_copy(qr[:], qt_sb[u][:])
                qt_r.append(qr)
            lam_rep = cpool.tile([128, 1], F32)
            nc.gpsimd.partition_broadcast(lam_rep[:], lam_sb[:])
            oml = cpool.tile([1, 1], F32)
            nc.vector.tensor_scalar(oml[:], lam_sb[:], -1.0, 1.0, ALU.mult, ALU.add)
            bv_rep = cpool.tile([128, 1024], F32)
            nc.gpsimd.partition_broadcast(bv_rep[:], bv_sb[:])

            ones_f32 = cpool.tile([128, 1], F32)
            nc.vector.memset(ones_f32[:], 1.0)
            ones2_f32 = cpool.tile([128, 2], F32)
            nc.vector.memset(ones2_f32[:], 1.0)
            ones_col = cpool.tile([128, 2], F32R)
            nc.vector.tensor_copy(ones_col[:], ones2_f32[:])

            # Wo chunks in bf16 (moving operand of the output matmuls);
            # DMAs queued after the projection weights
            wo_bf = []
            wo_stage = []
            for h3 in range(8):
                wsc = projpool.tile([128, 128], F32, tag="wosc", name=f"wosc{h3}", bufs=8)
                nc.sync.dma_start(wsc[:], woT[128 * h3: 128 * (h3 + 1), :])
                wo_stage.append(wsc)

            def emit_wo_copies():
                for h3 in range(8):
                    w = cpool.tile([128, 128], F32, name=f"wo_bf{h3}")
                    nc.vector.tensor_copy(w[:], wo_stage[h3][:])
                    wo_bf.append(w)

            p2_tiles = {0: [], 1: []}
            fT_sb = []
            ot_refs = {0: [], 1: []}  # otq tiles per unit (for deferred re-layout)

            cc_in = [dram.tile([1, 16], F32, name=f"cc_in{u}") for u in range(B)]
            cc_out = [dram.tile([8, 16], F32, addr_space="Shared", name=f"cc_out{u}")
                      for u in range(B)]
            gath = [tmppool.tile([1, 128], F32, tag="gath", name=f"gath_{u}", bufs=2)
                    for u in range(B)]
            scal = [{}, {}]

            def ptree(dst, src_tile, parts, width, nm, eng=None):
                eng = eng or nc.vector
                # partition-axis sum: DVE shift-copy + add down to 32 partitions
                # (TT needs equal base partitions; slices are 32-aligned),
                # then one gpsimd C-axis reduce for the final 32 -> 1.
                cur = src_tile
                while parts > 32:
                    parts //= 2
                    sh = tmppool.tile([parts, width], F32, tag=f"ps{parts}",
                                      name=f"ps_{nm}_{parts}", bufs=4)
                    eng.tensor_copy(sh[:], cur[parts: 2 * parts, :])
                    t = tmppool.tile([parts, width], F32, tag=f"pt{parts}",
                                     name=f"pt_{nm}_{parts}", bufs=4)
                    eng.tensor_tensor(t[:], cur[0:parts, :], sh[:], ALU.add)
                    cur = t
                nc.gpsimd.tensor_reduce(dst, cur[:], mybir.AxisListType.C, ALU.add)

            def emit_stats_export(u):
                stats_u = tmppool.tile([1, 16], F32, tag="stats", name=f"stats_{u}", bufs=2)
                for si, p2 in enumerate(p2_tiles[u]):
                    ptree(stats_u[:, 8 * si: 8 * si + 8], p2, 128, 8, f"st{u}{si}",
                          eng=(nc.vector, nc.gpsimd)[si])
                nc.sync.dma_start(cc_in[u][:], stats_u[:])
                nc.gpsimd.collective_compute(
                    "AllGather", ALU.bypass,
                    replica_groups=[list(range(N_CORES))],
                    ins=[cc_in[u][:]], outs=[cc_out[u][:]],
                )
                nc.gpsimd.dma_start(gath[u][:],
                                     cc_out[u][:].rearrange("a b -> (a b)").unsqueeze(0))

            def emit_scalars(u, cb_mm=None):
                # global stats for batch u -> A_rep[128,8], cb_rep[128,128]
                t = lambda nm: tmppool.tile([1, 8], F32, tag=nm, name=f"{nm}_{u}", bufs=2)
                glob = tmppool.tile([1, 16], F32, tag="globsb", name=f"glob_{u}", bufs=2)
                nc.vector.tensor_reduce(
                    glob[:], gath[u].rearrange("p (a b) -> p b a", a=8, b=16),
                    mybir.AxisListType.X, ALU.add,
                )
                moments = tmppool.tile([1, 16], F32, tag="mom", name=f"mom_{u}", bufs=2)
                nc.vector.tensor_scalar_mul(moments[:], glob[:], 1.0 / GROUP_N)
                mean, ex2 = moments[:, 0:8], moments[:, 8:16]
                var, veps = t("var"), t("veps")
                nc.vector.tensor_tensor(var[:], mean, mean, ALU.mult)
                nc.vector.tensor_tensor(var[:], ex2, var[:], ALU.subtract)
                nc.vector.tensor_scalar_add(veps[:], var[:], EPS)
                # rsqrt fully on DVE (ACT Sqrt would thrash the exp table set):
                # quake seed + 2 Newton steps
                I32 = mybir.dt.int32
                ti = tmppool.tile([1, 8], I32, tag="rsqi", name=f"rsqi_{u}", bufs=2)
                nc.vector.tensor_scalar(
                    ti[:], veps[:].bitcast(I32), 1, None, ALU.arith_shift_right
                )
                nc.vector.tensor_scalar(ti[:], ti[:], -1, 0x5F3759DF, ALU.mult, ALU.add)
                rstd, hf, nt = t("rstd"), t("hf"), t("nt")
                nc.vector.tensor_copy(rstd[:], ti[:].bitcast(F32))
                nc.vector.tensor_scalar_mul(hf[:], veps[:], 0.5)
                for _ in range(2):
                    nc.vector.tensor_tensor(nt[:], rstd[:], rstd[:], ALU.mult)
                    nc.vector.tensor_tensor(nt[:], nt[:], hf[:], ALU.mult)
                    nc.vector.tensor_scalar(nt[:], nt[:], -1.0, 1.5, ALU.mult, ALU.add)
                    nc.vector.tensor_tensor(rstd[:], rstd[:], nt[:], ALU.mult)
                A, Bc = t("A"), t("Bc")
                nc.vector.tensor_tensor(A[:], rstd[:], gnw_sb[:, 0:8], ALU.mult)
                nc.vector.tensor_tensor(Bc[:], mean, A[:], ALU.mult)
                nc.vector.tensor_tensor(Bc[:], gnb_sb[:, 0:8], Bc[:], ALU.subtract)
                nc.vector.tensor_scalar_mul(A[:], A[:], oml[:, 0:1])
                nc.vector.tensor_scalar_mul(Bc[:], Bc[:], oml[:, 0:1])
                A_rep = tmppool.tile([128, 8], F32, tag="A_rep", name=f"A_rep{u}", bufs=2)
                nc.gpsimd.partition_broadcast(A_rep[:], A[:])
                cb = tmppool.tile([1, 128], F32, tag="cb", name=f"cb_{u}", bufs=2)
                if cb_mm is None:
                    # serial stt chain (fine off the critical path)
                    nc.vector.tensor_scalar_mul(cb[:], wsum_sb[:, 0:128], Bc[:, 0:1])
                    for h3 in range(1, 8):
                        nc.vector.scalar_tensor_tensor(
                            cb[:], wsum_sb[:, 128 * h3: 128 * (h3 + 1)],
                            Bc[:, h3: h3 + 1], cb[:], ALU.mult, ALU.add,
                        )
                    nc.vector.tensor_tensor(cb[:], cb[:], bo_sb[:], ALU.add)
                else:
                    # critical path: cb = Bc(1x8) @ wsum_p8(8x128) via PE
                    # (transpose Bc to a column first), then + bo
                    ps_pool = cb_mm
                    btp = ps_pool.tile([8, 8], F32, tag="btp", name=f"btp_{u}")
                    nc.tensor.matmul(btp[:, 0:1], Bc[:], ones_f32[0:1, 0:1],
                                     is_transpose=True, start=True, stop=True)
                    bcol = tmppool.tile([8, 1], F32, tag="bcol", name=f"bcol_{u}", bufs=2)
                    nc.vector.tensor_copy(bcol[:], btp[:, 0:1])
                    cbp = ps_pool.tile([1, 128], F32, tag="cbp", name=f"cbp_{u}")
                    nc.tensor.matmul(cbp[:], bcol[:], wsum_p8[:], start=True, stop=True)
                    nc.vector.tensor_tensor(cb[:], cbp[:], bo_sb[:], ALU.add)
                cb_rep = tmppool.tile([128, 128], F32, tag="cb_rep", name=f"cbr_{u}", bufs=2)
                nc.gpsimd.partition_broadcast(cb_rep[:], cb[:])
                scal[u] = {"A_rep": A_rep, "cb_rep": cb_rep}

            qk = {}
            vts = []
            wsum_sb = cpool.tile([1, 1024], F32)
            wsum_f8 = cpool.tile([8, 128], F32)
            wsum_p8 = cpool.tile([8, 128], F32)

            # ================= attention-phase PSUM pools =================
            with (
                tc.tile_pool(name="ps_sgrp", bufs=2, space="PSUM") as ps_sgrp,
                tc.tile_pool(name="ps_u", bufs=1, space="PSUM") as ps_u,
                tc.tile_pool(name="ps_rdot", bufs=1, space="PSUM") as ps_rdot,
                tc.tile_pool(name="ps_rrow", bufs=1, space="PSUM") as ps_rrow,
            ):
                def emit_wsum():
                    # Wo column sums (for the GN-beta term), on the sgrp psum ring
                    wps = ps_sgrp.tile([128, 1024], F32, tag="sgrp", name="wps")
                    for h3 in range(8):
                        nc.tensor.matmul(wps[0:1, 128 * h3: 128 * (h3 + 1)],
                                         ones_f32[:, 0:1], wo_bf[h3][:],
                                         start=True, stop=True)
                    nc.vector.tensor_copy(wsum_sb[:, 0:1024], wps[0:1, 0:1024])
                    # row-per-h3 bf16 copy (for the PE-side cb matmul at the tail)
                    nc.sync.dma_start(
                        wsum_f8[:],
                        wsum_sb[:].rearrange("p (a b) -> (p a) b", a=8, b=128))
                    nc.vector.tensor_copy(wsum_p8[:], wsum_f8[:])

                def alloc_qk(u):
                    for nm in ("q1", "q2"):
                        qk[(u, nm)] = [
                            projpool.tile([128, 512], F32R, tag=f"{nm}t",
                                          name=f"{nm}t_{u}_{qb}", bufs=8)
                            for qb in range(4)
                        ]
                    for nm in ("k1", "k2"):
                        qk[(u, nm)] = [
                            projpool.tile([128, 1024], F32R, tag=f"{nm}t",
                                          name=f"{nm}t_{u}_{hh}", bufs=4)
                            for hh in range(2)
                        ]
                    vts.append(projpool.tile([128, 2048], F32R, tag="vp",
                                             name=f"vp_{u}", bufs=2))

                def proj_blocks(u, use_act=False):
                    # generator of closures: 10 psum-ring blocks per unit
                    # (4x q, 4x k with four 256-col chunks each; 2x v halves)
                    def qkblk(blk):
                        def emit():
                            ps = ps_sgrp.tile([128, 1024], F32, tag="sgrp",
                                              name=f"pp_{u}_{blk}")
                            wrh = wq_rh if blk < 4 else wk_rh
                            for c in range(4):
                                j = 4 * (blk % 4) + c
                                nc.tensor.matmul(
                                    ps[:, 256 * c: 256 * (c + 1)],
                                    wrh[j // 8][:, 128 * (j % 8): 128 * (j % 8 + 1)],
                                    qt_r[u][:], start=True, stop=True,
                                )
                            for c in range(4):
                                j = 4 * (blk % 4) + c
                                if blk < 4:
                                    dst = qk[(u, "q1" if j % 2 == 0 else "q2")][j // 4]
                                    col = 256 * ((j // 2) % 2)
                                    bias = bq_sb[:, j: j + 1]
                                else:
                                    dst = qk[(u, "k1" if j % 2 == 0 else "k2")][j // 8]
                                    col = 256 * ((j // 2) % 4)
                                    bias = bk_sb[:, j: j + 1]
                                if use_act and c % 2 == 0:
                                    # head phase: ACT is idle; Identity+bias is
                                    # in every table set (no exp-table thrash)
                                    nc.scalar.activation(
                                        dst[:, col: col + 256],
                                        ps[:, 256 * c: 256 * (c + 1)],
                                        AF.Identity, bias=bias,
                                    )
                                else:
                                    nc.vector.tensor_scalar_add(
                                        dst[:, col: col + 256],
                                        ps[:, 256 * c: 256 * (c + 1)], bias
                                    )
                        return emit

                    def vblk(rc):
                        def emit():
                            vt = vts[u]
                            ps = ps_sgrp.tile([128, 1024], F32, tag="sgrp",
                                              name=f"ppv_{u}_{rc}")
                            for fh in range(2):
                                nc.tensor.matmul(
                                    ps[:, 512 * fh: 512 * (fh + 1)],
                                    qt_r[u][:, 128 * rc: 128 * (rc + 1)],
                                    wv_r[:, 512 * fh: 512 * (fh + 1)],
                                    start=True, stop=True,
                                )
                            for fh in range(2):
                                nc.vector.tensor_tensor(
                                    vt[:, 1024 * rc + 512 * fh:
                                       1024 * rc + 512 * fh + 512],
                                    ps[:, 512 * fh: 512 * (fh + 1)],
                                    bv_rep[:, 512 * fh: 512 * (fh + 1)], ALU.add,
                                )
                        return emit

                    # pre: minimum to start attention qb0 (q j0-3, k j0-7, v);
                    # drip: the rest, fed into attention slots as DMA lands
                    pre = [qkblk(0), qkblk(4), qkblk(5), vblk(0), vblk(1)]
                    drip = [qkblk(6), qkblk(7), qkblk(1), qkblk(2), qkblk(3)]
                    return pre, drip

                def attention_unit(u, boundary_cb, slot_cb):
                    q1l, q2l = qk[(u, "q1")], qk[(u, "q2")]
                    kls = (qk[(u, "k1")], qk[(u, "k2")])
                    vt = vts[u]

                    def vchunk(kc):
                        return vt[:, 1024 * (kc % 2) + 128 * (kc // 2):
                                  1024 * (kc % 2) + 128 * (kc // 2) + 128]

                    fT = tmppool.tile([128, 2048], F32, tag="fT", name=f"fT_{u}")
                    fT_sb.append(fT)
                    p1a = tmppool.tile([128, 16], F32, tag="p1a", name=f"p1a_{u}")
                    p1b = tmppool.tile([128, 16], F32, tag="p1b", name=f"p1b_{u}")

                    LAG = 3  # consume items this many exp-slots behind issue
                    state = {}

                    def emit_r_chain(qb, m):
                        # per-branch: [128,8] dots -> f32 transpose into a
                        # [1,512] psum row -> reciprocal -> partition bcast
                        st = state[qb]
                        r_sb = tmppool.tile([128, 8], F32, tag=f"rsb{m}",
                                            name=f"rsb_{u}_{qb}_{m}", bufs=2)
                        nc.vector.tensor_copy(r_sb[:], st["rdot"][:, 8 * m: 8 * m + 8])
                        rrow = ps_rrow.tile([1, 512], F32, tag="rr",
                                            name=f"rr_{u}_{qb}_{m}")
                        for s4 in range(4):
                            nc.tensor.matmul(
                                rrow[0:1, 128 * s4: 128 * (s4 + 1)],
                                r_sb[:, 2 * s4: 2 * s4 + 1], eye_sb[:],
                                is_transpose=True, start=True, stop=True,
                            )
                        r_inv = tmppool.tile([1, 512], F32, tag=f"rinv{m}",
                                             name=f"rinv_{u}_{qb}_{m}", bufs=1)
                        nc.vector.reciprocal(r_inv[:], rrow[:])
                        rr = tmppool.tile([128, 512], F32, tag=f"r{m}rep",
                                          name=f"r{m}rep_{u}_{qb}", bufs=2)
                        nc.gpsimd.partition_broadcast(rr[:], r_inv[:])
                        st["rrep"][m] = rr

                    def finish_qb(qb):
                        # O = U1/R1 - lam*U2/R2  (t1 emitted early, in-branch)
                        st = state[qb]
                        t2 = tmppool.tile([128, 512], F32, tag="t2", name=f"t2_{u}_{qb}")
                        nc.vector.scalar_tensor_tensor(
                            t2[:], st["u2"][:], lam_rep[:, 0:1], st["rrep"][1][:],
                            ALU.mult, ALU.mult
                        )
                        otq = otpool.tile([128, 512], F32, tag="ot", name=f"ot_{u}_{qb}")
                        nc.vector.tensor_tensor(otq[:], st["t1"][:], t2[:], ALU.subtract)
                        ot_refs[u].append(otq)

                        # incremental GroupNorm partial stats for this q-block
                        # (free-dim layout within the block: (j2, g8, r32))
                        osl = otq.rearrange("p (j g r) -> p j g r", j=2, g=8, r=32)
                        red = tmppool.tile([128, 16], F32, tag="red", name=f"red_{u}_{qb}")
                        nc.vector.tensor_reduce(red[:], osl, mybir.AxisListType.X, ALU.add)
                        if qb == 0:
                            nc.vector.tensor_copy(p1a[:], red[:])
                        else:
                            nc.vector.tensor_tensor(p1a[:], p1a[:], red[:], ALU.add)
                        sq5 = tmppool.tile([128, 512], F32, tag="t1", name=f"sq5_{u}_{qb}")
                        sq_eng = nc.gpsimd if (u == 1 and qb == 3) else nc.vector
                        sq_eng.tensor_tensor(sq5[:], otq[:], otq[:], ALU.mult)
                        redb = tmppool.tile([128, 16], F32, tag="redb", name=f"redb_{u}_{qb}")
                        nc.vector.tensor_reduce(
                            redb[:], sq5.rearrange("p (j g r) -> p j g r", j=2, g=8, r=32),
                            mybir.AxisListType.X, ALU.add,
                        )
                        if qb == 0:
                            nc.vector.tensor_copy(p1b[:], redb[:])
                        else:
                            nc.vector.tensor_tensor(p1b[:], p1b[:], redb[:], ALU.add)
                        if u == 0 or qb < 3:
                            emit_relayout(u, qb)
                        boundary_cb(qb)

                    def consume(item):
                        qb, m, g, eg = item
                        if m == 0 and g == 0:
                            # lazy per-qb psum state: allocated only once the
                            # previous qb's readers are already emitted (FIFO)
                            rdot = ps_rdot.tile([128, 16], F32, tag="rd",
                                                name=f"rd_{u}_{qb}")
                            nc.vector.memset(rdot[:], 0.0)
                            state[qb] = {
                                "u1": ps_u.tile([128, 512], F32, tag="u1",
                                                name=f"u1_{u}_{qb}"),
                                "u2": ps_u.tile([128, 512], F32, tag="u2",
                                                name=f"u2_{u}_{qb}"),
                                "rdot": rdot, "rrep": [None, None], "t1": None,
                            }
                        st = state[qb]
                        uacc = (st["u1"], st["u2"])[m]
                        for c in range(2):
                            kc = 2 * g + c
                            if g == 7:
                                # last group: dots first so the denominator
                                # chain starts before the AV matmuls
                                for sl4 in range(4):
                                    col = 2 * (4 * m + sl4)
                                    nc.tensor.matmul(
                                        st["rdot"][:, col: col + 2],
                                        eg[:, 512 * c + 128 * sl4: 512 * c + 128 * sl4 + 128],
                                        ones_col[:],
                                        start=False, stop=False, skip_group_check=True,
                                    )
                        if g == 7:
                            emit_r_chain(qb, m)
                        for c in range(2):
                            kc = 2 * g + c
                            nc.tensor.matmul(
                                uacc[:], vchunk(kc), eg[:, 512 * c: 512 * (c + 1)],
                                start=(g == 0 and c == 0),
                                stop=(g == 7 and c == 1),
                            )
                            if g != 7:
                                for sl4 in range(4):
                                    col = 2 * (4 * m + sl4)
                                    nc.tensor.matmul(
                                        st["rdot"][:, col: col + 2],
                                        eg[:, 512 * c + 128 * sl4: 512 * c + 128 * sl4 + 128],
                                        ones_col[:],
                                        start=False, stop=False, skip_group_check=True,
                                    )
                        if g == 7:
                            if m == 0:
                                # t1 = U1/R1 early, while branch 2 streams
                                t1 = tmppool.tile([128, 512], F32, tag="t1",
                                                  name=f"t1_{u}_{qb}")
                                nc.vector.tensor_tensor(t1[:], st["u1"][:],
                                                        st["rrep"][0][:], ALU.mult)
                                st["t1"] = t1
                            else:
                                finish_qb(qb)

                    # flat 64-slot pipeline: the sgrp/exp stream never pauses
                    # at qb boundaries; qb bookkeeping rides inside the lagged
                    # consume stream
                    pending = []
                    for slot in range(64):
                        qb, rem = divmod(slot, 16)
                        m, g = divmod(rem, 8)
                        ql = (q1l, q2l)[m][qb]
                        kl = kls[m]
                        sg = ps_sgrp.tile([128, 1024], F32, tag="sgrp",
                                          name=f"sg_{u}_{qb}_{m}_{g}")
                        for c in range(2):
                            kc = 2 * g + c
                            nc.tensor.matmul(
                                sg[:, 512 * c: 512 * (c + 1)],
                                kl[kc // 8][:, 128 * (kc % 8): 128 * (kc % 8 + 1)],
                                ql[:], start=True, stop=True,
                            )
                        eg = epool.tile([128, 1024], F32R, tag="e",
                                        name=f"e_{u}_{qb}_{m}_{g}")
                        nc.scalar.activation(eg[:], sg[:], AF.Exp)
                        pending.append((qb, m, g, eg))
                        if len(pending) > LAG:
                            consume(pending.pop(0))
                        slot_cb(qb, rem)
                    for item in pending:
                        consume(item)

                    # fold (j mod 2) pairs -> per-group partials
                    for si, p1x in enumerate((p1a, p1b)):
                        p2 = tmppool.tile([128, 8], F32, tag="p2", name=f"p2_{u}_{si}")
                        nc.vector.tensor_reduce(
                            p2[:], p1x.rearrange("p (j g) -> p g j", j=2, g=8),
                            mybir.AxisListType.X, ALU.add,
                        )
                        p2_tiles[u].append(p2)

                def emit_relayout(u, qb):
                    # re-layout into fT (j-pair slab for this qb);
                    # src re-viewed g-outer to match the dst iteration order
                    fT = fT_sb[u]
                    fv4 = fT.rearrange("p (g j r) -> p g j r", g=8, j=8, r=32)
                    otq = ot_refs[u][qb]
                    nc.vector.tensor_copy(
                        fv4[:, :, 2 * qb: 2 * qb + 2, :],
                        otq.rearrange("p (j g r) -> p g j r", j=2, g=8, r=32),
                    )

                # ================= main schedule =================
                alloc_qk(0)
                alloc_qk(1)
                pre0, _ = proj_blocks(0, use_act=True)
                for blk in pre0:
                    blk()
                _, drip0 = proj_blocks(0)
                pre1, drip1 = proj_blocks(1)
                drip = drip0 + pre1 + drip1  # u0 stragglers first, then all of u1
                # slots at which to emit one proj block into u0's attention:
                # k j8-15 blocks early (needed by qb0 slot 4/6), then every 3rd
                drip_slots = [3, 5] + list(range(8, 64, 3))

                def u0_slot(qb, slot):
                    g = 16 * qb + slot
                    if drip and drip_slots and g >= drip_slots[0]:
                        drip_slots.pop(0)
                        drip.pop(0)()

                attention_unit(0, lambda qb: None, u0_slot)
                while drip:
                    drip.pop(0)()

                def u1_boundary(qb):
                    if qb == 0:
                        # unit-0 stats export + collective #1: lands on the Pool
                        # queue right after qb0's broadcasts; PE has a full qb of
                        # queued work to ride out the 15us Pool block.
                        emit_stats_export(0)
                        emit_wo_copies()
                        emit_wsum()
                    if qb == 2:
                        # unit-0 GN scalars (collective #1 landed long ago)
                        emit_scalars(0)

                attention_unit(1, u1_boundary, lambda qb, slot: None)
                # stats export for unit 1: as early as possible -> tail collective
                emit_stats_export(1)

            # ============== output stage (fills collective #2 window) ==============
            with tc.tile_pool(name="ps_out", bufs=1, space="PSUM") as ps_out:
                emit_relayout(1, 3)

                P_sb = {}

                def emit_partials(u):
                    # 4 P outputs packed per [128,512] psum bank tile, then
                    # drained to SBUF (cheap to re-read; fills collective #2)
                    fT = fT_sb[u]
                    for rh in range(2):
                        for h4 in range(2):
                            pps = ps_out.tile([128, 512], F32, tag="P", bufs=4,
                                              name=f"pps_{u}_{rh}_{h4}")
                            for hq in range(4):
                                h3 = 4 * h4 + hq
                                lhsT = fT[:, 256 * h3 + 128 * rh: 256 * h3 + 128 * rh + 128]
                                nc.tensor.matmul(pps[:, 128 * hq: 128 * (hq + 1)],
                                                 lhsT, wo_bf[h3][:], start=True, stop=True)
                            P_sb[(u, rh, h4)] = pps

                def emit_combine(u, use_pool):
                    # result = sum_h3 A[u,h3]*P[u][rh][h3] + cb_rep[u]
                    # split per rh into a DVE half-chain (h3 0-3) and a Pool
                    # half-chain (h3 4-7), joined by one add
                    A_rep = scal[u]["A_rep"]
                    cb_rep = scal[u]["cb_rep"]
                    accs = {}
                    for rh in range(2):
                        for h4 in range(2):
                            psb = P_sb[(u, rh, h4)]
                            acc = tmppool.tile([128, 128], F32, tag=f"acc{h4}",
                                               name=f"acc_{u}_{rh}_{h4}")
                            if False:
                                # Pool path: tsm x4 + tt tree (no stt on Pool)
                                sc = tmppool.tile([128, 512], F32, tag="pscl",
                                                  name=f"pscl_{u}_{rh}", bufs=2)
                                for hq in range(4):
                                    nc.gpsimd.tensor_scalar_mul(
                                        sc[:, 128 * hq: 128 * (hq + 1)],
                                        psb[:, 128 * hq: 128 * (hq + 1)],
                                        A_rep[:, 4 * h4 + hq: 4 * h4 + hq + 1])
                                nc.gpsimd.tensor_tensor(
                                    sc[:, 0:128], sc[:, 0:128], sc[:, 128:256], ALU.add)
                                nc.gpsimd.tensor_tensor(
                                    sc[:, 256:384], sc[:, 256:384], sc[:, 384:512], ALU.add)
                                nc.gpsimd.tensor_tensor(
                                    acc[:], sc[:, 0:128], sc[:, 256:384], ALU.add)
                            else:
                                nc.vector.tensor_scalar_mul(
                                    acc[:], psb[:, 0:128], A_rep[:, 4 * h4: 4 * h4 + 1]
                                )
                                for hq in range(1, 4):
                                    h3 = 4 * h4 + hq
                                    nc.vector.scalar_tensor_tensor(
                                        acc[:], psb[:, 128 * hq: 128 * (hq + 1)],
                                        A_rep[:, h3: h3 + 1], acc[:],
                                        ALU.mult, ALU.add,
                                    )
                            accs[(rh, h4)] = acc
                    for rh in range(2):
                        rsb = tmppool.tile([128, 128], F32, tag="rsb2",
                                           name=f"rsb_{u}_{rh}")
                        nc.vector.tensor_tensor(rsb[:], accs[(rh, 0)][:],
                                                accs[(rh, 1)][:], ALU.add)
                        nc.vector.tensor_tensor(rsb[:], rsb[:], cb_rep[:], ALU.add)
                        # contiguous block write; host undoes the row permutation
                        # (device row 128*rh+m'' holds rho = 8*(m''%32)+4*rh+m''//32)
                        nc.sync.dma_start(outp[u][128 * rh: 128 * (rh + 1), :], rsb[:])

                emit_partials(0)
                emit_combine(0, use_pool=False)  # fill work; Pool is blocked by collective #2
                emit_partials(1)
                emit_scalars(1)       # waits on collective #2 (hidden behind fill work)
                emit_combine(1, use_pool=True)

    nc.compile()
    return nc


def _prep_inputs(inputs):
    """Host-side: slice/transpose full inputs into per-core in_maps."""
    query = np.asarray(inputs["query"], np.float32)
    Wq = np.asarray(inputs["Wq"], np.float32)
    Wk = np.asarray(inputs["Wk"], np.float32)
    Wv = np.asarray(inputs["Wv"], np.float32)
    Wo = np.asarray(inputs["Wo"], np.float32)
    bq = np.asarray(inputs["bq"], np.float32)
    bk = np.asarray(inputs["bk"], np.float32)
    bv = np.asarray(inputs["bv"], np.float32)
    bo = np.asarray(inputs["bo"], np.float32)
    gn_w = np.asarray(inputs["gn_w"], np.float32)
    gn_b = np.asarray(inputs["gn_b"], np.float32)
    lam = np.asarray(inputs["lam"], np.float32).reshape(1, 1)

    shared = {
        "wqT": np.ascontiguousarray(Wq.T),
        "wkT": np.ascontiguousarray(Wk.T),
        "wvT": np.ascontiguousarray(Wv.T),
        "woT": np.ascontiguousarray(Wo.T),
        "bqT": np.ascontiguousarray(bq.reshape(16, 128).T),
        "bkT": np.ascontiguousarray(bk.reshape(16, 128).T),
        "bv": bv.reshape(1, 1024),
        "bo": bo.reshape(1, 128),
        "gnw2": np.tile(gn_w, 2).reshape(1, 16),
        "gnb2": np.tile(gn_b, 2).reshape(1, 16),
        "lam": lam,
        "eye": np.eye(128, dtype=np.float32),
    }
    in_maps = []
    for c in range(N_CORES):
        blk = query[:, 256 * c: 256 * (c + 1), :]  # [B, 256, 128]
        qT = np.ascontiguousarray(blk.transpose(0, 2, 1))  # [B, 128, 256]
        in_maps.append({"qT": qT, **shared})
    return in_maps


class _Runner:
    """Cached-jit SPMD executor (one trace/compile; cheap repeated calls)."""

    def __init__(self, nc):
        import jax
        from jax.sharding import Mesh, PartitionSpec
        from jax.experimental.shard_map import shard_map
        from concourse.bass2jax import (
            install_neuronx_cc_hook, _bass_exec_p, partition_id_tensor,
        )

        install_neuronx_cc_hook()
        self.jax = jax
        pname = nc.partition_id_tensor.name if nc.partition_id_tensor else None
        in_names, out_names, out_avals, zero_outs = [], [], [], []
        for alloc in nc.m.functions[0].allocations:
            if not isinstance(alloc, mybir.MemoryLocationSet):
                continue
            name = alloc.memorylocations[0].name
            if alloc.kind == "ExternalInput":
                if name != pname:
                    in_names.append(name)
            elif alloc.kind == "ExternalOutput":
                out_names.append(name)
                shape = tuple(alloc.tensor_shape)
                dtype = mybir.dt.np(alloc.dtype)
                out_avals.append(jax.core.ShapedArray(shape, dtype))
                zero_outs.append(np.zeros(shape, dtype))
        self.in_names, self.out_names = in_names, out_names
        n_params = len(in_names)
        all_names = list(in_names) + out_names
        if pname is not None:
            all_names.append(pname)

        def _body(*args):
            operands = list(args)
            if pname is not None:
                operands.append(partition_id_tensor())
            return tuple(_bass_exec_p.bind(
                *operands, out_avals=tuple(out_avals), in_names=tuple(all_names),
                out_names=tuple(out_names), lowering_input_output_aliases=(),
                sim_require_finite=True, sim_require_nnan=True, nc=nc))

        devices = jax.devices()[:N_CORES]
        mesh = Mesh(np.asarray(devices), ("core",))
        nio = n_params + len(out_names)
        self.fn = jax.jit(
            shard_map(_body, mesh=mesh, in_specs=(PartitionSpec("core"),) * nio,
                      out_specs=(PartitionSpec("core"),) * len(out_names),
                      check_rep=False),
            keep_unused=True,
        )
        self.zeros = [
            jax.device_put(np.zeros((N_CORES * z.shape[0], *z.shape[1:]), z.dtype))
            for z in zero_outs
        ]
        self.out_shapes = [tuple(a.shape) for a in out_avals]

    def run(self, in_maps):
        cat = [
            np.concatenate([np.asarray(m[n]) for m in in_maps], axis=0)
            for n in self.in_names
        ]
        # the accelerator intermittently throws a transient
        # NRT_EXEC_UNIT_UNRECOVERABLE (status 101); retry once
        for attempt in range(3):
            try:
                outs = self.fn(*cat, *self.zeros)
                self.jax.block_until_ready(outs)
                outs = [np.asarray(o) for o in outs]
                break
            except Exception:
                if attempt == 2:
                    raise
                import time as _t
                _t.sleep(5.0)
        return [
            {n: outs[i].reshape(N_CORES, *self.out_shapes[i])[c]
             for i, n in enumerate(self.out_names)}
            for c in range(N_CORES)
        ]


_CACHED_NC = None


def kernel(**inputs) -> np.ndarray:
    global _CACHED, _CACHED_NC
    if _CACHED is None:
        _CACHED_NC = build_nc()
        _CACHED = _Runner(_CACHED_NC)
    in_maps = _prep_inputs(inputs)
    results = _CACHED.run(in_maps)
    # device row (rh, m'') holds output row rho = 8*(m'' % 32) + 4*rh + m''//32
    mpp = np.arange(128)
    rho = np.concatenate([8 * (mpp % 32) + 4 * rh + mpp // 32 for rh in (0, 1)])
    inv = np.argsort(rho)
    out = np.empty((B, S, H * D // 8), np.float32)  # (2, 2048, 128)
    for c in range(N_CORES):
        o = results[c]["outp"]  # [B, 256, 128] in device (rh, m'') row order
        for b in range(B):
            out[b, c::8, :] = o[b][inv]  # rows s3 = 8*rho + c
    return out
